# revision 1
# baseline (speedup 1.0000x reference)
"""Trainium2 Bass kernel for nn_CachedSpectralGPSLayer (8-core SPMD).

Self-contained: takes FULL inputs, shards per-core internally, runs one
Bass/Tile program SPMD on 8 NeuronCores, gathers the full output.
"""
import sys

sys.path.insert(0, "/opt/trn_rl_repo")

import numpy as np
import ml_dtypes

import concourse.bacc as bacc
import concourse.bass as bass
import concourse.mybir as mybir
import concourse.tile as tile
from concourse import library_config
from concourse.bass_utils import run_bass_kernel_spmd

BF16 = ml_dtypes.bfloat16
FP8 = ml_dtypes.float8_e4m3
F32 = np.float32

N, C, K, KEIG, B, NG, H = 32768, 128, 5, 128, 64, 512, 4
NCORES = 8
NLOC = N // NCORES          # 4096
NT = NLOC // 128            # 32 node tiles per core
Bd = 64                     # dst nodes per block
NBLK = NLOC // Bd           # 64 blocks per core
GPC = B // NCORES           # 8 graphs per core
DH = C // H                 # 32
EPS = 1e-5
BPG = 4                     # blocks per dma_gather call
NCALL = NBLK // BPG         # 16 gather calls per hop
FP8V = False                # gather/AllGather payload (v~) in fp8e4m3
VSCALE = 16.0               # v~ pre-scale (keeps fp8 out of subnormals)

fp32 = mybir.dt.float32
bf16 = mybir.dt.bfloat16
fp8 = mybir.dt.float8e4
i16 = mybir.dt.int16

# bias-pack column indices
(BI_SPA1, BI_SPA2, BI_SPE1, BI_SPE2, BI_Q, BI_K, BI_OUTP, BI_CHEB,
 BI_M1A, BI_M1B, BI_M2, BI_BN1W, BI_BN1B, BI_BN2W, BI_BN2B, BI_BN3W,
 BI_BN3B, BI_EPSC) = range(18)
NBIAS = 18

_CACHE = {}


def _wrap_idx(idx_flat):
    """dma_gather wrapped layout per call: idx i -> [i%16, i//16], replicated
    to all 8 groups of 16 partitions. idx_flat: [ncalls, n_per_call]."""
    ncalls, npc = idx_flat.shape
    base = idx_flat.reshape(ncalls, npc // 16, 16).transpose(0, 2, 1)  # [ncalls,16,npc/16]
    out = np.tile(base, (1, 8, 1))                                     # [ncalls,128,npc/16]
    return np.concatenate(list(out), axis=1)                           # [128, ncalls*npc/16]


def _preprocess(inputs):
    src = np.asarray(inputs["edge_index"][0]).astype(np.int64)
    dst = np.asarray(inputs["edge_index"][1]).astype(np.int64)
    deg = np.bincount(src, minlength=N).astype(np.float64)
    dis = np.where(deg > 0, 1.0 / np.sqrt(deg), 0.0).astype(F32)
    lam = float(np.asarray(inputs["lambda_max"]).reshape(-1)[0])
    scale = 2.0 / lam

    order = np.argsort(dst, kind="stable")
    srcs, dsts = src[order], dst[order]
    counts = np.bincount(dst // Bd, minlength=N // Bd)
    ngrp = int(np.ceil(counts.max() / 128))
    epb = ngrp * 128                    # padded edges per block
    epad = NBLK * epb                   # per core

    cores = []
    bounds = np.searchsorted(dsts, np.arange(0, N + 1, NLOC))
    for c in range(NCORES):
        lo = c * NLOC
        sl = slice(bounds[c], bounds[c + 1])
        sc, dc = srcs[sl], dsts[sl] - lo
        blk = dc // Bd
        cnt = np.bincount(blk, minlength=NBLK)
        csum = np.concatenate([[0], np.cumsum(cnt)])
        pos_in_blk = np.arange(len(sc)) - csum[blk]
        slot = blk * epb + pos_in_blk
        src_pad = np.zeros(epad, np.int64)
        src_pad[slot] = sc
        # S one-hot fp8 bytes [128, ngroups*Bd]
        ngroups = NBLK * ngrp
        s8 = np.zeros((128, ngroups * Bd), np.uint8)
        g = slot // 128
        p = slot % 128
        s8[p, g * Bd + (dc - blk * Bd)] = 0x38  # fp8e4m3 1.0
        assert src_pad.max() < 2 ** 15
        gidx = _wrap_idx(src_pad.astype(np.int16).reshape(NCALL, BPG * epb))
        cores.append(dict(s8=s8.view(FP8), gidx=gidx))
    return cores, dis, scale, ngrp


def _build(ngrp, scale, hops=K - 1, do_mha=True, do_spec=True):
    do_spec = True
    """Build + compile the SPMD Bass program. Returns (nc, input names)."""
    epb = ngrp * 128
    epad = NBLK * epb
    ngroups = NBLK * ngrp

    nc = bacc.Bacc("TRN2", target_bir_lowering=False, debug=False,
                   enable_asserts=True, num_devices=NCORES)

    def din(name, shape, dt):
        return nc.dram_tensor(name, shape, dt, kind="ExternalInput").ap()

    xT = din("xT", [128, NLOC], fp32)
    u_nm = din("u_nm", [NLOC, 128], fp32)
    usT = din("usT", [128, NLOC], fp32)
    disc = din("disc", [128, NT], fp32)
    disc_m = din("disc_m", [128, NT], fp32)
    gidx = din("gidx", [128, epad // 16], i16)
    s8 = din("s8", [128, ngroups * Bd], fp8)
    wspa1 = din("wspa1", [128, 128], fp32)
    wspa2 = din("wspa2", [128, 128], fp32)
    wspe1 = din("wspe1", [128, 128], fp32)
    wspe2 = din("wspe2", [128, 128], fp32)
    wproj = din("wproj", [128, 128], fp32)
    chebw = din("chebw", [128, K * 128], fp32)
    wq = din("wq", [128, 128], fp32)
    wk = din("wk", [128, 128], fp32)
    wv = din("wv", [128, 128], fp32)
    wout = din("wout", [128, 128], fp32)
    mw1 = din("mw1", [128, 256], fp32)
    mw2a = din("mw2a", [128, 128], fp32)
    mw2b = din("mw2b", [128, 128], fp32)
    biasp = din("biasp", [128, NBIAS], fp32)
    bqh = din("bqh", [32, H], fp32)
    bkh = din("bkh", [32, H], fp32)
    idm = din("idm", [128, 128], fp32)
    gmat = din("gmat", [128, 128], fp32)
    u1col = din("u1col", [128, 1], fp32)
    us_nm = din("us_nm", [NLOC, 128], fp32)
    onesf = din("onesf", [128, 1], fp32)
    one1 = din("one1", [1, 1], fp32)
    m1x = din("m1x", [128, 128], fp32)
    xfT = din("xfT", [128, N], fp32)
    discf = din("discf", [128, N // 128], fp32)

    out_nm = nc.dram_tensor("out_nm", [NLOC, 128], fp32, kind="ExternalOutput").ap()

    AF = mybir.ActivationFunctionType
    OP = mybir.AluOpType

    with tile.TileContext(nc) as tc:
        with tc.tile_pool(name="const", bufs=1) as cp, \
             tc.tile_pool(name="big", bufs=1) as bp, \
             tc.tile_pool(name="work", bufs=2) as wp, \
             tc.tile_pool(name="psmm", bufs=2, space="PSUM") as psmm, \
             tc.tile_pool(name="psat", bufs=2, space="PSUM") as psatp, \
             tc.tile_pool(name="pspt", bufs=2, space="PSUM") as psptp, \
             tc.tile_pool(name="psagg", bufs=2, space="PSUM") as psaggp, \
             tc.tile_pool(name="dram", bufs=1, space="DRAM") as dp:

            # uniform-tag psum allocators (PSUM = 8 banks total: 2+2+2+2)
            def PS_MM():   # transient [128,512] matmul outputs
                return psmm.tile([128, 512], fp32, tag="mm", name="psmm_t")

            def PS_AT(shape):  # long-lived accumulators / phase-2 partial
                return psatp.tile(shape, fp32, tag="at", name="psat_t")

            def PS_ATW():  # MHA attn+denom accumulator [128, 132]
                return psatp.tile([128, 33 * H], fp32, tag="at", name="psatw_t")

            def PS_PT2():  # [32, 512] head q/k psum
                return psptp.tile([32, 512], fp32, tag="pt", name="pspt2_t")

            def PS_PT():   # [128,128] transposes / small matmuls
                return psptp.tile([128, 128], fp32, tag="pt", name="pspt_t")

            def PS_AGG():  # [128,128] cheb aggregation
                return psaggp.tile([128, 128], fp32, tag="agg", name="psagg_t")

            nc.gpsimd.load_library(library_config.mlp)

            # ---- load constants ----
            def ld(ap_in, shape, dt, name, eng=None):
                t = cp.tile(shape, dt, tag=name, name=name)
                (eng or nc.sync).dma_start(t[:], ap_in[:])
                return t

            xT_s = bp.tile([128, NLOC], fp32, tag="xT")
            nc.sync.dma_start(xT_s[:], xT[:])
            s8_s = ld(s8, [128, ngroups * Bd], fp8, "s8")
            wspa1_s = ld(wspa1, [128, 128], fp32, "wspa1")
            wspa2_s = ld(wspa2, [128, 128], fp32, "wspa2")
            wspe1_s = ld(wspe1, [128, 128], fp32, "wspe1")
            wspe2_s = ld(wspe2, [128, 128], fp32, "wspe2")
            wproj_s = ld(wproj, [128, 128], fp32, "wproj")
            chebw_s = ld(chebw, [128, K * 128], fp32, "chebw")
            wq_s = ld(wq, [128, 128], fp32, "wq")
            wk_s = ld(wk, [128, 128], fp32, "wk")
            wv_s = ld(wv, [128, 128], fp32, "wv")
            wout_s = ld(wout, [128, 128], fp32, "wout")
            mw1_s = ld(mw1, [128, 256], fp32, "mw1")
            mw2a_s = ld(mw2a, [128, 128], fp32, "mw2a")
            mw2b_s = ld(mw2b, [128, 128], fp32, "mw2b")
            biasp_s = ld(biasp, [128, NBIAS], fp32, "biasp")
            bqh_s = ld(bqh, [32, H], fp32, "bqh")
            bkh_s = ld(bkh, [32, H], fp32, "bkh")
            idm_s = ld(idm, [128, 128], fp32, "idm")
            gmat_s = ld(gmat, [128, 128], fp32, "gmat")
            u1col_s = ld(u1col, [128, 1], fp32, "u1col")
            onesf_s = ld(onesf, [128, 1], fp32, "onesf")
            one1_s = ld(one1, [1, 1], fp32, "one1")
            m1x_s = ld(m1x, [128, 128], fp32, "m1x")
            m1sb = cp.tile([128, 128], fp32, tag="m1sb", name="m1sb")
            discf_s = ld(discf, [128, N // 128], fp32, "discf")
            disc_s = ld(disc, [128, NT], fp32, "disc")
            discm_s = ld(disc_m, [128, NT], fp32, "discm")

            def bcol(i):
                return biasp_s[:, i:i + 1]

            # persistent big buffers
            TxA = bp.tile([128, NT, 128], fp32, tag="TxA")   # node-major
            TxB = bp.tile([128, NT, 128], fp32, tag="TxB")
            chebT = bp.tile([128, NLOC], fp32, tag="chebT")  # later: pre1, outT
            h2T = bp.tile([128, NLOC], fp32, tag="h2T")      # later: pre2, out2T
            if not do_mha:
                nc.vector.memset(h2T[:], 0.0)
            statc = cp.tile([128, 32], fp32, tag="statc")    # stats/affine cols
            statc2 = cp.tile([128, 16], fp32, tag="statc2")  # per-graph h2 stats
            outnm = TxA  # reuse (dead after cheb)

            # DRAM bounce buffers
            vdt = fp8 if FP8V else bf16
            ag_in = dp.tile([NLOC, 128], vdt, tag="ag_in")
            ag_outs = [None] + [dp.tile([N, 128], vdt, tag=f"ag_out{h}", name=f"ag_out{h}", addr_space="Shared") for h in range(1, 4)]
            vfull0 = dp.tile([N, 128], vdt, tag="vfull0", name="vfull0")
            bn12_in = dp.tile([128, 260], fp32, tag="bn12_in")
            bn12_out = dp.tile([128, 260], fp32, tag="bn12_out", addr_space="Shared")
            bn3_in = dp.tile([128, 2], fp32, tag="bn3_in")
            bn3_out = dp.tile([128, 2], fp32, tag="bn3_out", addr_space="Shared")

            CH = NLOC // 512  # 8 chunks of 512

            from contextlib import ExitStack
            ep_stack = ExitStack()
            ep = ep_stack.enter_context(tc.tile_pool(name="early", bufs=1))
            xspT = ep.tile([128, NLOC], fp32, tag="xspT", name="xspT")

            # ================= Phase 1: local spa MLP (feature-major) =======
            # (the spe MLP + spectral partial run inside AllGather window 1,
            # recomputed from the resident xT_s, off the pre-hop critical path)
            for ch in range(CH):
                sl = slice(ch * 512, (ch + 1) * 512)
                p1 = PS_MM()
                nc.tensor.matmul(p1[:], lhsT=wspa1_s[:], rhs=xT_s[:, sl],
                                 start=True, stop=True)
                t1 = wp.tile([128, 512], fp32, tag="t1")
                nc.scalar.activation(t1[:], p1[:], AF.Relu, bias=bcol(BI_SPA1))
                p2 = PS_MM()
                nc.tensor.matmul(p2[:], lhsT=wspa2_s[:], rhs=t1[:],
                                 start=True, stop=True)
                nc.scalar.activation(xspT[:, sl], p2[:], AF.Identity,
                                     bias=bcol(BI_SPA2))

            # Tx0 node-major (local shard, for recurrence)
            for t in range(NT):
                tsl = slice(t * 128, (t + 1) * 128)
                pt = PS_PT()
                nc.tensor.transpose(pt[:], xspT[:, tsl], idm_s[:])
                nc.vector.tensor_copy(TxB[:, t, :], pt[:])
            # v~0 for ALL nodes computed locally (replaces hop-1 AllGather):
            # every core redundantly runs the spatial MLP over the full x.
            if hops >= 1:
                for gch in range(N // 512):
                    gsl2 = slice(gch * 512, (gch + 1) * 512)
                    xc = wp.tile([128, 512], fp32, tag="t1")
                    nc.sync.dma_start(xc[:], xfT[:, gsl2])
                    pf1 = PS_MM()
                    nc.tensor.matmul(pf1[:], lhsT=wspa1_s[:], rhs=xc[:],
                                     start=True, stop=True)
                    tf1 = wp.tile([128, 512], fp32, tag="mida")
                    nc.scalar.activation(tf1[:], pf1[:], AF.Relu,
                                         bias=bcol(BI_SPA1))
                    pf2 = PS_MM()
                    nc.tensor.matmul(pf2[:], lhsT=wspa2_s[:], rhs=tf1[:],
                                     start=True, stop=True)
                    spf = wp.tile([128, 512], fp32, tag="t1")
                    nc.scalar.activation(spf[:], pf2[:], AF.Identity,
                                         bias=bcol(BI_SPA2))
                    vt4 = wp.tile([128, 4, 128], vdt, tag="vt4", bufs=2)
                    for j in range(4):
                        tg = gch * 4 + j
                        ptf = PS_PT()
                        nc.tensor.transpose(ptf[:], spf[:, j * 128:(j + 1) * 128],
                                            idm_s[:])
                        nc.scalar.activation(vt4[:, j, :], ptf[:], AF.Identity,
                                             scale=discf_s[:, tg:tg + 1])
                    nc.sync.dma_start(
                        vfull0[gch * 512:(gch + 1) * 512, :]
                        .rearrange("(t p) c -> p t c", p=128), vt4[:])

            for ch in range(CH):
                sl = slice(ch * 512, (ch + 1) * 512)
                pw = PS_MM()
                nc.tensor.matmul(pw[:], lhsT=chebw_s[:, 0:128], rhs=xspT[:, sl],
                                 start=True, stop=True)
                nc.vector.tensor_copy(chebT[:, sl], pw[:])
            # T_0 = Tx0^T Us (accumulate over tiles); m1sb = m1x + W0^T T_0
            t_ps = PS_AGG()
            for t0 in range(0, NT, 4):
                ub = wp.tile([128, 4, 128], fp32, tag="ut4", bufs=1)
                nc.scalar.dma_start(
                    ub[:], us_nm[t0 * 128:(t0 + 4) * 128, :]
                    .rearrange("(t p) c -> p t c", p=128))
                for j in range(4):
                    t = t0 + j
                    nc.tensor.matmul(t_ps[:], lhsT=TxB[:, t, :], rhs=ub[:, j, :],
                                     start=(t == 0), stop=(t == NT - 1))
            tsb = wp.tile([128, 128], fp32, tag="tsb")
            nc.vector.tensor_copy(tsb[:], t_ps[:])
            pWt = PS_PT()
            nc.tensor.matmul(pWt[:], lhsT=chebw_s[:, 0:128], rhs=tsb[:],
                             start=True, stop=True)
            nc.vector.tensor_add(m1sb[:], m1x_s[:], pWt[:])

            # ===== Phase 2 (deferred): spectral partial, run in AG window 1 =
            # Recomputes the spe MLP chunk-wise from resident xT_s so xspecT
            # needs no SBUF residency across the hops.
            part_s = wp.tile([128, 128], fp32, tag="part_s", bufs=1)

            def spectral_partial():
                part_ps = PS_AT([128, 128])
                for ch2 in range(CH):
                    sl2 = slice(ch2 * 512, (ch2 + 1) * 512)
                    p3 = PS_MM()
                    nc.tensor.matmul(p3[:], lhsT=wspe1_s[:], rhs=xT_s[:, sl2],
                                     start=True, stop=True)
                    t2 = wp.tile([128, 512], fp32, tag="t1")
                    nc.scalar.activation(t2[:], p3[:], AF.Relu,
                                         bias=bcol(BI_SPE1))
                    p4 = PS_MM()
                    nc.tensor.matmul(p4[:], lhsT=wspe2_s[:], rhs=t2[:],
                                     start=True, stop=True)
                    xsp2 = wp.tile([128, 512], fp32, tag="t1")
                    nc.scalar.activation(xsp2[:], p4[:], AF.Identity,
                                         bias=bcol(BI_SPE2))
                    ub = wp.tile([128, 4, 128], fp32, tag="ut4", bufs=1)
                    nc.sync.dma_start(
                        ub[:], u_nm[ch2 * 512:(ch2 + 1) * 512, :]
                        .rearrange("(t p) c -> p t c", p=128))
                    for j in range(4):
                        t = ch2 * 4 + j
                        ph = PS_PT()
                        nc.tensor.matmul(ph[:], lhsT=xsp2[:, j * 128:(j + 1) * 128],
                                         rhs=wproj_s[:], start=True, stop=True)
                        hp = wp.tile([128, 128], fp32, tag="hp")
                        nc.vector.tensor_copy(hp[:], ph[:])
                        nc.tensor.matmul(part_ps[:], lhsT=ub[:, j, :], rhs=hp[:],
                                         start=(t == 0), stop=(t == NT - 1))
                nc.vector.tensor_copy(part_s[:], part_ps[:])

            ep_stack.close()  # free xspT space for later pools
            late_stack = ExitStack()
            gp = late_stack.enter_context(tc.tile_pool(name="gath", bufs=2))
            mp = late_stack.enter_context(tc.tile_pool(name="mha", bufs=2))

            # ---- MHA for one graph (interleaved into AllGather windows) ----
            def mha_graph(g):
                gsl = slice(g * 512, (g + 1) * 512)
                # head-major q/k: per-head matmuls so all operands are base-0
                qT = mp.tile([32, H * 512], bf16, tag="qT", bufs=1)
                kT = mp.tile([32, H * 512], bf16, tag="kT", bufs=1)
                for hh in range(H):
                    csl = slice(hh * 32, (hh + 1) * 32)
                    pqh = PS_PT2()
                    nc.tensor.matmul(pqh[:], lhsT=wq_s[:, csl],
                                     rhs=xT_s[:, gsl], start=True, stop=True)
                    nc.scalar.activation(qT[:, hh * 512:(hh + 1) * 512], pqh[:],
                                         AF.Identity, bias=bqh_s[:, hh:hh + 1])
                    pkh = PS_PT2()
                    nc.tensor.matmul(pkh[:], lhsT=wk_s[:, csl],
                                     rhs=xT_s[:, gsl], start=True, stop=True)
                    nc.scalar.activation(kT[:, hh * 512:(hh + 1) * 512], pkh[:],
                                         AF.Identity, bias=bkh_s[:, hh:hh + 1])
                # v node-major, augmented per head with a ones column
                vaug = mp.tile([128, 4, 33 * H], bf16, tag="vaug")
                nc.vector.memset(vaug[:, :, 32::33], 1.0)
                for j in range(4):
                    pv = PS_PT()
                    nc.tensor.matmul(pv[:], lhsT=xT_s[:, g * 512 + j * 128:
                                                      g * 512 + (j + 1) * 128],
                                     rhs=wv_s[:], start=True, stop=True)
                    for hh in range(H):
                        nc.vector.tensor_copy(
                            vaug[:, j, hh * 33:hh * 33 + 32],
                            pv[:, hh * 32:(hh + 1) * 32])
                # scores_T + exp, per (head, k-chunk)
                ess = {}
                for hh in range(H):
                    qsl = slice(hh * 512, (hh + 1) * 512)
                    for j in range(4):
                        pss = PS_MM()
                        nc.tensor.matmul(
                            pss[:], lhsT=kT[:, hh * 512 + j * 128:
                                            hh * 512 + (j + 1) * 128],
                            rhs=qT[:, qsl], start=True, stop=True)
                        es = mp.tile([128, 512], bf16, tag="es", bufs=16)
                        nc.scalar.activation(es[:], pss[:], AF.Exp)
                        ess[(hh, j)] = es
                # attn + denom per q-chunk
                for qc in range(4):
                    pat = PS_ATW()
                    for hh in range(H):
                        for j in range(4):
                            nc.tensor.matmul(
                                pat[:, hh * 33:(hh + 1) * 33],
                                lhsT=ess[(hh, j)][:, qc * 128:(qc + 1) * 128],
                                rhs=vaug[:, j, hh * 33:(hh + 1) * 33],
                                start=(j == 0), stop=(j == 3),
                                skip_group_check=True)
                    recip = wp.tile([128, 4], fp32, tag="recip")
                    nc.vector.reciprocal(recip[:], pat[:, 32::33])
                    anm = wp.tile([128, 128], fp32, tag="anm")
                    for hh in range(H):
                        nc.vector.tensor_scalar(
                            out=anm[:, hh * 32:(hh + 1) * 32],
                            in0=pat[:, hh * 33:hh * 33 + 32],
                            scalar1=recip[:, hh:hh + 1], scalar2=None,
                            op0=OP.mult)
                    ptr = PS_PT()
                    nc.tensor.transpose(ptr[:], anm[:], idm_s[:])
                    attnT = wp.tile([128, 128], fp32, tag="attnT", bufs=1)
                    nc.vector.tensor_copy(attnT[:], ptr[:])
                    ph2 = PS_PT()
                    nc.tensor.matmul(ph2[:], lhsT=wout_s[:], rhs=attnT[:],
                                     start=True, stop=True)
                    osl = slice(g * 512 + qc * 128, g * 512 + (qc + 1) * 128)
                    # pre2 = h2 + b_out' + x
                    nc.vector.scalar_tensor_tensor(
                        out=h2T[:, osl], in0=ph2[:], scalar=bcol(BI_OUTP),
                        in1=xT_s[:, osl], op0=OP.add, op1=OP.add)
                # incremental BN2 stats for this graph's 512 columns
                nc.vector.tensor_reduce(statc2[:, g:g + 1], h2T[:, gsl],
                                        mybir.AxisListType.X, OP.add)
                tsq = wp.tile([128, 512], fp32, tag="sqt", bufs=1)
                nc.vector.scalar_tensor_tensor(
                    out=tsq[:], in0=h2T[:, gsl], scalar=1.0, in1=h2T[:, gsl],
                    op0=OP.mult, op1=OP.mult,
                    accum_out=statc2[:, 8 + g:9 + g])

            # graphs run inside AllGather wait windows (PE idle otherwise)
            mha_sched = {1: [0, 1, 2], 2: [3, 4, 5], 3: [6, 7]} \
                if (do_mha and hops == K - 1) else {}
            mha_left = [g for g in range(GPC if do_mha else 0)
                        if not any(g in v for v in mha_sched.values())]

            # ================= Phase 3: cheb hops ===========================
            # tile_wait_until stamps are scheduler-only hints (virtual
            # earliest-start): they stop the list scheduler from hoisting
            # hop h+1's recurrence ops ahead of the window-h MHA work in
            # the in-order DVE queue (head-of-line blocking during the
            # AllGather). They emit no HW waits.
            cur, prev = TxB, TxA  # cur holds Tx_{h-1}; prev gets Tx_h
            for h in range(1, 1 + hops):
                t_hop = 0.45 + 0.40 * (h - 1)
                ag_src = vfull0 if h == 1 else ag_outs[h - 1]
                hop_stack = ExitStack()
                hop_stack.enter_context(tc.tile_wait_until(t_hop))
                for q in range(NCALL):
                    gt = gp.tile([128, BPG * ngrp, 128], vdt, tag="gt")
                    isl = slice(q * BPG * epb // 16, (q + 1) * BPG * epb // 16)
                    gix = wp.tile([128, BPG * epb // 16], i16, tag="gix", bufs=2)
                    nc.sync.dma_start(gix[:], gidx[:, isl])
                    nc.gpsimd.dma_gather(gt[:], ag_src[:], gix[:],
                                         BPG * epb, BPG * epb, 128,
                                         single_packet=False)
                    for r in range(BPG):
                        b = q * BPG + r
                        t, half = b // 2, b % 2
                        if half == 0:
                            aps = PS_AGG()
                        for j in range(ngrp):
                            gcol = b * ngrp + j
                            nc.tensor.matmul(
                                aps[half * 64:(half + 1) * 64, :],
                                lhsT=s8_s[:, gcol * Bd:(gcol + 1) * Bd],
                                rhs=gt[:, r * ngrp + j, :],
                                start=(j == 0), stop=(j == ngrp - 1))
                        if half == 1:
                            # recurrence for tile t
                            tmp = wp.tile([128, 128], fp32, tag="rectmp")
                            if h == 1:
                                nc.vector.tensor_scalar(
                                    out=tmp[:], in0=aps[:],
                                    scalar1=discm_s[:, t:t + 1], scalar2=None,
                                    op0=OP.mult)
                                # Tx1 = (scale-1)*Tx0 + tmp
                                nc.vector.scalar_tensor_tensor(
                                    out=prev[:, t, :], in0=cur[:, t, :],
                                    scalar=float(scale - 1.0), in1=tmp[:],
                                    op0=OP.mult, op1=OP.add)
                            else:
                                nc.vector.tensor_scalar(
                                    out=tmp[:], in0=aps[:],
                                    scalar1=discm_s[:, t:t + 1], scalar2=2.0,
                                    op0=OP.mult, op1=OP.mult)
                                # tmp2 = tmp - Tx_{h-2}
                                tmp2 = wp.tile([128, 128], fp32, tag="rectmp2")
                                nc.vector.scalar_tensor_tensor(
                                    out=tmp2[:], in0=prev[:, t, :],
                                    scalar=-1.0, in1=tmp[:],
                                    op0=OP.mult, op1=OP.add)
                                # Tx_h = 2(scale-1)*Tx_{h-1} + tmp2
                                nc.vector.scalar_tensor_tensor(
                                    out=prev[:, t, :], in0=cur[:, t, :],
                                    scalar=float(2.0 * (scale - 1.0)),
                                    in1=tmp2[:], op0=OP.mult, op1=OP.add)
                            if h < hops:
                                vt = wp.tile([128, 128], vdt, tag="vt", bufs=3)
                                nc.scalar.activation(
                                    vt[:], prev[:, t, :], AF.Identity,
                                    scale=disc_s[:, t:t + 1])
                                nc.sync.dma_start(
                                    ag_in[t * 128:(t + 1) * 128, :], vt[:])
                # launch AG for next hop once all v~ tiles written
                if h < hops:
                    nc.gpsimd.collective_compute(
                        "AllGather", OP.bypass,
                        replica_groups=[list(range(NCORES))],
                        ins=[ag_in.opt()], outs=[ag_outs[h].opt()])
                hop_stack.close()
                win_stack = ExitStack()
                win_stack.enter_context(tc.tile_wait_until(t_hop + 0.20))
                # out_cheb += Tx_h @ W_h  (transpose tiles chunk-wise)
                for ch in range(CH):
                    tpb = wp.tile([128, 512], fp32, tag="tpb", bufs=1)
                    for j in range(4):
                        t = ch * 4 + j
                        pt = PS_PT()
                        nc.tensor.transpose(pt[:], prev[:, t, :], idm_s[:])
                        nc.vector.tensor_copy(tpb[:, j * 128:(j + 1) * 128], pt[:])
                    sl = slice(ch * 512, (ch + 1) * 512)
                    pw = PS_MM()
                    nc.tensor.matmul(pw[:], lhsT=chebw_s[:, h * 128:(h + 1) * 128],
                                     rhs=tpb[:], start=True, stop=True)
                    nc.vector.tensor_add(chebT[:, sl], chebT[:, sl], pw[:])
                # T_h = Tx_h^T Us ; m1sb += W_h^T T_h  (off the tail)
                t_ps = PS_AGG()
                for t0 in range(0, NT, 4):
                    ub = wp.tile([128, 4, 128], fp32, tag="ut4", bufs=1)
                    nc.scalar.dma_start(
                        ub[:], us_nm[t0 * 128:(t0 + 4) * 128, :]
                        .rearrange("(t p) c -> p t c", p=128))
                    for j in range(4):
                        t = t0 + j
                        nc.tensor.matmul(t_ps[:], lhsT=prev[:, t, :],
                                         rhs=ub[:, j, :],
                                         start=(t == 0), stop=(t == NT - 1))
                tsb = wp.tile([128, 128], fp32, tag="tsb")
                nc.vector.tensor_copy(tsb[:], t_ps[:])
                pWt = PS_PT()
                nc.tensor.matmul(pWt[:], lhsT=chebw_s[:, h * 128:(h + 1) * 128],
                                 rhs=tsb[:], start=True, stop=True)
                nc.vector.tensor_add(m1sb[:], m1sb[:], pWt[:])
                if h == 1:
                    spectral_partial()
                for g in mha_sched.get(h, []):
                    mha_graph(g)
                win_stack.close()
                cur, prev = prev, cur

            # ================= Phase 4: MHA (remaining graphs) ==============
            if hops < 1:
                spectral_partial()
            for g in mha_left:
                mha_graph(g)

            tail_stack = ExitStack()
            tail_stack.enter_context(tc.tile_wait_until(0.45 + 0.40 * hops))
            # ===== Phase 5: pre1' (no spec) + BN stats + M1 + joint AR ======
            for ch in range(CH):
                sl = slice(ch * 512, (ch + 1) * 512)
                # pre1' = chebT + cheb_b + x   (overwrite chebT)
                nc.vector.scalar_tensor_tensor(
                    out=chebT[:, sl], in0=chebT[:, sl], scalar=bcol(BI_CHEB),
                    in1=xT_s[:, sl], op0=OP.add, op1=OP.add)
            # BN1/BN2 stats
            def sumsq(buf, out_col):
                for c2 in range(CH):
                    s2 = slice(c2 * 512, (c2 + 1) * 512)
                    tt = wp.tile([128, 512], fp32, tag="sqt", bufs=1)
                    nc.vector.scalar_tensor_tensor(
                        out=tt[:], in0=buf[:, s2], scalar=1.0, in1=buf[:, s2],
                        op0=OP.mult, op1=OP.mult,
                        accum_out=statc[:, 24 + c2:25 + c2])
                nc.vector.tensor_reduce(out_col, statc[:, 24:32],
                                        mybir.AxisListType.X, OP.add)

            nc.vector.tensor_reduce(statc[:, 0:1], chebT[:], mybir.AxisListType.X, OP.add)
            sumsq(chebT, statc[:, 1:2])
            if do_mha:
                nc.vector.tensor_reduce(statc[:, 2:3], statc2[:, 0:8],
                                        mybir.AxisListType.X, OP.add)
                nc.vector.tensor_reduce(statc[:, 3:4], statc2[:, 8:16],
                                        mybir.AxisListType.X, OP.add)
            else:
                nc.vector.tensor_reduce(statc[:, 2:3], h2T[:], mybir.AxisListType.X, OP.add)
                sumsq(h2T, statc[:, 3:4])
            st12 = wp.tile([128, 260], fp32, tag="st12", bufs=1)
            nc.vector.tensor_copy(st12[:, 0:4], statc[:, 0:4])
            nc.vector.tensor_copy(st12[:, 4:132], m1sb[:])
            nc.vector.tensor_copy(st12[:, 132:260], part_s[:])
            nc.sync.dma_start(bn12_in[:], st12[:])
            nc.gpsimd.collective_compute(
                "AllReduce", OP.add, replica_groups=[list(range(NCORES))],
                ins=[bn12_in.opt()], outs=[bn12_out.opt()])
            sr12 = wp.tile([128, 260], fp32, tag="sr12", bufs=1)
            nc.sync.dma_start(sr12[:], bn12_out[:])
            Pm = sr12[:, 132:260]   # AR'd spectral partial [keig, C]
            M1g = sr12[:, 4:132]    # [C, keig]
            # spectral stat terms:
            # s_sum[c] = sum_k u1[k] P[k,c];  s_sq[c] = sum_k P[k,c](G P)[k,c]
            # cross[c] = sum_k M1g[c,k] P[k,c]
            w12 = wp.tile([128, 256], fp32, tag="w12", bufs=1)
            nc.vector.tensor_scalar(out=w12[:, 0:128], in0=Pm,
                                    scalar1=u1col_s[:], scalar2=None,
                                    op0=OP.mult)
            t1_ps = PS_PT()
            nc.tensor.matmul(t1_ps[:], lhsT=gmat_s[:], rhs=Pm,
                             start=True, stop=True)
            nc.vector.tensor_tensor(w12[:, 128:256], t1_ps[:], Pm, OP.mult)
            # column sums: ones-matmul -> [1,256] row, then row -> two cols
            r_ps = PS_MM()
            nc.tensor.matmul(r_ps[0:1, 0:256], lhsT=onesf_s[:], rhs=w12[:],
                             start=True, stop=True)
            rowbuf = wp.tile([1, 256], fp32, tag="rowbuf")
            nc.vector.tensor_copy(rowbuf[:], r_ps[0:1, 0:256])
            c_ps = PS_PT()
            nc.tensor.matmul(c_ps[:, 0:1], lhsT=rowbuf[:, 0:128], rhs=one1_s[:],
                             start=True, stop=True, skip_group_check=True)
            nc.tensor.matmul(c_ps[:, 1:2], lhsT=rowbuf[:, 128:256], rhs=one1_s[:],
                             start=True, stop=True, skip_group_check=True)
            s_cols = wp.tile([128, 2], fp32, tag="s_cols")
            nc.vector.tensor_copy(s_cols[:], c_ps[:, 0:2])
            # cross: transpose P, multiply with M1g, reduce
            pt_ps = PS_PT()
            nc.tensor.transpose(pt_ps[:], sr12[:, 132:260], idm_s[:])
            ptm = wp.tile([128, 128], fp32, tag="w1t")
            nc.vector.tensor_tensor(ptm[:], pt_ps[:], M1g, OP.mult)
            crossc = statc[:, 14:15]
            nc.vector.tensor_reduce(crossc, ptm[:], mybir.AxisListType.X, OP.add)
            # BN1 totals: sum1 = sr12[:,0] + s_sum ; sq1 = sr12[:,1] + 2*cross + s_sq
            sum1c = statc[:, 15:16]
            nc.vector.tensor_tensor(sum1c, sr12[:, 0:1], s_cols[:, 0:1], OP.add)
            sq1c = statc[:, 16:17]
            nc.vector.scalar_tensor_tensor(out=sq1c, in0=crossc, scalar=2.0,
                                           in1=sr12[:, 1:2], op0=OP.mult,
                                           op1=OP.add)
            nc.vector.tensor_tensor(sq1c, sq1c, s_cols[:, 1:2], OP.add)

            # affine coefs: A = w/sqrt(var+eps), Bc = b - mu*A
            def bn_affine(sum_col, sq_col, w_col, b_col, a_out, b_out_col):
                mu = statc[:, 8:9]
                nc.vector.tensor_scalar(out=mu, in0=sum_col, scalar1=1.0 / N,
                                        scalar2=None, op0=OP.mult)
                msq = statc[:, 9:10]
                nc.vector.tensor_scalar(out=msq, in0=sq_col, scalar1=1.0 / N,
                                        scalar2=None, op0=OP.mult)
                nvar = statc[:, 10:11]
                nc.vector.scalar_tensor_tensor(out=nvar, in0=mu, scalar=mu,
                                               in1=msq, op0=OP.mult,
                                               op1=OP.subtract)  # mu^2 - msq
                sd = statc[:, 11:12]
                nc.scalar.activation(sd, nvar, AF.Sqrt, bias=bcol(BI_EPSC),
                                     scale=-1.0)
                rsd = statc[:, 12:13]
                nc.vector.reciprocal(rsd, sd)
                nc.vector.tensor_tensor(a_out, rsd, w_col, OP.mult)
                nbc = statc[:, 13:14]
                nc.vector.scalar_tensor_tensor(out=nbc, in0=mu, scalar=a_out,
                                               in1=b_col, op0=OP.mult,
                                               op1=OP.subtract)  # mu*A - b
                nc.vector.tensor_scalar(out=b_out_col, in0=nbc, scalar1=-1.0,
                                        scalar2=None, op0=OP.mult)

            A1, B1 = statc[:, 4:5], statc[:, 5:6]
            A2, B2 = statc[:, 6:7], statc[:, 7:8]
            bn_affine(statc[:, 15:16], statc[:, 16:17], bcol(BI_BN1W), bcol(BI_BN1B), A1, B1)
            bn_affine(sr12[:, 2:3], sr12[:, 3:4], bcol(BI_BN2W), bcol(BI_BN2B), A2, B2)
            B12 = statc[:, 5:6]
            nc.vector.tensor_tensor(B12, B1, B2, OP.add)  # B1 += B2 (in place)

            # ============ Phase 6: out = h1 + h2n; MLP2; BN3 ================
            outT = chebT  # overwrite pre1 per chunk
            out2T = h2T   # overwrite pre2 per chunk
            for ch in range(CH):
                sl = slice(ch * 512, (ch + 1) * 512)
                ust = wp.tile([128, 512], fp32, tag="ust")
                nc.scalar.dma_start(ust[:], usT[:, sl])
                pso = PS_MM()
                nc.tensor.matmul(pso[:], lhsT=sr12[:, 132:260], rhs=ust[:],
                                 start=True, stop=True)
                t1 = wp.tile([128, 512], fp32, tag="t1")
                nc.scalar.activation(t1[:], chebT[:, sl], AF.Identity,
                                     bias=B12, scale=A1)
                tsp = wp.tile([128, 512], fp32, tag="t1")
                nc.vector.scalar_tensor_tensor(
                    out=tsp[:], in0=pso[:], scalar=A1, in1=t1[:],
                    op0=OP.mult, op1=OP.add)
                nc.vector.scalar_tensor_tensor(
                    out=outT[:, sl], in0=h2T[:, sl], scalar=A2, in1=tsp[:],
                    op0=OP.mult, op1=OP.add)
                pma = PS_MM()
                nc.tensor.matmul(pma[:], lhsT=mw1_s[:, 0:128], rhs=outT[:, sl],
                                 start=True, stop=True)
                mida = wp.tile([128, 512], fp32, tag="mida")
                nc.scalar.activation(mida[:], pma[:], AF.Relu, bias=bcol(BI_M1A))
                pmb = PS_MM()
                nc.tensor.matmul(pmb[:], lhsT=mw1_s[:, 128:256], rhs=outT[:, sl],
                                 start=True, stop=True)
                midb = wp.tile([128, 512], fp32, tag="mida")
                nc.scalar.activation(midb[:], pmb[:], AF.Relu, bias=bcol(BI_M1B))
                pmo = PS_MM()
                nc.tensor.matmul(pmo[:], lhsT=mw2a_s[:], rhs=mida[:],
                                 start=True, stop=False)
                nc.tensor.matmul(pmo[:], lhsT=mw2b_s[:], rhs=midb[:],
                                 start=False, stop=True)
                # out2 = out + mlp_b2 + psum
                nc.vector.scalar_tensor_tensor(
                    out=out2T[:, sl], in0=outT[:, sl], scalar=bcol(BI_M2),
                    in1=pmo[:], op0=OP.add, op1=OP.add)
            # BN3 stats
            nc.vector.tensor_reduce(statc[:, 0:1], out2T[:], mybir.AxisListType.X, OP.add)
            sumsq(out2T, statc[:, 1:2])
            st3 = wp.tile([128, 2], fp32, tag="st")
            nc.vector.tensor_copy(st3[:], statc[:, 0:2])
            nc.sync.dma_start(bn3_in[:], st3[:])
            nc.gpsimd.collective_compute(
                "AllReduce", OP.add, replica_groups=[list(range(NCORES))],
                ins=[bn3_in.opt()], outs=[bn3_out.opt()])
            sr3 = wp.tile([128, 2], fp32, tag="st")
            nc.sync.dma_start(sr3[:], bn3_out[:])
            A3, B3 = statc[:, 4:5], statc[:, 5:6]
            bn_affine(sr3[:, 0:1], sr3[:, 1:2], bcol(BI_BN3W), bcol(BI_BN3B), A3, B3)

            # apply BN3, transpose to node-major, write out
            for ch in range(CH):
                sl = slice(ch * 512, (ch + 1) * 512)
                bn3b = wp.tile([128, 512], fp32, tag="t1")
                nc.scalar.activation(bn3b[:], out2T[:, sl], AF.Identity,
                                     bias=B3, scale=A3)
                for j in range(4):
                    t = ch * 4 + j
                    pt = PS_PT()
                    nc.tensor.transpose(pt[:], bn3b[:, j * 128:(j + 1) * 128],
                                        idm_s[:])
                    nc.vector.tensor_copy(outnm[:, t, :], pt[:])
            nc.sync.dma_start(
                out_nm[:].rearrange("(t p) c -> p t c", p=128), outnm[:])
            tail_stack.close()
            late_stack.close()

    nc.compile()
    return nc


def kernel(**inputs):
    inp = {k: np.asarray(v) for k, v in inputs.items()}
    cores, dis, scale, ngrp = _preprocess(inp)

    key = (ngrp, float(scale))
    if key not in _CACHE:
        _CACHE[key] = _build(ngrp, scale)
    nc = _CACHE[key]

    x = inp["x"].astype(F32)
    U = inp["U"].astype(F32)
    s_lam = np.exp(-float(inp["gamma"].reshape(-1)[0]) *
                   inp["Lambda"].astype(np.float64) ** 2).astype(F32)

    wqkv = inp["w_qkv"].astype(F32)
    bqkv = inp["b_qkv"].astype(F32)
    wq = (wqkv[:, :C] / np.sqrt(DH)).astype(F32)
    bq = (bqkv[:C] / np.sqrt(DH)).astype(F32)
    wk, bk = wqkv[:, C:2 * C].copy(), bqkv[C:2 * C]
    wv, bv = wqkv[:, 2 * C:].copy(), bqkv[2 * C:]
    b_out_p = (bv @ inp["w_out"] + inp["b_out"]).astype(F32)

    biasp = np.zeros((128, NBIAS), F32)
    for i, vec in [(BI_SPA1, inp["b_spa1"]), (BI_SPA2, inp["b_spa2"]),
                   (BI_SPE1, inp["b_spe1"]), (BI_SPE2, inp["b_spe2"]),
                   (BI_Q, bq), (BI_K, bk), (BI_OUTP, b_out_p),
                   (BI_CHEB, inp["cheb_b"]),
                   (BI_M1A, inp["mlp_b1"][:128]), (BI_M1B, inp["mlp_b1"][128:]),
                   (BI_M2, inp["mlp_b2"]),
                   (BI_BN1W, inp["bn1_w"]), (BI_BN1B, inp["bn1_b"]),
                   (BI_BN2W, inp["bn2_w"]), (BI_BN2B, inp["bn2_b"]),
                   (BI_BN3W, inp["bn3_w"]), (BI_BN3B, inp["bn3_b"]),
                   (BI_EPSC, np.full(128, EPS, F32))]:
        biasp[:, i] = vec.astype(F32)

    chebw_cols = np.concatenate([inp["cheb_w"][k].astype(F32) for k in range(K)],
                                axis=1)  # [128, 5*128]

    common = dict(
        wspa1=inp["w_spa1"].astype(F32), wspa2=inp["w_spa2"].astype(F32),
        wspe1=inp["w_spe1"].astype(F32), wspe2=inp["w_spe2"].astype(F32),
        wproj=inp["w_proj"].astype(F32), chebw=chebw_cols,
        wq=wq, wk=wk, wv=wv, wout=inp["w_out"].astype(F32),
        mw1=inp["mlp_w1"].astype(F32),
        mw2a=inp["mlp_w2"][:128].astype(F32), mw2b=inp["mlp_w2"][128:].astype(F32),
        biasp=biasp,
        bqh=np.ascontiguousarray(bq.reshape(H, DH).T),
        bkh=np.ascontiguousarray(bk.astype(F32).reshape(H, DH).T),
        idm=np.eye(128, dtype=F32),
        gmat=None, u1col=None, onesf=np.ones((128, 1), F32),
        one1=np.ones((1, 1), F32),
    )

    vs = VSCALE if FP8V else 1.0
    Us_full = (U * s_lam[None, :]).astype(F32)
    xfT_np = np.ascontiguousarray(x.T)
    discf_np = np.ascontiguousarray((vs * dis).reshape(N // 128, 128).T)
    gmat_np = (Us_full.T @ Us_full).astype(F32)
    u1_np = np.ascontiguousarray(Us_full.sum(0).astype(F32)[:, None])
    in_maps = []
    for c in range(NCORES):
        sl = slice(c * NLOC, (c + 1) * NLOC)
        dis_c = dis[sl]
        m = dict(common)
        m["xT"] = np.ascontiguousarray(x[sl].T)
        m["u_nm"] = np.ascontiguousarray(U[sl])
        m["usT"] = np.ascontiguousarray((U[sl] * s_lam[None, :]).T)
        m["us_nm"] = np.ascontiguousarray(U[sl] * s_lam[None, :])
        m["gmat"] = gmat_np
        m["u1col"] = u1_np
        m["xfT"] = xfT_np
        m["discf"] = discf_np
        us_loc = Us_full[sl]
        m["m1x"] = np.ascontiguousarray(
            x[sl].T @ us_loc
            + np.outer(inp["cheb_b"].astype(F32), us_loc.sum(0)))
        m["disc"] = np.ascontiguousarray((vs * dis_c).reshape(NT, 128).T)
        m["disc_m"] = np.ascontiguousarray(
            (-scale / vs * dis_c).reshape(NT, 128).T)
        m["gidx"] = cores[c]["gidx"]
        m["s8"] = cores[c]["s8"]
        in_maps.append(m)

    import os
    global LAST_NC, LAST_IN_MAPS
    LAST_NC = nc
    LAST_IN_MAPS = in_maps
    trace = os.environ.get("KERNEL_TRACE", "0") == "1"
    res = run_bass_kernel_spmd(nc, in_maps, core_ids=list(range(NCORES)),
                               trace=trace)
    global LAST_EXEC_NS, LAST_RESULT
    LAST_EXEC_NS = res.exec_time_ns
    LAST_RESULT = res
    out = np.concatenate([res.results[c]["out_nm"] for c in range(NCORES)], axis=0)
    return out.astype(inp["x"].dtype)



# revision 37
# speedup vs baseline: 1.2552x; 1.2552x over previous
"""Trainium2 Bass kernel for nn_CachedSpectralGPSLayer (8-core SPMD).

Self-contained: takes FULL inputs, shards per-core internally, runs one
Bass/Tile program SPMD on 8 NeuronCores, gathers the full output.
"""
import sys

sys.path.insert(0, "/opt/trn_rl_repo")

import numpy as np
import ml_dtypes

import concourse.bacc as bacc
import concourse.bass as bass
import concourse.mybir as mybir
import concourse.tile as tile
from concourse import library_config
from concourse.bass_utils import run_bass_kernel_spmd

BF16 = ml_dtypes.bfloat16
FP8 = ml_dtypes.float8_e4m3
F32 = np.float32

N, C, K, KEIG, B, NG, H = 32768, 128, 5, 128, 64, 512, 4
NCORES = 8
NLOC = N // NCORES          # 4096
NT = NLOC // 128            # 32 node tiles per core
Bd = 64                     # dst nodes per block
NBLK = NLOC // Bd           # 64 blocks per core
GPC = B // NCORES           # 8 graphs per core
DH = C // H                 # 32
EPS = 1e-5
BPG = 4                     # blocks per dma_gather call
NCALL = NBLK // BPG         # 16 gather calls per hop
FP8V = False                # gather/AllGather payload (v~) in fp8e4m3
VSCALE = 16.0               # v~ pre-scale (keeps fp8 out of subnormals)

fp32 = mybir.dt.float32
f32r = mybir.dt.float32r
bf16 = mybir.dt.bfloat16
fp8 = mybir.dt.float8e4
i16 = mybir.dt.int16


def R(ap):
    """Bitcast an fp32 AP to float32r: bit-identical fp32 data, but the PE
    runs replicated mode (1 cyc/row when moving dim >=256 vs 4 for fp32)."""
    return ap.bitcast(f32r)

# bias-pack column indices
(BI_SPA1, BI_SPA2, BI_SPE1, BI_SPE2, BI_Q, BI_K, BI_OUTP, BI_CHEB,
 BI_M1A, BI_M1B, BI_M2, BI_BN1W, BI_BN1B, BI_BN2W, BI_BN2B, BI_BN3W,
 BI_BN3B, BI_EPSC) = range(18)
NBIAS = 18

_CACHE = {}


def _wrap_idx(idx_flat):
    """dma_gather wrapped layout per call: idx i -> [i%16, i//16], replicated
    to all 8 groups of 16 partitions. idx_flat: [ncalls, n_per_call]."""
    ncalls, npc = idx_flat.shape
    base = idx_flat.reshape(ncalls, npc // 16, 16).transpose(0, 2, 1)  # [ncalls,16,npc/16]
    out = np.tile(base, (1, 8, 1))                                     # [ncalls,128,npc/16]
    return np.concatenate(list(out), axis=1)                           # [128, ncalls*npc/16]


def _preprocess(inputs):
    src = np.asarray(inputs["edge_index"][0]).astype(np.int64)
    dst = np.asarray(inputs["edge_index"][1]).astype(np.int64)
    deg = np.bincount(src, minlength=N).astype(np.float64)
    dis = np.where(deg > 0, 1.0 / np.sqrt(deg), 0.0).astype(F32)
    lam = float(np.asarray(inputs["lambda_max"]).reshape(-1)[0])
    scale = 2.0 / lam

    order = np.argsort(dst, kind="stable")
    srcs, dsts = src[order], dst[order]
    counts = np.bincount(dst // Bd, minlength=N // Bd)
    ngrp = int(np.ceil(counts.max() / 128))
    epb = ngrp * 128                    # padded edges per block
    epad = NBLK * epb                   # per core

    cores = []
    bounds = np.searchsorted(dsts, np.arange(0, N + 1, NLOC))
    for c in range(NCORES):
        lo = c * NLOC
        sl = slice(bounds[c], bounds[c + 1])
        sc, dc = srcs[sl], dsts[sl] - lo
        blk = dc // Bd
        ord2 = np.lexsort((sc, blk))  # sort by src within each dst block
        sc, dc, blk = sc[ord2], dc[ord2], blk[ord2]
        cnt = np.bincount(blk, minlength=NBLK)
        csum = np.concatenate([[0], np.cumsum(cnt)])
        pos_in_blk = np.arange(len(sc)) - csum[blk]
        slot = blk * epb + pos_in_blk
        src_pad = np.zeros(epad, np.int64)
        src_pad[slot] = sc
        # S one-hot fp8 bytes [128, ngroups*Bd]
        ngroups = NBLK * ngrp
        s8 = np.zeros((128, ngroups * Bd), np.uint8)
        g = slot // 128
        p = slot % 128
        s8[p, g * Bd + (dc - blk * Bd)] = 0x38  # fp8e4m3 1.0
        assert src_pad.max() < 2 ** 15
        gidx = _wrap_idx(src_pad.astype(np.int16).reshape(NCALL, BPG * epb))
        cores.append(dict(s8=s8.view(FP8), gidx=gidx))
    return cores, dis, scale, ngrp


def _build(ngrp, scale, hops=K - 1, do_mha=True, do_spec=True):
    do_spec = True
    """Build + compile the SPMD Bass program. Returns (nc, input names)."""
    epb = ngrp * 128
    epad = NBLK * epb
    ngroups = NBLK * ngrp

    nc = bacc.Bacc("TRN2", target_bir_lowering=False, debug=False,
                   enable_asserts=True, num_devices=NCORES)

    def din(name, shape, dt):
        return nc.dram_tensor(name, shape, dt, kind="ExternalInput").ap()

    xT = din("xT", [128, NLOC], bf16)
    u_nm = din("u_nm", [NLOC, 128], bf16)
    usT = din("usT", [128, NLOC], bf16)
    disc = din("disc", [128, NT], fp32)
    disc_m = din("disc_m", [128, NT], fp32)
    gidx = din("gidx", [128, epad // 16], i16)
    s8 = din("s8", [128, ngroups * Bd], fp8)
    wspa1 = din("wspa1", [128, 128], bf16)
    wspa2 = din("wspa2", [128, 128], bf16)
    wspe1 = din("wspe1", [128, 128], bf16)
    wspe2 = din("wspe2", [128, 128], bf16)
    wproj = din("wproj", [128, 128], bf16)
    chebw = din("chebw", [128, K * 128], bf16)
    wq = din("wq", [128, 128], bf16)
    wk = din("wk", [128, 128], bf16)
    wv = din("wv", [128, 128], bf16)
    wout = din("wout", [128, 128], bf16)
    mw1 = din("mw1", [128, 256], bf16)
    mw2a = din("mw2a", [128, 128], bf16)
    mw2b = din("mw2b", [128, 128], bf16)
    biasp = din("biasp", [128, NBIAS], fp32)
    bqh = din("bqh", [32, H], fp32)
    bkh = din("bkh", [32, H], fp32)
    idm = din("idm", [128, 128], fp32)
    gmat = din("gmat", [128, 128], fp32)
    u1col = din("u1col", [128, 1], fp32)
    usd = din("usd", [NLOC, 128], bf16)
    idmb = din("idmb", [128, 128], bf16)
    onesf = din("onesf", [128, 1], fp32)
    one1 = din("one1", [1, 1], fp32)
    m1x = din("m1x", [128, 128], fp32)
    xfT = din("xfT", [128, N], bf16)
    discf = din("discf", [128, N // 128], fp32)

    out_nm = nc.dram_tensor("out_nm", [NLOC, 128], fp32, kind="ExternalOutput").ap()

    AF = mybir.ActivationFunctionType
    OP = mybir.AluOpType

    with tile.TileContext(nc) as tc:
        with tc.tile_pool(name="const", bufs=1) as cp, \
             tc.tile_pool(name="big", bufs=1) as bp, \
             tc.tile_pool(name="work", bufs=2) as wp, \
             tc.tile_pool(name="psmm", bufs=2, space="PSUM") as psmm, \
             tc.tile_pool(name="psat", bufs=2, space="PSUM") as psatp, \
             tc.tile_pool(name="pspt", bufs=2, space="PSUM") as psptp, \
             tc.tile_pool(name="psagg", bufs=2, space="PSUM") as psaggp, \
             tc.tile_pool(name="dram", bufs=1, space="DRAM") as dp:

            # uniform-tag psum allocators (PSUM = 8 banks total: 2+2+2+2)
            def PS_MM():   # transient [128,512] matmul outputs
                return psmm.tile([128, 512], fp32, tag="mm", name="psmm_t")

            def PS_AT(shape):  # long-lived accumulators / phase-2 partial
                return psatp.tile(shape, fp32, tag="at", name="psat_t")

            def PS_ATW():  # MHA attn+denom accumulator [128, 132]
                return psatp.tile([128, 33 * H], fp32, tag="at", name="psatw_t")

            def PS_PT2():  # [32, 512] head q/k psum
                return psptp.tile([32, 512], fp32, tag="pt", name="pspt2_t")

            def PS_PT():   # [128,128] transposes / small matmuls
                return psptp.tile([128, 128], fp32, tag="pt", name="pspt_t")

            def PS_AGG():  # [128,128] cheb aggregation
                return psaggp.tile([128, 128], fp32, tag="agg", name="psagg_t")

            nc.gpsimd.load_library(library_config.mlp)

            # ---- load constants ----
            def ld(ap_in, shape, dt, name, eng=None):
                t = cp.tile(shape, dt, tag=name, name=name)
                (eng or nc.sync).dma_start(t[:], ap_in[:])
                return t

            xT_s = bp.tile([128, NLOC], bf16, tag="xT")
            nc.sync.dma_start(xT_s[:], xT[:])
            s8_s = ld(s8, [128, ngroups * Bd], fp8, "s8")
            wspa1_s = ld(wspa1, [128, 128], bf16, "wspa1")
            wspa2_s = ld(wspa2, [128, 128], bf16, "wspa2")
            wspe1_s = ld(wspe1, [128, 128], bf16, "wspe1")
            wspe2_s = ld(wspe2, [128, 128], bf16, "wspe2")
            wproj_s = ld(wproj, [128, 128], bf16, "wproj")
            chebw_s = ld(chebw, [128, K * 128], bf16, "chebw")
            wq_s = ld(wq, [128, 128], bf16, "wq")
            wk_s = ld(wk, [128, 128], bf16, "wk")
            wv_s = ld(wv, [128, 128], bf16, "wv")
            wout_s = ld(wout, [128, 128], bf16, "wout")
            mw1_s = ld(mw1, [128, 256], bf16, "mw1")
            mw2a_s = ld(mw2a, [128, 128], bf16, "mw2a")
            mw2b_s = ld(mw2b, [128, 128], bf16, "mw2b")
            biasp_s = ld(biasp, [128, NBIAS], fp32, "biasp")
            bqh_s = ld(bqh, [32, H], fp32, "bqh")
            bkh_s = ld(bkh, [32, H], fp32, "bkh")
            idm_s = ld(idm, [128, 128], fp32, "idm")
            idmb_s = ld(idmb, [128, 128], bf16, "idmb")
            usd_s = bp.tile([128, NT, 128], bf16, tag="usd")
            nc.sync.dma_start(usd_s[:], usd[:].rearrange("(t p) c -> p t c", p=128))
            vbuf = bp.tile([128, NT, 128], bf16, tag="vbuf")
            gmat_s = ld(gmat, [128, 128], fp32, "gmat")
            u1col_s = ld(u1col, [128, 1], fp32, "u1col")
            onesf_s = ld(onesf, [128, 1], fp32, "onesf")
            one1_s = ld(one1, [1, 1], fp32, "one1")
            m1x_s = ld(m1x, [128, 128], fp32, "m1x")
            m1sb = cp.tile([128, 128], fp32, tag="m1sb", name="m1sb")
            discf_s = ld(discf, [128, N // 128], fp32, "discf")
            disc_s = ld(disc, [128, NT], fp32, "disc")
            discm_s = ld(disc_m, [128, 2 * NT], fp32, "discm")
            discm2_s = discm_s[:, NT:2 * NT]

            def bcol(i):
                return biasp_s[:, i:i + 1]

            # persistent big buffers
            TxA = bp.tile([128, NT, 128], fp32, tag="TxA")   # node-major
            TxB = bp.tile([128, NT, 128], fp32, tag="TxB")
            chebT = bp.tile([128, NLOC], fp32, tag="chebT")  # later: pre1, outT
            h2T = bp.tile([128, NLOC], fp32, tag="h2T")      # later: pre2, out2T
            if not do_mha:
                nc.vector.memset(h2T[:], 0.0)
            statc = cp.tile([128, 32], fp32, tag="statc")    # stats/affine cols
            statc2 = cp.tile([128, 16], fp32, tag="statc2")  # per-graph h2 stats
            outnm = TxA  # reuse (dead after cheb)

            # DRAM bounce buffers
            vdt = fp8 if FP8V else bf16
            ag_in = dp.tile([NLOC, 128], vdt, tag="ag_in")
            ag_outs = [None] + [dp.tile([N, 128], vdt, tag=f"ag_out{h}", name=f"ag_out{h}", addr_space="Shared") for h in range(1, 4)]
            vfull0 = dp.tile([N, 128], vdt, tag="vfull0", name="vfull0")
            bn12_in = dp.tile([128, 260], fp32, tag="bn12_in")
            bn12_out = dp.tile([128, 260], fp32, tag="bn12_out", addr_space="Shared")
            bn3_in = dp.tile([128, 2], fp32, tag="bn3_in")
            bn3_out = dp.tile([128, 2], fp32, tag="bn3_out", addr_space="Shared")

            CH = NLOC // 512  # 8 chunks of 512

            from contextlib import ExitStack
            ep_stack = ExitStack()
            ep = ep_stack.enter_context(tc.tile_pool(name="early", bufs=1))
            xspT = ep.tile([128, NLOC], bf16, tag="xspT", name="xspT")

            # ================= Phase 1: local spa MLP (feature-major) =======
            # (the spe MLP + spectral partial run inside AllGather window 1,
            # recomputed from the resident xT_s, off the pre-hop critical path)
            for ch in range(CH):
                sl = slice(ch * 512, (ch + 1) * 512)
                p1 = PS_MM()
                nc.tensor.matmul(p1[:], lhsT=wspa1_s[:], rhs=xT_s[:, sl],
                                 start=True, stop=True)
                t1 = wp.tile([128, 512], bf16, tag="t1b")
                nc.scalar.activation(t1[:], p1[:], AF.Relu, bias=bcol(BI_SPA1))
                p2 = PS_MM()
                nc.tensor.matmul(p2[:], lhsT=wspa2_s[:], rhs=t1[:],
                                 start=True, stop=True)
                nc.scalar.activation(xspT[:, sl], p2[:], AF.Identity,
                                     bias=bcol(BI_SPA2))

            # Tx0 node-major (local shard, for recurrence) + v~0 into vbuf
            for t in range(NT):
                tsl = slice(t * 128, (t + 1) * 128)
                pt = PS_PT()
                nc.tensor.matmul(pt[:], lhsT=xspT[:, tsl], rhs=idmb_s[:],
                                 start=True, stop=True)
                nc.vector.tensor_copy(TxB[:, t, :], pt[:])
                nc.scalar.activation(vbuf[:, t, :], pt[:], AF.Identity,
                                     scale=disc_s[:, t:t + 1])
            # v~0 for ALL nodes computed locally (replaces hop-1 AllGather):
            # every core redundantly runs the spatial MLP over the full x.
            if hops >= 1:
                for gch in range(N // 512):
                    gsl2 = slice(gch * 512, (gch + 1) * 512)
                    xc = wp.tile([128, 512], bf16, tag="t1b")
                    nc.sync.dma_start(xc[:], xfT[:, gsl2])
                    pf1 = PS_MM()
                    nc.tensor.matmul(pf1[:], lhsT=wspa1_s[:], rhs=xc[:],
                                     start=True, stop=True)
                    tf1 = wp.tile([128, 512], bf16, tag="midab")
                    nc.scalar.activation(tf1[:], pf1[:], AF.Relu,
                                         bias=bcol(BI_SPA1))
                    pf2 = PS_MM()
                    nc.tensor.matmul(pf2[:], lhsT=wspa2_s[:], rhs=tf1[:],
                                     start=True, stop=True)
                    spf = wp.tile([128, 512], bf16, tag="t1b")
                    nc.scalar.activation(spf[:], pf2[:], AF.Identity,
                                         bias=bcol(BI_SPA2))
                    vt4 = wp.tile([128, 4, 128], vdt, tag="vt4", bufs=2)
                    for j in range(4):
                        tg = gch * 4 + j
                        ptf = PS_PT()
                        nc.tensor.matmul(ptf[:], lhsT=spf[:, j * 128:(j + 1) * 128],
                                         rhs=idmb_s[:], start=True, stop=True)
                        nc.scalar.activation(vt4[:, j, :], ptf[:], AF.Identity,
                                             scale=discf_s[:, tg:tg + 1])
                    nc.sync.dma_start(
                        vfull0[gch * 512:(gch + 1) * 512, :]
                        .rearrange("(t p) c -> p t c", p=128), vt4[:])

            for ch in range(CH):
                sl = slice(ch * 512, (ch + 1) * 512)
                pw = PS_MM()
                nc.tensor.matmul(pw[:], lhsT=chebw_s[:, 0:128], rhs=xspT[:, sl],
                                 start=True, stop=True)
                nc.vector.tensor_copy(chebT[:, sl], pw[:])
            # T_0 = v~0^T (Us/dis) (accumulate over tiles); m1sb = m1x + W0^T T_0
            t_ps = PS_AGG()
            for t in range(NT):
                nc.tensor.matmul(t_ps[:], lhsT=vbuf[:, t, :], rhs=usd_s[:, t, :],
                                 start=(t == 0), stop=(t == NT - 1))
            tsb = wp.tile([128, 128], bf16, tag="tsb")
            nc.vector.tensor_copy(tsb[:], t_ps[:])
            pWt = PS_PT()
            nc.tensor.matmul(pWt[:], lhsT=chebw_s[:, 0:128], rhs=tsb[:],
                             start=True, stop=True)
            nc.vector.tensor_add(m1sb[:], m1x_s[:], pWt[:])

            # ===== Phase 2 (deferred): spectral partial, run in AG window 1 =
            # Recomputes the spe MLP chunk-wise from resident xT_s so xspecT
            # needs no SBUF residency across the hops.
            part_s = wp.tile([128, 128], fp32, tag="part_s", bufs=1)

            def spectral_partial():
                part_ps = PS_AT([128, 128])
                for ch2 in range(CH):
                    sl2 = slice(ch2 * 512, (ch2 + 1) * 512)
                    p3 = PS_MM()
                    nc.tensor.matmul(p3[:], lhsT=wspe1_s[:], rhs=xT_s[:, sl2],
                                     start=True, stop=True)
                    t2 = wp.tile([128, 512], bf16, tag="t1b")
                    nc.scalar.activation(t2[:], p3[:], AF.Relu,
                                         bias=bcol(BI_SPE1))
                    p4 = PS_MM()
                    nc.tensor.matmul(p4[:], lhsT=wspe2_s[:], rhs=t2[:],
                                     start=True, stop=True)
                    xsp2 = wp.tile([128, 512], bf16, tag="t1b")
                    nc.scalar.activation(xsp2[:], p4[:], AF.Identity,
                                         bias=bcol(BI_SPE2))
                    ub = wp.tile([128, 4, 128], bf16, tag="ut4", bufs=1)
                    nc.sync.dma_start(
                        ub[:], u_nm[ch2 * 512:(ch2 + 1) * 512, :]
                        .rearrange("(t p) c -> p t c", p=128))
                    for j in range(4):
                        t = ch2 * 4 + j
                        ph = PS_PT()
                        nc.tensor.matmul(ph[:], lhsT=xsp2[:, j * 128:(j + 1) * 128],
                                         rhs=wproj_s[:], start=True, stop=True)
                        hp = wp.tile([128, 128], bf16, tag="hp")
                        nc.vector.tensor_copy(hp[:], ph[:])
                        nc.tensor.matmul(part_ps[:], lhsT=ub[:, j, :], rhs=hp[:],
                                         start=(t == 0), stop=(t == NT - 1))
                nc.vector.tensor_copy(part_s[:], part_ps[:])

            ep_stack.close()  # free xspT space for later pools
            late_stack = ExitStack()
            gp = late_stack.enter_context(tc.tile_pool(name="gath", bufs=2))
            mp = late_stack.enter_context(tc.tile_pool(name="mha", bufs=2))

            # ---- MHA for one graph (interleaved into AllGather windows) ----
            def mha_graph(g):
                gsl = slice(g * 512, (g + 1) * 512)
                # head-major q/k: per-head matmuls so all operands are base-0
                qT = mp.tile([32, H * 512], bf16, tag="qT", bufs=1)
                kT = mp.tile([32, H * 512], bf16, tag="kT", bufs=1)
                for hh in range(H):
                    csl = slice(hh * 32, (hh + 1) * 32)
                    pqh = PS_PT2()
                    nc.tensor.matmul(pqh[:], lhsT=wq_s[:, csl],
                                     rhs=xT_s[:, gsl], start=True, stop=True)
                    nc.scalar.activation(qT[:, hh * 512:(hh + 1) * 512], pqh[:],
                                         AF.Identity, bias=bqh_s[:, hh:hh + 1])
                    pkh = PS_PT2()
                    nc.tensor.matmul(pkh[:], lhsT=wk_s[:, csl],
                                     rhs=xT_s[:, gsl], start=True, stop=True)
                    nc.scalar.activation(kT[:, hh * 512:(hh + 1) * 512], pkh[:],
                                         AF.Identity, bias=bkh_s[:, hh:hh + 1])
                # v node-major, augmented per head with a ones column
                vaug = mp.tile([128, 4, 33 * H], bf16, tag="vaug")
                nc.vector.memset(vaug[:, :, 32::33], 1.0)
                for j in range(4):
                    pv = PS_PT()
                    nc.tensor.matmul(pv[:], lhsT=xT_s[:, g * 512 + j * 128:
                                                      g * 512 + (j + 1) * 128],
                                     rhs=wv_s[:], start=True, stop=True)
                    for hh in range(H):
                        nc.vector.tensor_copy(
                            vaug[:, j, hh * 33:hh * 33 + 32],
                            pv[:, hh * 32:(hh + 1) * 32])
                # scores_T + exp, per (head, k-chunk)
                ess = {}
                for hh in range(H):
                    qsl = slice(hh * 512, (hh + 1) * 512)
                    for j in range(4):
                        pss = PS_MM()
                        nc.tensor.matmul(
                            pss[:], lhsT=kT[:, hh * 512 + j * 128:
                                            hh * 512 + (j + 1) * 128],
                            rhs=qT[:, qsl], start=True, stop=True)
                        es = mp.tile([128, 512], bf16, tag="es", bufs=16)
                        nc.scalar.activation(es[:], pss[:], AF.Exp)
                        ess[(hh, j)] = es
                # attn + denom per q-chunk
                for qc in range(4):
                    pat = PS_ATW()
                    for hh in range(H):
                        for j in range(4):
                            nc.tensor.matmul(
                                pat[:, hh * 33:(hh + 1) * 33],
                                lhsT=ess[(hh, j)][:, qc * 128:(qc + 1) * 128],
                                rhs=vaug[:, j, hh * 33:(hh + 1) * 33],
                                start=(j == 0), stop=(j == 3),
                                skip_group_check=True)
                    recip = wp.tile([128, 4], fp32, tag="recip")
                    nc.vector.reciprocal(recip[:], pat[:, 32::33])
                    anm = wp.tile([128, 128], fp32, tag="anm")
                    for hh in range(H):
                        nc.vector.tensor_scalar(
                            out=anm[:, hh * 32:(hh + 1) * 32],
                            in0=pat[:, hh * 33:hh * 33 + 32],
                            scalar1=recip[:, hh:hh + 1], scalar2=None,
                            op0=OP.mult)
                    ptr = PS_PT()
                    nc.tensor.transpose(ptr[:], anm[:], idm_s[:])
                    attnT = wp.tile([128, 128], bf16, tag="attnT", bufs=1)
                    nc.vector.tensor_copy(attnT[:], ptr[:])
                    ph2 = PS_PT()
                    nc.tensor.matmul(ph2[:], lhsT=wout_s[:], rhs=attnT[:],
                                     start=True, stop=True)
                    osl = slice(g * 512 + qc * 128, g * 512 + (qc + 1) * 128)
                    # pre2 = h2 + b_out' + x
                    nc.vector.scalar_tensor_tensor(
                        out=h2T[:, osl], in0=ph2[:], scalar=bcol(BI_OUTP),
                        in1=xT_s[:, osl], op0=OP.add, op1=OP.add)
                # incremental BN2 stats for this graph's 512 columns
                nc.vector.tensor_reduce(statc2[:, g:g + 1], h2T[:, gsl],
                                        mybir.AxisListType.X, OP.add)
                tsq = wp.tile([128, 512], fp32, tag="sqt", bufs=1)
                nc.vector.scalar_tensor_tensor(
                    out=tsq[:], in0=h2T[:, gsl], scalar=1.0, in1=h2T[:, gsl],
                    op0=OP.mult, op1=OP.mult,
                    accum_out=statc2[:, 8 + g:9 + g])

            # graphs run inside AllGather wait windows (PE idle otherwise)
            mha_sched = {1: [0, 1, 2], 2: [3, 4, 5], 3: [6, 7]} \
                if (do_mha and hops == K - 1) else {}
            mha_left = [g for g in range(GPC if do_mha else 0)
                        if not any(g in v for v in mha_sched.values())]

            # ================= Phase 3: cheb hops ===========================
            # tile_wait_until stamps are scheduler-only hints (virtual
            # earliest-start): they stop the list scheduler from hoisting
            # hop h+1's recurrence ops ahead of the window-h MHA work in
            # the in-order DVE queue (head-of-line blocking during the
            # AllGather). They emit no HW waits.
            cur, prev = TxB, TxA  # cur holds Tx_{h-1}; prev gets Tx_h
            for h in range(1, 1 + hops):
                t_hop = 0.45 + 0.40 * (h - 1)
                ag_src = vfull0 if h == 1 else ag_outs[h - 1]
                hop_stack = ExitStack()
                hop_stack.enter_context(tc.tile_wait_until(t_hop))
                for q in range(NCALL):
                    gt = gp.tile([128, BPG * ngrp, 128], vdt, tag="gt")
                    isl = slice(q * BPG * epb // 16, (q + 1) * BPG * epb // 16)
                    gix = wp.tile([128, BPG * epb // 16], i16, tag="gix", bufs=2)
                    nc.sync.dma_start(gix[:], gidx[:, isl])
                    nc.gpsimd.dma_gather(gt[:], ag_src[:], gix[:],
                                         BPG * epb, BPG * epb, 128,
                                         single_packet=False)
                    for r in range(BPG):
                        b = q * BPG + r
                        t, half = b // 2, b % 2
                        if half == 0:
                            aps = PS_AGG()
                        for j in range(ngrp):
                            gcol = b * ngrp + j
                            nc.tensor.matmul(
                                aps[half * 64:(half + 1) * 64, :],
                                lhsT=s8_s[:, gcol * Bd:(gcol + 1) * Bd],
                                rhs=gt[:, r * ngrp + j, :],
                                start=(j == 0), stop=(j == ngrp - 1))
                        if half == 1:
                            # recurrence for tile t
                            tmp = wp.tile([128, 128], fp32, tag="rectmp")
                            if h == 1:
                                nc.vector.tensor_scalar(
                                    out=tmp[:], in0=aps[:],
                                    scalar1=discm_s[:, t:t + 1], scalar2=None,
                                    op0=OP.mult)
                                # Tx1 = (scale-1)*Tx0 + tmp
                                nc.vector.scalar_tensor_tensor(
                                    out=prev[:, t, :], in0=cur[:, t, :],
                                    scalar=float(scale - 1.0), in1=tmp[:],
                                    op0=OP.mult, op1=OP.add)
                            else:
                                nc.vector.tensor_scalar(
                                    out=tmp[:], in0=aps[:],
                                    scalar1=discm_s[:, t:t + 1], scalar2=2.0,
                                    op0=OP.mult, op1=OP.mult)
                                # tmp2 = tmp - Tx_{h-2}
                                tmp2 = wp.tile([128, 128], fp32, tag="rectmp2")
                                nc.vector.scalar_tensor_tensor(
                                    out=tmp2[:], in0=prev[:, t, :],
                                    scalar=-1.0, in1=tmp[:],
                                    op0=OP.mult, op1=OP.add)
                                # Tx_h = 2(scale-1)*Tx_{h-1} + tmp2
                                nc.vector.scalar_tensor_tensor(
                                    out=prev[:, t, :], in0=cur[:, t, :],
                                    scalar=float(2.0 * (scale - 1.0)),
                                    in1=tmp2[:], op0=OP.mult, op1=OP.add)
                            nc.scalar.activation(
                                vbuf[:, t, :], prev[:, t, :], AF.Identity,
                                scale=disc_s[:, t:t + 1])
                            if h < hops:
                                nc.sync.dma_start(
                                    ag_in[t * 128:(t + 1) * 128, :],
                                    vbuf[:, t, :])
                # launch AG for next hop once all v~ tiles written
                if h < hops:
                    nc.gpsimd.collective_compute(
                        "AllGather", OP.bypass,
                        replica_groups=[list(range(NCORES))],
                        ins=[ag_in.opt()], outs=[ag_outs[h].opt()])
                hop_stack.close()
                win_stack = ExitStack()
                win_stack.enter_context(tc.tile_wait_until(t_hop + 0.20))
                # out_cheb += Tx_h @ W_h  (transpose tiles chunk-wise)
                for ch in range(CH):
                    tpb = wp.tile([128, 512], bf16, tag="tpb", bufs=1)
                    for j in range(4):
                        t = ch * 4 + j
                        pt = PS_PT()
                        nc.tensor.transpose(pt[:], prev[:, t, :], idm_s[:])
                        nc.vector.tensor_copy(tpb[:, j * 128:(j + 1) * 128], pt[:])
                    sl = slice(ch * 512, (ch + 1) * 512)
                    pw = PS_MM()
                    nc.tensor.matmul(pw[:], lhsT=chebw_s[:, h * 128:(h + 1) * 128],
                                     rhs=tpb[:], start=True, stop=True)
                    nc.vector.tensor_add(chebT[:, sl], chebT[:, sl], pw[:])
                # T_h = v~_h^T (Us/dis) ; m1sb += W_h^T T_h  (off the tail)
                t_ps = PS_AGG()
                for t in range(NT):
                    nc.tensor.matmul(t_ps[:], lhsT=vbuf[:, t, :],
                                     rhs=usd_s[:, t, :],
                                     start=(t == 0), stop=(t == NT - 1))
                tsb = wp.tile([128, 128], bf16, tag="tsb")
                nc.vector.tensor_copy(tsb[:], t_ps[:])
                pWt = PS_PT()
                nc.tensor.matmul(pWt[:], lhsT=chebw_s[:, h * 128:(h + 1) * 128],
                                 rhs=tsb[:], start=True, stop=True)
                nc.vector.tensor_add(m1sb[:], m1sb[:], pWt[:])
                if h == 1:
                    spectral_partial()
                for g in mha_sched.get(h, []):
                    mha_graph(g)
                win_stack.close()
                cur, prev = prev, cur

            # ================= Phase 4: MHA (remaining graphs) ==============
            for g in mha_left:
                mha_graph(g)

            tail_stack = ExitStack()
            tail_stack.enter_context(tc.tile_wait_until(0.45 + 0.40 * hops))
            # ===== Phase 5: pre1' (no spec) + BN stats + M1 + joint AR ======
            for ch in range(CH):
                sl = slice(ch * 512, (ch + 1) * 512)
                # pre1' = chebT + cheb_b + x   (overwrite chebT)
                nc.vector.scalar_tensor_tensor(
                    out=chebT[:, sl], in0=chebT[:, sl], scalar=bcol(BI_CHEB),
                    in1=xT_s[:, sl], op0=OP.add, op1=OP.add)
            # BN1/BN2 stats
            def sumsq(buf, out_col):
                for c2 in range(CH):
                    s2 = slice(c2 * 512, (c2 + 1) * 512)
                    tt = wp.tile([128, 512], fp32, tag="sqt", bufs=1)
                    nc.vector.scalar_tensor_tensor(
                        out=tt[:], in0=buf[:, s2], scalar=1.0, in1=buf[:, s2],
                        op0=OP.mult, op1=OP.mult,
                        accum_out=statc[:, 24 + c2:25 + c2])
                nc.vector.tensor_reduce(out_col, statc[:, 24:32],
                                        mybir.AxisListType.X, OP.add)

            nc.vector.tensor_reduce(statc[:, 0:1], chebT[:], mybir.AxisListType.X, OP.add)
            sumsq(chebT, statc[:, 1:2])
            if do_mha:
                nc.vector.tensor_reduce(statc[:, 2:3], statc2[:, 0:8],
                                        mybir.AxisListType.X, OP.add)
                nc.vector.tensor_reduce(statc[:, 3:4], statc2[:, 8:16],
                                        mybir.AxisListType.X, OP.add)
            else:
                nc.vector.tensor_reduce(statc[:, 2:3], h2T[:], mybir.AxisListType.X, OP.add)
                sumsq(h2T, statc[:, 3:4])
            st12 = wp.tile([128, 260], fp32, tag="st12", bufs=1)
            nc.vector.tensor_copy(st12[:, 0:4], statc[:, 0:4])
            nc.vector.tensor_copy(st12[:, 4:132], m1sb[:])
            nc.vector.tensor_copy(st12[:, 132:260], part_s[:])
            nc.sync.dma_start(bn12_in[:], st12[:])
            nc.gpsimd.collective_compute(
                "AllReduce", OP.add, replica_groups=[list(range(NCORES))],
                ins=[bn12_in.opt()], outs=[bn12_out.opt()])
            sr12 = wp.tile([128, 260], fp32, tag="sr12", bufs=1)
            nc.sync.dma_start(sr12[:], bn12_out[:])
            Pm = sr12[:, 132:260]   # AR'd spectral partial [keig, C]
            M1g = sr12[:, 4:132]    # [C, keig]
            # spectral stat terms:
            # s_sum[c] = sum_k u1[k] P[k,c];  s_sq[c] = sum_k P[k,c](G P)[k,c]
            # cross[c] = sum_k M1g[c,k] P[k,c]
            w12 = wp.tile([128, 256], fp32, tag="w12", bufs=1)
            nc.vector.tensor_scalar(out=w12[:, 0:128], in0=Pm,
                                    scalar1=u1col_s[:], scalar2=None,
                                    op0=OP.mult)
            t1_ps = PS_PT()
            nc.tensor.matmul(t1_ps[:], lhsT=gmat_s[:], rhs=Pm,
                             start=True, stop=True)
            nc.vector.tensor_tensor(w12[:, 128:256], t1_ps[:], Pm, OP.mult)
            # column sums: ones-matmul -> [1,256] row, then row -> two cols
            r_ps = PS_MM()
            nc.tensor.matmul(r_ps[0:1, 0:256], lhsT=onesf_s[:], rhs=w12[:],
                             start=True, stop=True)
            rowbuf = wp.tile([1, 256], fp32, tag="rowbuf")
            nc.vector.tensor_copy(rowbuf[:], r_ps[0:1, 0:256])
            c_ps = PS_PT()
            nc.tensor.matmul(c_ps[:, 0:1], lhsT=rowbuf[:, 0:128], rhs=one1_s[:],
                             start=True, stop=True, skip_group_check=True)
            nc.tensor.matmul(c_ps[:, 1:2], lhsT=rowbuf[:, 128:256], rhs=one1_s[:],
                             start=True, stop=True, skip_group_check=True)
            s_cols = wp.tile([128, 2], fp32, tag="s_cols")
            nc.vector.tensor_copy(s_cols[:], c_ps[:, 0:2])
            # cross: transpose P, multiply with M1g, reduce
            pt_ps = PS_PT()
            nc.tensor.transpose(pt_ps[:], sr12[:, 132:260], idm_s[:])
            ptm = wp.tile([128, 128], fp32, tag="w1t")
            nc.vector.tensor_tensor(ptm[:], pt_ps[:], M1g, OP.mult)
            crossc = statc[:, 14:15]
            nc.vector.tensor_reduce(crossc, ptm[:], mybir.AxisListType.X, OP.add)
            # BN1 totals: sum1 = sr12[:,0] + s_sum ; sq1 = sr12[:,1] + 2*cross + s_sq
            sum1c = statc[:, 15:16]
            nc.vector.tensor_tensor(sum1c, sr12[:, 0:1], s_cols[:, 0:1], OP.add)
            sq1c = statc[:, 16:17]
            nc.vector.scalar_tensor_tensor(out=sq1c, in0=crossc, scalar=2.0,
                                           in1=sr12[:, 1:2], op0=OP.mult,
                                           op1=OP.add)
            nc.vector.tensor_tensor(sq1c, sq1c, s_cols[:, 1:2], OP.add)

            # affine coefs: A = w/sqrt(var+eps), Bc = b - mu*A
            def bn_affine(sum_col, sq_col, w_col, b_col, a_out, b_out_col):
                mu = statc[:, 8:9]
                nc.vector.tensor_scalar(out=mu, in0=sum_col, scalar1=1.0 / N,
                                        scalar2=None, op0=OP.mult)
                msq = statc[:, 9:10]
                nc.vector.tensor_scalar(out=msq, in0=sq_col, scalar1=1.0 / N,
                                        scalar2=None, op0=OP.mult)
                nvar = statc[:, 10:11]
                nc.vector.scalar_tensor_tensor(out=nvar, in0=mu, scalar=mu,
                                               in1=msq, op0=OP.mult,
                                               op1=OP.subtract)  # mu^2 - msq
                sd = statc[:, 11:12]
                nc.scalar.activation(sd, nvar, AF.Sqrt, bias=bcol(BI_EPSC),
                                     scale=-1.0)
                rsd = statc[:, 12:13]
                nc.vector.reciprocal(rsd, sd)
                nc.vector.tensor_tensor(a_out, rsd, w_col, OP.mult)
                nbc = statc[:, 13:14]
                nc.vector.scalar_tensor_tensor(out=nbc, in0=mu, scalar=a_out,
                                               in1=b_col, op0=OP.mult,
                                               op1=OP.subtract)  # mu*A - b
                nc.vector.tensor_scalar(out=b_out_col, in0=nbc, scalar1=-1.0,
                                        scalar2=None, op0=OP.mult)

            A1, B1 = statc[:, 4:5], statc[:, 5:6]
            A2, B2 = statc[:, 6:7], statc[:, 7:8]
            bn_affine(statc[:, 15:16], statc[:, 16:17], bcol(BI_BN1W), bcol(BI_BN1B), A1, B1)
            bn_affine(sr12[:, 2:3], sr12[:, 3:4], bcol(BI_BN2W), bcol(BI_BN2B), A2, B2)
            B12 = statc[:, 5:6]
            nc.vector.tensor_tensor(B12, B1, B2, OP.add)  # B1 += B2 (in place)

            # ============ Phase 6: out = h1 + h2n; MLP2; BN3 ================
            out2T = h2T   # overwrite pre2 per chunk
            Pmb = wp.tile([128, 128], bf16, tag="pmb16", bufs=1)
            nc.vector.tensor_copy(Pmb[:], sr12[:, 132:260])
            for ch in range(CH):
                sl = slice(ch * 512, (ch + 1) * 512)
                ust = wp.tile([128, 512], bf16, tag="ust")
                nc.scalar.dma_start(ust[:], usT[:, sl])
                pso = PS_MM()
                nc.tensor.matmul(pso[:], lhsT=Pmb[:], rhs=ust[:],
                                 start=True, stop=True)
                t1 = wp.tile([128, 512], fp32, tag="t1")
                nc.scalar.activation(t1[:], chebT[:, sl], AF.Identity,
                                     bias=B12, scale=A1)
                tsp = wp.tile([128, 512], fp32, tag="t1")
                nc.vector.scalar_tensor_tensor(
                    out=tsp[:], in0=pso[:], scalar=A1, in1=t1[:],
                    op0=OP.mult, op1=OP.add)
                outT = wp.tile([128, 512], bf16, tag="outTb", bufs=2)
                nc.vector.scalar_tensor_tensor(
                    out=outT[:], in0=h2T[:, sl], scalar=A2, in1=tsp[:],
                    op0=OP.mult, op1=OP.add)
                pma = PS_MM()
                nc.tensor.matmul(pma[:], lhsT=mw1_s[:, 0:128], rhs=outT[:],
                                 start=True, stop=True)
                mida = wp.tile([128, 512], bf16, tag="midab")
                nc.scalar.activation(mida[:], pma[:], AF.Relu, bias=bcol(BI_M1A))
                pmb = PS_MM()
                nc.tensor.matmul(pmb[:], lhsT=mw1_s[:, 128:256], rhs=outT[:],
                                 start=True, stop=True)
                midb = wp.tile([128, 512], bf16, tag="midab")
                nc.scalar.activation(midb[:], pmb[:], AF.Relu, bias=bcol(BI_M1B))
                pmo = PS_MM()
                nc.tensor.matmul(pmo[:], lhsT=mw2a_s[:], rhs=mida[:],
                                 start=True, stop=False)
                nc.tensor.matmul(pmo[:], lhsT=mw2b_s[:], rhs=midb[:],
                                 start=False, stop=True)
                # out2 = out + mlp_b2 + psum
                nc.vector.scalar_tensor_tensor(
                    out=out2T[:, sl], in0=outT[:], scalar=bcol(BI_M2),
                    in1=pmo[:], op0=OP.add, op1=OP.add)
            # BN3 stats
            nc.vector.tensor_reduce(statc[:, 0:1], out2T[:], mybir.AxisListType.X, OP.add)
            sumsq(out2T, statc[:, 1:2])
            st3 = wp.tile([128, 2], fp32, tag="st")
            nc.vector.tensor_copy(st3[:], statc[:, 0:2])
            nc.sync.dma_start(bn3_in[:], st3[:])
            nc.gpsimd.collective_compute(
                "AllReduce", OP.add, replica_groups=[list(range(NCORES))],
                ins=[bn3_in.opt()], outs=[bn3_out.opt()])
            sr3 = wp.tile([128, 2], fp32, tag="st")
            nc.sync.dma_start(sr3[:], bn3_out[:])
            A3, B3 = statc[:, 4:5], statc[:, 5:6]
            bn_affine(sr3[:, 0:1], sr3[:, 1:2], bcol(BI_BN3W), bcol(BI_BN3B), A3, B3)

            # apply BN3, transpose to node-major, write out
            for ch in range(CH):
                sl = slice(ch * 512, (ch + 1) * 512)
                bn3b = wp.tile([128, 512], fp32, tag="t1")
                nc.scalar.activation(bn3b[:], out2T[:, sl], AF.Identity,
                                     bias=B3, scale=A3)
                for j in range(4):
                    t = ch * 4 + j
                    pt = PS_PT()
                    nc.tensor.transpose(pt[:], bn3b[:, j * 128:(j + 1) * 128],
                                        idm_s[:])
                    nc.vector.tensor_copy(outnm[:, t, :], pt[:])
            nc.sync.dma_start(
                out_nm[:].rearrange("(t p) c -> p t c", p=128), outnm[:])
            tail_stack.close()
            late_stack.close()

    nc.compile()
    return nc


def kernel(**inputs):
    inp = {k: np.asarray(v) for k, v in inputs.items()}
    cores, dis, scale, ngrp = _preprocess(inp)

    key = (ngrp, float(scale))
    if key not in _CACHE:
        _CACHE[key] = _build(ngrp, scale)
    nc = _CACHE[key]

    x = inp["x"].astype(F32)
    U = inp["U"].astype(F32)
    s_lam = np.exp(-float(inp["gamma"].reshape(-1)[0]) *
                   inp["Lambda"].astype(np.float64) ** 2).astype(F32)

    wqkv = inp["w_qkv"].astype(F32)
    bqkv = inp["b_qkv"].astype(F32)
    wq = (wqkv[:, :C] / np.sqrt(DH)).astype(F32)
    bq = (bqkv[:C] / np.sqrt(DH)).astype(F32)
    wk, bk = wqkv[:, C:2 * C].copy(), bqkv[C:2 * C]
    wv, bv = wqkv[:, 2 * C:].copy(), bqkv[2 * C:]
    b_out_p = (bv @ inp["w_out"] + inp["b_out"]).astype(F32)

    biasp = np.zeros((128, NBIAS), F32)
    for i, vec in [(BI_SPA1, inp["b_spa1"]), (BI_SPA2, inp["b_spa2"]),
                   (BI_SPE1, inp["b_spe1"]), (BI_SPE2, inp["b_spe2"]),
                   (BI_Q, bq), (BI_K, bk), (BI_OUTP, b_out_p),
                   (BI_CHEB, inp["cheb_b"]),
                   (BI_M1A, inp["mlp_b1"][:128]), (BI_M1B, inp["mlp_b1"][128:]),
                   (BI_M2, inp["mlp_b2"]),
                   (BI_BN1W, inp["bn1_w"]), (BI_BN1B, inp["bn1_b"]),
                   (BI_BN2W, inp["bn2_w"]), (BI_BN2B, inp["bn2_b"]),
                   (BI_BN3W, inp["bn3_w"]), (BI_BN3B, inp["bn3_b"]),
                   (BI_EPSC, np.full(128, EPS, F32))]:
        biasp[:, i] = vec.astype(F32)

    chebw_cols = np.concatenate([inp["cheb_w"][k].astype(F32) for k in range(K)],
                                axis=1)  # [128, 5*128]

    common = dict(
        wspa1=inp["w_spa1"].astype(BF16), wspa2=inp["w_spa2"].astype(BF16),
        wspe1=inp["w_spe1"].astype(BF16), wspe2=inp["w_spe2"].astype(BF16),
        wproj=inp["w_proj"].astype(BF16), chebw=chebw_cols.astype(BF16),
        wq=wq.astype(BF16), wk=wk.astype(BF16), wv=wv.astype(BF16),
        wout=inp["w_out"].astype(BF16),
        mw1=inp["mlp_w1"].astype(BF16),
        mw2a=inp["mlp_w2"][:128].astype(BF16), mw2b=inp["mlp_w2"][128:].astype(BF16),
        biasp=biasp,
        bqh=np.ascontiguousarray(bq.reshape(H, DH).T),
        bkh=np.ascontiguousarray(bk.astype(F32).reshape(H, DH).T),
        idm=np.eye(128, dtype=F32), idmb=np.eye(128, dtype=BF16),
        gmat=None, u1col=None, onesf=np.ones((128, 1), F32),
        one1=np.ones((1, 1), F32),
    )

    vs = VSCALE if FP8V else 1.0
    Us_full = (U * s_lam[None, :]).astype(F32)
    dis_cl = np.where(dis > 0, dis, 1.0).astype(F32)
    xfT_np = np.ascontiguousarray(x.T).astype(BF16)
    discf_np = np.ascontiguousarray((vs * dis_cl).reshape(N // 128, 128).T)
    gmat_np = (Us_full.T @ Us_full).astype(F32)
    u1_np = np.ascontiguousarray(Us_full.sum(0).astype(F32)[:, None])
    in_maps = []
    for c in range(NCORES):
        sl = slice(c * NLOC, (c + 1) * NLOC)
        dis_c = dis[sl]
        m = dict(common)
        m["xT"] = np.ascontiguousarray(x[sl].T).astype(BF16)
        m["u_nm"] = np.ascontiguousarray(U[sl]).astype(BF16)
        m["usT"] = np.ascontiguousarray((U[sl] * s_lam[None, :]).T).astype(BF16)
        m["usd"] = np.ascontiguousarray(
            Us_full[sl] / dis_cl[sl][:, None]).astype(BF16)
        m["gmat"] = gmat_np
        m["u1col"] = u1_np
        m["xfT"] = xfT_np
        m["discf"] = discf_np
        us_loc = Us_full[sl]
        m["m1x"] = np.ascontiguousarray(
            x[sl].T @ us_loc
            + np.outer(inp["cheb_b"].astype(F32), us_loc.sum(0)))
        m["disc"] = np.ascontiguousarray(
            (vs * dis_cl[sl]).reshape(NT, 128).T)
        m["disc_m"] = np.ascontiguousarray(
            (-scale / vs * dis_c).reshape(NT, 128).T)
        m["gidx"] = cores[c]["gidx"]
        m["s8"] = cores[c]["s8"]
        in_maps.append(m)

    import os
    global LAST_NC, LAST_IN_MAPS
    LAST_NC = nc
    LAST_IN_MAPS = in_maps
    trace = os.environ.get("KERNEL_TRACE", "0") == "1"
    res = run_bass_kernel_spmd(nc, in_maps, core_ids=list(range(NCORES)),
                               trace=trace)
    global LAST_EXEC_NS, LAST_RESULT
    LAST_EXEC_NS = res.exec_time_ns
    LAST_RESULT = res
    out = np.concatenate([res.results[c]["out_nm"] for c in range(NCORES)], axis=0)
    return out.astype(inp["x"].dtype)



# revision 40
# speedup vs baseline: 1.3372x; 1.0653x over previous
"""Trainium2 Bass kernel for nn_CachedSpectralGPSLayer (8-core SPMD).

Self-contained: takes FULL inputs, shards per-core internally, runs one
Bass/Tile program SPMD on 8 NeuronCores, gathers the full output.
"""
import os
import sys

sys.path.insert(0, "/opt/trn_rl_repo")

import numpy as np
import ml_dtypes

import concourse.bacc as bacc
import concourse.bass as bass
import concourse.mybir as mybir
import concourse.tile as tile
from concourse import library_config
from concourse.bass_utils import run_bass_kernel_spmd

BF16 = ml_dtypes.bfloat16
FP8 = ml_dtypes.float8_e4m3
F32 = np.float32

N, C, K, KEIG, B, NG, H = 32768, 128, 5, 128, 64, 512, 4
NCORES = 8
NLOC = N // NCORES          # 4096
NT = NLOC // 128            # 32 node tiles per core
Bd = 64                     # dst nodes per block
NBLK = NLOC // Bd           # 64 blocks per core
GPC = B // NCORES           # 8 graphs per core
DH = C // H                 # 32
EPS = 1e-5
BPG = 4                     # blocks per dma_gather call
NCALL = NBLK // BPG         # 16 gather calls per hop
FP8V = False                # gather/AllGather payload (v~) in fp8e4m3
VSCALE = 16.0               # v~ pre-scale (keeps fp8 out of subnormals)
# virtual-schedule stamps (ms units): hop start = ST0 + STH*(h-1),
# window = hop + STW, tail = ST0 + STH*hops. Scheduler ordering hints only.
ST0 = float(os.environ.get("K_ST0", "0.45"))
STH = float(os.environ.get("K_STH", "0.40"))
STW = float(os.environ.get("K_STW", "0.20"))

fp32 = mybir.dt.float32
f32r = mybir.dt.float32r
bf16 = mybir.dt.bfloat16
fp8 = mybir.dt.float8e4
i16 = mybir.dt.int16


def R(ap):
    """Bitcast an fp32 AP to float32r: bit-identical fp32 data, but the PE
    runs replicated mode (1 cyc/row when moving dim >=256 vs 4 for fp32)."""
    return ap.bitcast(f32r)

# bias-pack column indices
(BI_SPA1, BI_SPA2, BI_SPE1, BI_SPE2, BI_Q, BI_K, BI_OUTP, BI_CHEB,
 BI_M1A, BI_M1B, BI_M2, BI_BN1W, BI_BN1B, BI_BN2W, BI_BN2B, BI_BN3W,
 BI_BN3B, BI_EPSC) = range(18)
NBIAS = 18

_CACHE = {}


def _wrap_idx(idx_flat):
    """dma_gather wrapped layout per call: idx i -> [i%16, i//16], replicated
    to all 8 groups of 16 partitions. idx_flat: [ncalls, n_per_call]."""
    ncalls, npc = idx_flat.shape
    base = idx_flat.reshape(ncalls, npc // 16, 16).transpose(0, 2, 1)  # [ncalls,16,npc/16]
    out = np.tile(base, (1, 8, 1))                                     # [ncalls,128,npc/16]
    return np.concatenate(list(out), axis=1)                           # [128, ncalls*npc/16]


def _preprocess(inputs):
    src = np.asarray(inputs["edge_index"][0]).astype(np.int64)
    dst = np.asarray(inputs["edge_index"][1]).astype(np.int64)
    deg = np.bincount(src, minlength=N).astype(np.float64)
    dis = np.where(deg > 0, 1.0 / np.sqrt(deg), 0.0).astype(F32)
    lam = float(np.asarray(inputs["lambda_max"]).reshape(-1)[0])
    scale = 2.0 / lam

    order = np.argsort(dst, kind="stable")
    srcs, dsts = src[order], dst[order]
    counts = np.bincount(dst // Bd, minlength=N // Bd)
    ngrp = int(np.ceil(counts.max() / 128))
    epb = ngrp * 128                    # padded edges per block
    epad = NBLK * epb                   # per core

    cores = []
    bounds = np.searchsorted(dsts, np.arange(0, N + 1, NLOC))
    for c in range(NCORES):
        lo = c * NLOC
        sl = slice(bounds[c], bounds[c + 1])
        sc, dc = srcs[sl], dsts[sl] - lo
        blk = dc // Bd
        ord2 = np.lexsort((sc, blk))  # sort by src within each dst block
        sc, dc, blk = sc[ord2], dc[ord2], blk[ord2]
        cnt = np.bincount(blk, minlength=NBLK)
        csum = np.concatenate([[0], np.cumsum(cnt)])
        pos_in_blk = np.arange(len(sc)) - csum[blk]
        slot = blk * epb + pos_in_blk
        src_pad = np.zeros(epad, np.int64)
        src_pad[slot] = sc
        # S one-hot fp8 bytes [128, ngroups*Bd]
        ngroups = NBLK * ngrp
        s8 = np.zeros((128, ngroups * Bd), np.uint8)
        g = slot // 128
        p = slot % 128
        s8[p, g * Bd + (dc - blk * Bd)] = 0x38  # fp8e4m3 1.0
        assert src_pad.max() < 2 ** 15
        gidx = _wrap_idx(src_pad.astype(np.int16).reshape(NCALL, BPG * epb))
        cores.append(dict(s8=s8.view(FP8), gidx=gidx))
    return cores, dis, scale, ngrp


def _build(ngrp, scale, hops=K - 1, do_mha=True, do_spec=True):
    do_spec = True
    """Build + compile the SPMD Bass program. Returns (nc, input names)."""
    epb = ngrp * 128
    epad = NBLK * epb
    ngroups = NBLK * ngrp

    nc = bacc.Bacc("TRN2", target_bir_lowering=False, debug=False,
                   enable_asserts=True, num_devices=NCORES,
                   num_swdge_queues=4)

    def din(name, shape, dt):
        return nc.dram_tensor(name, shape, dt, kind="ExternalInput").ap()

    xT = din("xT", [128, NLOC], bf16)
    u_nm = din("u_nm", [NLOC, 128], bf16)
    usT = din("usT", [128, NLOC], bf16)
    disc = din("disc", [128, NT], fp32)
    disc_m = din("disc_m", [128, NT], fp32)
    gidx = din("gidx", [128, epad // 16], i16)
    s8 = din("s8", [128, ngroups * Bd], fp8)
    wspa1 = din("wspa1", [128, 128], bf16)
    wspa2 = din("wspa2", [128, 128], bf16)
    wspe1 = din("wspe1", [128, 128], bf16)
    wspe2 = din("wspe2", [128, 128], bf16)
    wproj = din("wproj", [128, 128], bf16)
    chebw = din("chebw", [128, K * 128], bf16)
    wq = din("wq", [128, 128], bf16)
    wk = din("wk", [128, 128], bf16)
    wv = din("wv", [128, 128], bf16)
    wout = din("wout", [128, 128], bf16)
    mw1 = din("mw1", [128, 256], bf16)
    mw2a = din("mw2a", [128, 128], bf16)
    mw2b = din("mw2b", [128, 128], bf16)
    biasp = din("biasp", [128, NBIAS], fp32)
    bqh = din("bqh", [32, H], fp32)
    bkh = din("bkh", [32, H], fp32)
    idm = din("idm", [128, 128], fp32)
    gmat = din("gmat", [128, 128], fp32)
    u1col = din("u1col", [128, 1], fp32)
    usd = din("usd", [NLOC, 128], bf16)
    idmb = din("idmb", [128, 128], bf16)
    onesf = din("onesf", [128, 1], fp32)
    one1 = din("one1", [1, 1], fp32)
    m1x = din("m1x", [128, 128], fp32)
    xfT = din("xfT", [128, N], bf16)
    discf = din("discf", [128, N // 128], fp32)

    out_nm = nc.dram_tensor("out_nm", [NLOC, 128], fp32, kind="ExternalOutput").ap()

    AF = mybir.ActivationFunctionType
    OP = mybir.AluOpType

    with tile.TileContext(nc) as tc:
        with tc.tile_pool(name="const", bufs=1) as cp, \
             tc.tile_pool(name="big", bufs=1) as bp, \
             tc.tile_pool(name="work", bufs=2) as wp, \
             tc.tile_pool(name="psmm", bufs=2, space="PSUM") as psmm, \
             tc.tile_pool(name="psat", bufs=2, space="PSUM") as psatp, \
             tc.tile_pool(name="pspt", bufs=2, space="PSUM") as psptp, \
             tc.tile_pool(name="psagg", bufs=2, space="PSUM") as psaggp, \
             tc.tile_pool(name="dram", bufs=1, space="DRAM") as dp:

            # uniform-tag psum allocators (PSUM = 8 banks total: 2+2+2+2)
            def PS_MM():   # transient [128,512] matmul outputs
                return psmm.tile([128, 512], fp32, tag="mm", name="psmm_t")

            def PS_AT(shape):  # long-lived accumulators / phase-2 partial
                return psatp.tile(shape, fp32, tag="at", name="psat_t")

            def PS_ATW():  # MHA attn+denom accumulator [128, 132]
                return psatp.tile([128, 33 * H], fp32, tag="at", name="psatw_t")

            def PS_PT2():  # [32, 512] head q/k psum
                return psptp.tile([32, 512], fp32, tag="pt", name="pspt2_t")

            def PS_PT():   # [128,128] transposes / small matmuls
                return psptp.tile([128, 128], fp32, tag="pt", name="pspt_t")

            def PS_AGG():  # [128,128] cheb aggregation
                return psaggp.tile([128, 128], fp32, tag="agg", name="psagg_t")

            nc.gpsimd.load_library(library_config.mlp)

            # ---- load constants ----
            def ld(ap_in, shape, dt, name, eng=None):
                t = cp.tile(shape, dt, tag=name, name=name)
                (eng or nc.sync).dma_start(t[:], ap_in[:])
                return t

            xT_s = bp.tile([128, NLOC], bf16, tag="xT")
            nc.sync.dma_start(xT_s[:], xT[:])
            s8_s = ld(s8, [128, ngroups * Bd], fp8, "s8")
            wspa1_s = ld(wspa1, [128, 128], bf16, "wspa1")
            wspa2_s = ld(wspa2, [128, 128], bf16, "wspa2")
            wspe1_s = ld(wspe1, [128, 128], bf16, "wspe1")
            wspe2_s = ld(wspe2, [128, 128], bf16, "wspe2")
            wproj_s = ld(wproj, [128, 128], bf16, "wproj")
            chebw_s = ld(chebw, [128, K * 128], bf16, "chebw")
            wq_s = ld(wq, [128, 128], bf16, "wq")
            wk_s = ld(wk, [128, 128], bf16, "wk")
            wv_s = ld(wv, [128, 128], bf16, "wv")
            wout_s = ld(wout, [128, 128], bf16, "wout")
            mw1_s = ld(mw1, [128, 256], bf16, "mw1")
            mw2a_s = ld(mw2a, [128, 128], bf16, "mw2a")
            mw2b_s = ld(mw2b, [128, 128], bf16, "mw2b")
            biasp_s = ld(biasp, [128, NBIAS], fp32, "biasp")
            bqh_s = ld(bqh, [32, H], fp32, "bqh")
            bkh_s = ld(bkh, [32, H], fp32, "bkh")
            idm_s = ld(idm, [128, 128], fp32, "idm")
            idmb_s = ld(idmb, [128, 128], bf16, "idmb")
            usd_s = bp.tile([128, NT, 128], bf16, tag="usd")
            nc.sync.dma_start(usd_s[:], usd[:].rearrange("(t p) c -> p t c", p=128))
            vbuf = bp.tile([128, NT, 128], bf16, tag="vbuf")
            gmat_s = ld(gmat, [128, 128], fp32, "gmat")
            u1col_s = ld(u1col, [128, 1], fp32, "u1col")
            onesf_s = ld(onesf, [128, 1], fp32, "onesf")
            one1_s = ld(one1, [1, 1], fp32, "one1")
            m1x_s = ld(m1x, [128, 128], fp32, "m1x")
            m1sb = cp.tile([128, 128], fp32, tag="m1sb", name="m1sb")
            discf_s = ld(discf, [128, N // 128], fp32, "discf")
            disc_s = ld(disc, [128, NT], fp32, "disc")
            discm_s = ld(disc_m, [128, 2 * NT], fp32, "discm")
            discm2_s = discm_s[:, NT:2 * NT]

            def bcol(i):
                return biasp_s[:, i:i + 1]

            # persistent big buffers
            TxA = bp.tile([128, NT, 128], fp32, tag="TxA")   # node-major
            TxB = bp.tile([128, NT, 128], fp32, tag="TxB")
            chebT = bp.tile([128, NLOC], fp32, tag="chebT")  # later: pre1, outT
            h2T = bp.tile([128, NLOC], fp32, tag="h2T")      # later: pre2, out2T
            if not do_mha:
                nc.vector.memset(h2T[:], 0.0)
            statc = cp.tile([128, 32], fp32, tag="statc")    # stats/affine cols
            statc2 = cp.tile([128, 16], fp32, tag="statc2")  # per-graph h2 stats
            outnm = TxA  # reuse (dead after cheb)

            # DRAM bounce buffers
            vdt = fp8 if FP8V else bf16
            ag_in = dp.tile([NLOC, 128], vdt, tag="ag_in")
            ag_outs = [None] + [dp.tile([N, 128], vdt, tag=f"ag_out{h}", name=f"ag_out{h}", addr_space="Shared") for h in range(1, 4)]
            vfull0 = dp.tile([N, 128], vdt, tag="vfull0", name="vfull0")
            bn12_in = dp.tile([128, 260], fp32, tag="bn12_in")
            bn12_out = dp.tile([128, 260], fp32, tag="bn12_out", addr_space="Shared")
            bn3_in = dp.tile([128, 2], fp32, tag="bn3_in")
            bn3_out = dp.tile([128, 2], fp32, tag="bn3_out", addr_space="Shared")

            CH = NLOC // 512  # 8 chunks of 512

            from contextlib import ExitStack
            ep_stack = ExitStack()
            ep = ep_stack.enter_context(tc.tile_pool(name="early", bufs=1))
            xspT = ep.tile([128, NLOC], bf16, tag="xspT", name="xspT")

            # ================= Phase 1: local spa MLP (feature-major) =======
            # (the spe MLP + spectral partial run inside AllGather window 1,
            # recomputed from the resident xT_s, off the pre-hop critical path)
            for ch in range(CH):
                sl = slice(ch * 512, (ch + 1) * 512)
                p1 = PS_MM()
                nc.tensor.matmul(p1[:], lhsT=wspa1_s[:], rhs=xT_s[:, sl],
                                 start=True, stop=True)
                t1 = wp.tile([128, 512], bf16, tag="t1b")
                nc.scalar.activation(t1[:], p1[:], AF.Relu, bias=bcol(BI_SPA1))
                p2 = PS_MM()
                nc.tensor.matmul(p2[:], lhsT=wspa2_s[:], rhs=t1[:],
                                 start=True, stop=True)
                nc.scalar.activation(xspT[:, sl], p2[:], AF.Identity,
                                     bias=bcol(BI_SPA2))

            # Tx0 node-major (local shard, for recurrence) + v~0 into vbuf
            for t in range(NT):
                tsl = slice(t * 128, (t + 1) * 128)
                pt = PS_PT()
                nc.tensor.matmul(pt[:], lhsT=xspT[:, tsl], rhs=idmb_s[:],
                                 start=True, stop=True)
                nc.vector.tensor_copy(TxB[:, t, :], pt[:])
                nc.scalar.activation(vbuf[:, t, :], pt[:], AF.Identity,
                                     scale=disc_s[:, t:t + 1])
            # v~0 for ALL nodes computed locally (replaces hop-1 AllGather):
            # every core redundantly runs the spatial MLP over the full x.
            if hops >= 1:
                for gch in range(N // 512):
                    gsl2 = slice(gch * 512, (gch + 1) * 512)
                    xc = wp.tile([128, 512], bf16, tag="t1b")
                    nc.sync.dma_start(xc[:], xfT[:, gsl2])
                    pf1 = PS_MM()
                    nc.tensor.matmul(pf1[:], lhsT=wspa1_s[:], rhs=xc[:],
                                     start=True, stop=True)
                    tf1 = wp.tile([128, 512], bf16, tag="midab")
                    nc.scalar.activation(tf1[:], pf1[:], AF.Relu,
                                         bias=bcol(BI_SPA1))
                    pf2 = PS_MM()
                    nc.tensor.matmul(pf2[:], lhsT=wspa2_s[:], rhs=tf1[:],
                                     start=True, stop=True)
                    spf = wp.tile([128, 512], bf16, tag="t1b")
                    nc.scalar.activation(spf[:], pf2[:], AF.Identity,
                                         bias=bcol(BI_SPA2))
                    vt4 = wp.tile([128, 4, 128], vdt, tag="vt4", bufs=2)
                    for j in range(4):
                        tg = gch * 4 + j
                        ptf = PS_PT()
                        nc.tensor.matmul(ptf[:], lhsT=spf[:, j * 128:(j + 1) * 128],
                                         rhs=idmb_s[:], start=True, stop=True)
                        nc.scalar.activation(vt4[:, j, :], ptf[:], AF.Identity,
                                             scale=discf_s[:, tg:tg + 1])
                    nc.sync.dma_start(
                        vfull0[gch * 512:(gch + 1) * 512, :]
                        .rearrange("(t p) c -> p t c", p=128), vt4[:])

            for ch in range(CH):
                sl = slice(ch * 512, (ch + 1) * 512)
                pw = PS_MM()
                nc.tensor.matmul(pw[:], lhsT=chebw_s[:, 0:128], rhs=xspT[:, sl],
                                 start=True, stop=True)
                nc.vector.tensor_copy(chebT[:, sl], pw[:])
            # T_0 = v~0^T (Us/dis) (accumulate over tiles); m1sb = m1x + W0^T T_0
            t_ps = PS_AGG()
            for t in range(NT):
                nc.tensor.matmul(t_ps[:], lhsT=vbuf[:, t, :], rhs=usd_s[:, t, :],
                                 start=(t == 0), stop=(t == NT - 1))
            tsb = wp.tile([128, 128], bf16, tag="tsb")
            nc.vector.tensor_copy(tsb[:], t_ps[:])
            pWt = PS_PT()
            nc.tensor.matmul(pWt[:], lhsT=chebw_s[:, 0:128], rhs=tsb[:],
                             start=True, stop=True)
            nc.vector.tensor_add(m1sb[:], m1x_s[:], pWt[:])

            # ===== Phase 2 (deferred): spectral partial, run in AG window 1 =
            # Recomputes the spe MLP chunk-wise from resident xT_s so xspecT
            # needs no SBUF residency across the hops.
            part_s = wp.tile([128, 128], fp32, tag="part_s", bufs=1)

            def spectral_partial():
                part_ps = PS_AT([128, 128])
                for ch2 in range(CH):
                    sl2 = slice(ch2 * 512, (ch2 + 1) * 512)
                    p3 = PS_MM()
                    nc.tensor.matmul(p3[:], lhsT=wspe1_s[:], rhs=xT_s[:, sl2],
                                     start=True, stop=True)
                    t2 = wp.tile([128, 512], bf16, tag="t1b")
                    nc.scalar.activation(t2[:], p3[:], AF.Relu,
                                         bias=bcol(BI_SPE1))
                    p4 = PS_MM()
                    nc.tensor.matmul(p4[:], lhsT=wspe2_s[:], rhs=t2[:],
                                     start=True, stop=True)
                    xsp2 = wp.tile([128, 512], bf16, tag="t1b")
                    nc.scalar.activation(xsp2[:], p4[:], AF.Identity,
                                         bias=bcol(BI_SPE2))
                    ub = wp.tile([128, 4, 128], bf16, tag="ut4", bufs=1)
                    nc.sync.dma_start(
                        ub[:], u_nm[ch2 * 512:(ch2 + 1) * 512, :]
                        .rearrange("(t p) c -> p t c", p=128))
                    for j in range(4):
                        t = ch2 * 4 + j
                        ph = PS_PT()
                        nc.tensor.matmul(ph[:], lhsT=xsp2[:, j * 128:(j + 1) * 128],
                                         rhs=wproj_s[:], start=True, stop=True)
                        hp = wp.tile([128, 128], bf16, tag="hp")
                        nc.vector.tensor_copy(hp[:], ph[:])
                        nc.tensor.matmul(part_ps[:], lhsT=ub[:, j, :], rhs=hp[:],
                                         start=(t == 0), stop=(t == NT - 1))
                nc.vector.tensor_copy(part_s[:], part_ps[:])

            ep_stack.close()  # free xspT space for later pools
            late_stack = ExitStack()
            gp = late_stack.enter_context(tc.tile_pool(name="gath", bufs=2))
            mp = late_stack.enter_context(tc.tile_pool(name="mha", bufs=2))

            # ---- MHA for one graph (interleaved into AllGather windows) ----
            def mha_graph(g):
                gsl = slice(g * 512, (g + 1) * 512)
                # head-major q/k: per-head matmuls so all operands are base-0
                qT = mp.tile([32, H * 512], bf16, tag="qT", bufs=1)
                kT = mp.tile([32, H * 512], bf16, tag="kT", bufs=1)
                for hh in range(H):
                    csl = slice(hh * 32, (hh + 1) * 32)
                    pqh = PS_PT2()
                    nc.tensor.matmul(pqh[:], lhsT=wq_s[:, csl],
                                     rhs=xT_s[:, gsl], start=True, stop=True)
                    nc.scalar.activation(qT[:, hh * 512:(hh + 1) * 512], pqh[:],
                                         AF.Identity, bias=bqh_s[:, hh:hh + 1])
                    pkh = PS_PT2()
                    nc.tensor.matmul(pkh[:], lhsT=wk_s[:, csl],
                                     rhs=xT_s[:, gsl], start=True, stop=True)
                    nc.scalar.activation(kT[:, hh * 512:(hh + 1) * 512], pkh[:],
                                         AF.Identity, bias=bkh_s[:, hh:hh + 1])
                # v node-major, augmented per head with a ones column
                vaug = mp.tile([128, 4, 33 * H], bf16, tag="vaug")
                nc.vector.memset(vaug[:, :, 32::33], 1.0)
                for j in range(4):
                    pv = PS_PT()
                    nc.tensor.matmul(pv[:], lhsT=xT_s[:, g * 512 + j * 128:
                                                      g * 512 + (j + 1) * 128],
                                     rhs=wv_s[:], start=True, stop=True)
                    for hh in range(H):
                        nc.vector.tensor_copy(
                            vaug[:, j, hh * 33:hh * 33 + 32],
                            pv[:, hh * 32:(hh + 1) * 32])
                # scores_T + exp, per (head, k-chunk)
                ess = {}
                for hh in range(H):
                    qsl = slice(hh * 512, (hh + 1) * 512)
                    for j in range(4):
                        pss = PS_MM()
                        nc.tensor.matmul(
                            pss[:], lhsT=kT[:, hh * 512 + j * 128:
                                            hh * 512 + (j + 1) * 128],
                            rhs=qT[:, qsl], start=True, stop=True)
                        es = mp.tile([128, 512], bf16, tag="es", bufs=16)
                        nc.scalar.activation(es[:], pss[:], AF.Exp)
                        ess[(hh, j)] = es
                # attn + denom per q-chunk
                for qc in range(4):
                    pat = PS_ATW()
                    for hh in range(H):
                        for j in range(4):
                            nc.tensor.matmul(
                                pat[:, hh * 33:(hh + 1) * 33],
                                lhsT=ess[(hh, j)][:, qc * 128:(qc + 1) * 128],
                                rhs=vaug[:, j, hh * 33:(hh + 1) * 33],
                                start=(j == 0), stop=(j == 3),
                                skip_group_check=True)
                    recip = wp.tile([128, 4], fp32, tag="recip")
                    nc.vector.reciprocal(recip[:], pat[:, 32::33])
                    anm = wp.tile([128, 128], fp32, tag="anm")
                    for hh in range(H):
                        nc.vector.tensor_scalar(
                            out=anm[:, hh * 32:(hh + 1) * 32],
                            in0=pat[:, hh * 33:hh * 33 + 32],
                            scalar1=recip[:, hh:hh + 1], scalar2=None,
                            op0=OP.mult)
                    ptr = PS_PT()
                    nc.tensor.transpose(ptr[:], anm[:], idm_s[:])
                    attnT = wp.tile([128, 128], bf16, tag="attnT", bufs=1)
                    nc.vector.tensor_copy(attnT[:], ptr[:])
                    ph2 = PS_PT()
                    nc.tensor.matmul(ph2[:], lhsT=wout_s[:], rhs=attnT[:],
                                     start=True, stop=True)
                    osl = slice(g * 512 + qc * 128, g * 512 + (qc + 1) * 128)
                    # pre2 = h2 + b_out' + x
                    nc.vector.scalar_tensor_tensor(
                        out=h2T[:, osl], in0=ph2[:], scalar=bcol(BI_OUTP),
                        in1=xT_s[:, osl], op0=OP.add, op1=OP.add)
                # incremental BN2 stats for this graph's 512 columns
                nc.vector.tensor_reduce(statc2[:, g:g + 1], h2T[:, gsl],
                                        mybir.AxisListType.X, OP.add)
                tsq = wp.tile([128, 512], fp32, tag="sqt", bufs=1)
                nc.vector.scalar_tensor_tensor(
                    out=tsq[:], in0=h2T[:, gsl], scalar=1.0, in1=h2T[:, gsl],
                    op0=OP.mult, op1=OP.mult,
                    accum_out=statc2[:, 8 + g:9 + g])

            # graphs run inside AllGather wait windows (PE idle otherwise)
            mha_sched = {1: [0, 1, 2], 2: [3, 4, 5], 3: [6, 7]} \
                if (do_mha and hops == K - 1) else {}
            mha_left = [g for g in range(GPC if do_mha else 0)
                        if not any(g in v for v in mha_sched.values())]

            # ================= Phase 3: cheb hops ===========================
            # tile_wait_until stamps are scheduler-only hints (virtual
            # earliest-start): they stop the list scheduler from hoisting
            # hop h+1's recurrence ops ahead of the window-h MHA work in
            # the in-order DVE queue (head-of-line blocking during the
            # AllGather). They emit no HW waits.
            cur, prev = TxB, TxA  # cur holds Tx_{h-1}; prev gets Tx_h
            for h in range(1, 1 + hops):
                t_hop = ST0 + STH * (h - 1)
                ag_src = vfull0 if h == 1 else ag_outs[h - 1]
                hop_stack = ExitStack()
                hop_stack.enter_context(tc.tile_wait_until(t_hop))
                for q in range(NCALL):
                    gt = gp.tile([128, BPG * ngrp, 128], vdt, tag="gt")
                    isl = slice(q * BPG * epb // 16, (q + 1) * BPG * epb // 16)
                    gix = wp.tile([128, BPG * epb // 16], i16, tag="gix", bufs=4)
                    nc.sync.dma_start(gix[:], gidx[:, isl])
                    nc.gpsimd.dma_gather(gt[:], ag_src[:], gix[:],
                                         BPG * epb, BPG * epb, 128,
                                         single_packet=False,
                                         queue_num=q % 4)
                    for r in range(BPG):
                        b = q * BPG + r
                        t, half = b // 2, b % 2
                        if half == 0:
                            aps = PS_AGG()
                        for j in range(ngrp):
                            gcol = b * ngrp + j
                            nc.tensor.matmul(
                                aps[half * 64:(half + 1) * 64, :],
                                lhsT=s8_s[:, gcol * Bd:(gcol + 1) * Bd],
                                rhs=gt[:, r * ngrp + j, :],
                                start=(j == 0), stop=(j == ngrp - 1))
                        if half == 1:
                            # recurrence for tile t
                            tmp = wp.tile([128, 128], fp32, tag="rectmp")
                            if h == 1:
                                nc.vector.tensor_scalar(
                                    out=tmp[:], in0=aps[:],
                                    scalar1=discm_s[:, t:t + 1], scalar2=None,
                                    op0=OP.mult)
                                # Tx1 = (scale-1)*Tx0 + tmp
                                nc.vector.scalar_tensor_tensor(
                                    out=prev[:, t, :], in0=cur[:, t, :],
                                    scalar=float(scale - 1.0), in1=tmp[:],
                                    op0=OP.mult, op1=OP.add)
                            else:
                                nc.vector.tensor_scalar(
                                    out=tmp[:], in0=aps[:],
                                    scalar1=discm_s[:, t:t + 1], scalar2=2.0,
                                    op0=OP.mult, op1=OP.mult)
                                # tmp2 = tmp - Tx_{h-2}
                                tmp2 = wp.tile([128, 128], fp32, tag="rectmp2")
                                nc.vector.scalar_tensor_tensor(
                                    out=tmp2[:], in0=prev[:, t, :],
                                    scalar=-1.0, in1=tmp[:],
                                    op0=OP.mult, op1=OP.add)
                                # Tx_h = 2(scale-1)*Tx_{h-1} + tmp2
                                nc.vector.scalar_tensor_tensor(
                                    out=prev[:, t, :], in0=cur[:, t, :],
                                    scalar=float(2.0 * (scale - 1.0)),
                                    in1=tmp2[:], op0=OP.mult, op1=OP.add)
                            nc.scalar.activation(
                                vbuf[:, t, :], prev[:, t, :], AF.Identity,
                                scale=disc_s[:, t:t + 1])
                            if h < hops:
                                nc.sync.dma_start(
                                    ag_in[t * 128:(t + 1) * 128, :],
                                    vbuf[:, t, :])
                # launch AG for next hop once all v~ tiles written
                if h < hops:
                    nc.gpsimd.collective_compute(
                        "AllGather", OP.bypass,
                        replica_groups=[list(range(NCORES))],
                        ins=[ag_in.opt()], outs=[ag_outs[h].opt()])
                hop_stack.close()
                win_stack = ExitStack()
                win_stack.enter_context(tc.tile_wait_until(t_hop + STW))
                # out_cheb += Tx_h @ W_h  (transpose tiles chunk-wise)
                for ch in range(CH):
                    tpb = wp.tile([128, 512], bf16, tag="tpb", bufs=1)
                    for j in range(4):
                        t = ch * 4 + j
                        pt = PS_PT()
                        nc.tensor.transpose(pt[:], prev[:, t, :], idm_s[:])
                        nc.vector.tensor_copy(tpb[:, j * 128:(j + 1) * 128], pt[:])
                    sl = slice(ch * 512, (ch + 1) * 512)
                    pw = PS_MM()
                    nc.tensor.matmul(pw[:], lhsT=chebw_s[:, h * 128:(h + 1) * 128],
                                     rhs=tpb[:], start=True, stop=True)
                    nc.vector.tensor_add(chebT[:, sl], chebT[:, sl], pw[:])
                # T_h = v~_h^T (Us/dis) ; m1sb += W_h^T T_h  (off the tail)
                t_ps = PS_AGG()
                for t in range(NT):
                    nc.tensor.matmul(t_ps[:], lhsT=vbuf[:, t, :],
                                     rhs=usd_s[:, t, :],
                                     start=(t == 0), stop=(t == NT - 1))
                tsb = wp.tile([128, 128], bf16, tag="tsb")
                nc.vector.tensor_copy(tsb[:], t_ps[:])
                pWt = PS_PT()
                nc.tensor.matmul(pWt[:], lhsT=chebw_s[:, h * 128:(h + 1) * 128],
                                 rhs=tsb[:], start=True, stop=True)
                nc.vector.tensor_add(m1sb[:], m1sb[:], pWt[:])
                if h == 1:
                    spectral_partial()
                for g in mha_sched.get(h, []):
                    mha_graph(g)
                win_stack.close()
                cur, prev = prev, cur

            # ================= Phase 4: MHA (remaining graphs) ==============
            for g in mha_left:
                mha_graph(g)

            tail_stack = ExitStack()
            tail_stack.enter_context(tc.tile_wait_until(ST0 + STH * hops))
            # ===== Phase 5: pre1' (no spec) + BN stats + M1 + joint AR ======
            for ch in range(CH):
                sl = slice(ch * 512, (ch + 1) * 512)
                # pre1' = chebT + cheb_b + x   (overwrite chebT)
                nc.vector.scalar_tensor_tensor(
                    out=chebT[:, sl], in0=chebT[:, sl], scalar=bcol(BI_CHEB),
                    in1=xT_s[:, sl], op0=OP.add, op1=OP.add)
            # BN1/BN2 stats
            def sumsq(buf, out_col):
                for c2 in range(CH):
                    s2 = slice(c2 * 512, (c2 + 1) * 512)
                    tt = wp.tile([128, 512], fp32, tag="sqt", bufs=1)
                    nc.vector.scalar_tensor_tensor(
                        out=tt[:], in0=buf[:, s2], scalar=1.0, in1=buf[:, s2],
                        op0=OP.mult, op1=OP.mult,
                        accum_out=statc[:, 24 + c2:25 + c2])
                nc.vector.tensor_reduce(out_col, statc[:, 24:32],
                                        mybir.AxisListType.X, OP.add)

            nc.vector.tensor_reduce(statc[:, 0:1], chebT[:], mybir.AxisListType.X, OP.add)
            sumsq(chebT, statc[:, 1:2])
            if do_mha:
                nc.vector.tensor_reduce(statc[:, 2:3], statc2[:, 0:8],
                                        mybir.AxisListType.X, OP.add)
                nc.vector.tensor_reduce(statc[:, 3:4], statc2[:, 8:16],
                                        mybir.AxisListType.X, OP.add)
            else:
                nc.vector.tensor_reduce(statc[:, 2:3], h2T[:], mybir.AxisListType.X, OP.add)
                sumsq(h2T, statc[:, 3:4])
            st12 = wp.tile([128, 260], fp32, tag="st12", bufs=1)
            nc.vector.tensor_copy(st12[:, 0:4], statc[:, 0:4])
            nc.vector.tensor_copy(st12[:, 4:132], m1sb[:])
            nc.vector.tensor_copy(st12[:, 132:260], part_s[:])
            nc.sync.dma_start(bn12_in[:], st12[:])
            nc.gpsimd.collective_compute(
                "AllReduce", OP.add, replica_groups=[list(range(NCORES))],
                ins=[bn12_in.opt()], outs=[bn12_out.opt()])
            sr12 = wp.tile([128, 260], fp32, tag="sr12", bufs=1)
            nc.sync.dma_start(sr12[:], bn12_out[:])
            Pm = sr12[:, 132:260]   # AR'd spectral partial [keig, C]
            M1g = sr12[:, 4:132]    # [C, keig]
            # spectral stat terms:
            # s_sum[c] = sum_k u1[k] P[k,c];  s_sq[c] = sum_k P[k,c](G P)[k,c]
            # cross[c] = sum_k M1g[c,k] P[k,c]
            w12 = wp.tile([128, 256], fp32, tag="w12", bufs=1)
            nc.vector.tensor_scalar(out=w12[:, 0:128], in0=Pm,
                                    scalar1=u1col_s[:], scalar2=None,
                                    op0=OP.mult)
            t1_ps = PS_PT()
            nc.tensor.matmul(t1_ps[:], lhsT=gmat_s[:], rhs=Pm,
                             start=True, stop=True)
            nc.vector.tensor_tensor(w12[:, 128:256], t1_ps[:], Pm, OP.mult)
            # column sums: ones-matmul -> [1,256] row, then row -> two cols
            r_ps = PS_MM()
            nc.tensor.matmul(r_ps[0:1, 0:256], lhsT=onesf_s[:], rhs=w12[:],
                             start=True, stop=True)
            rowbuf = wp.tile([1, 256], fp32, tag="rowbuf")
            nc.vector.tensor_copy(rowbuf[:], r_ps[0:1, 0:256])
            c_ps = PS_PT()
            nc.tensor.matmul(c_ps[:, 0:1], lhsT=rowbuf[:, 0:128], rhs=one1_s[:],
                             start=True, stop=True, skip_group_check=True)
            nc.tensor.matmul(c_ps[:, 1:2], lhsT=rowbuf[:, 128:256], rhs=one1_s[:],
                             start=True, stop=True, skip_group_check=True)
            s_cols = wp.tile([128, 2], fp32, tag="s_cols")
            nc.vector.tensor_copy(s_cols[:], c_ps[:, 0:2])
            # cross: transpose P, multiply with M1g, reduce
            pt_ps = PS_PT()
            nc.tensor.transpose(pt_ps[:], sr12[:, 132:260], idm_s[:])
            ptm = wp.tile([128, 128], fp32, tag="w1t")
            nc.vector.tensor_tensor(ptm[:], pt_ps[:], M1g, OP.mult)
            crossc = statc[:, 14:15]
            nc.vector.tensor_reduce(crossc, ptm[:], mybir.AxisListType.X, OP.add)
            # BN1 totals: sum1 = sr12[:,0] + s_sum ; sq1 = sr12[:,1] + 2*cross + s_sq
            sum1c = statc[:, 15:16]
            nc.vector.tensor_tensor(sum1c, sr12[:, 0:1], s_cols[:, 0:1], OP.add)
            sq1c = statc[:, 16:17]
            nc.vector.scalar_tensor_tensor(out=sq1c, in0=crossc, scalar=2.0,
                                           in1=sr12[:, 1:2], op0=OP.mult,
                                           op1=OP.add)
            nc.vector.tensor_tensor(sq1c, sq1c, s_cols[:, 1:2], OP.add)

            # affine coefs: A = w/sqrt(var+eps), Bc = b - mu*A
            def bn_affine(sum_col, sq_col, w_col, b_col, a_out, b_out_col):
                mu = statc[:, 8:9]
                nc.vector.tensor_scalar(out=mu, in0=sum_col, scalar1=1.0 / N,
                                        scalar2=None, op0=OP.mult)
                msq = statc[:, 9:10]
                nc.vector.tensor_scalar(out=msq, in0=sq_col, scalar1=1.0 / N,
                                        scalar2=None, op0=OP.mult)
                nvar = statc[:, 10:11]
                nc.vector.scalar_tensor_tensor(out=nvar, in0=mu, scalar=mu,
                                               in1=msq, op0=OP.mult,
                                               op1=OP.subtract)  # mu^2 - msq
                sd = statc[:, 11:12]
                nc.scalar.activation(sd, nvar, AF.Sqrt, bias=bcol(BI_EPSC),
                                     scale=-1.0)
                rsd = statc[:, 12:13]
                nc.vector.reciprocal(rsd, sd)
                nc.vector.tensor_tensor(a_out, rsd, w_col, OP.mult)
                nbc = statc[:, 13:14]
                nc.vector.scalar_tensor_tensor(out=nbc, in0=mu, scalar=a_out,
                                               in1=b_col, op0=OP.mult,
                                               op1=OP.subtract)  # mu*A - b
                nc.vector.tensor_scalar(out=b_out_col, in0=nbc, scalar1=-1.0,
                                        scalar2=None, op0=OP.mult)

            A1, B1 = statc[:, 4:5], statc[:, 5:6]
            A2, B2 = statc[:, 6:7], statc[:, 7:8]
            bn_affine(statc[:, 15:16], statc[:, 16:17], bcol(BI_BN1W), bcol(BI_BN1B), A1, B1)
            bn_affine(sr12[:, 2:3], sr12[:, 3:4], bcol(BI_BN2W), bcol(BI_BN2B), A2, B2)
            B12 = statc[:, 5:6]
            nc.vector.tensor_tensor(B12, B1, B2, OP.add)  # B1 += B2 (in place)

            # ============ Phase 6: out = h1 + h2n; MLP2; BN3 ================
            out2T = h2T   # overwrite pre2 per chunk
            Pmb = wp.tile([128, 128], bf16, tag="pmb16", bufs=1)
            nc.vector.tensor_copy(Pmb[:], sr12[:, 132:260])
            for ch in range(CH):
                sl = slice(ch * 512, (ch + 1) * 512)
                ust = wp.tile([128, 512], bf16, tag="ust")
                nc.scalar.dma_start(ust[:], usT[:, sl])
                pso = PS_MM()
                nc.tensor.matmul(pso[:], lhsT=Pmb[:], rhs=ust[:],
                                 start=True, stop=True)
                t1 = wp.tile([128, 512], fp32, tag="t1")
                nc.scalar.activation(t1[:], chebT[:, sl], AF.Identity,
                                     bias=B12, scale=A1)
                tsp = wp.tile([128, 512], fp32, tag="t1")
                nc.vector.scalar_tensor_tensor(
                    out=tsp[:], in0=pso[:], scalar=A1, in1=t1[:],
                    op0=OP.mult, op1=OP.add)
                outT = wp.tile([128, 512], bf16, tag="outTb", bufs=2)
                nc.vector.scalar_tensor_tensor(
                    out=outT[:], in0=h2T[:, sl], scalar=A2, in1=tsp[:],
                    op0=OP.mult, op1=OP.add)
                pma = PS_MM()
                nc.tensor.matmul(pma[:], lhsT=mw1_s[:, 0:128], rhs=outT[:],
                                 start=True, stop=True)
                mida = wp.tile([128, 512], bf16, tag="midab")
                nc.scalar.activation(mida[:], pma[:], AF.Relu, bias=bcol(BI_M1A))
                pmb = PS_MM()
                nc.tensor.matmul(pmb[:], lhsT=mw1_s[:, 128:256], rhs=outT[:],
                                 start=True, stop=True)
                midb = wp.tile([128, 512], bf16, tag="midab")
                nc.scalar.activation(midb[:], pmb[:], AF.Relu, bias=bcol(BI_M1B))
                pmo = PS_MM()
                nc.tensor.matmul(pmo[:], lhsT=mw2a_s[:], rhs=mida[:],
                                 start=True, stop=False)
                nc.tensor.matmul(pmo[:], lhsT=mw2b_s[:], rhs=midb[:],
                                 start=False, stop=True)
                # out2 = out + mlp_b2 + psum
                nc.vector.scalar_tensor_tensor(
                    out=out2T[:, sl], in0=outT[:], scalar=bcol(BI_M2),
                    in1=pmo[:], op0=OP.add, op1=OP.add)
            # BN3 stats
            nc.vector.tensor_reduce(statc[:, 0:1], out2T[:], mybir.AxisListType.X, OP.add)
            sumsq(out2T, statc[:, 1:2])
            st3 = wp.tile([128, 2], fp32, tag="st")
            nc.vector.tensor_copy(st3[:], statc[:, 0:2])
            nc.sync.dma_start(bn3_in[:], st3[:])
            nc.gpsimd.collective_compute(
                "AllReduce", OP.add, replica_groups=[list(range(NCORES))],
                ins=[bn3_in.opt()], outs=[bn3_out.opt()])
            sr3 = wp.tile([128, 2], fp32, tag="st")
            nc.sync.dma_start(sr3[:], bn3_out[:])
            A3, B3 = statc[:, 4:5], statc[:, 5:6]
            bn_affine(sr3[:, 0:1], sr3[:, 1:2], bcol(BI_BN3W), bcol(BI_BN3B), A3, B3)

            # apply BN3, transpose to node-major, write out
            for ch in range(CH):
                sl = slice(ch * 512, (ch + 1) * 512)
                bn3b = wp.tile([128, 512], fp32, tag="t1")
                nc.scalar.activation(bn3b[:], out2T[:, sl], AF.Identity,
                                     bias=B3, scale=A3)
                for j in range(4):
                    t = ch * 4 + j
                    pt = PS_PT()
                    nc.tensor.transpose(pt[:], bn3b[:, j * 128:(j + 1) * 128],
                                        idm_s[:])
                    nc.vector.tensor_copy(outnm[:, t, :], pt[:])
            nc.sync.dma_start(
                out_nm[:].rearrange("(t p) c -> p t c", p=128), outnm[:])
            tail_stack.close()
            late_stack.close()

    nc.compile()
    return nc


def kernel(**inputs):
    inp = {k: np.asarray(v) for k, v in inputs.items()}
    cores, dis, scale, ngrp = _preprocess(inp)

    key = (ngrp, float(scale))
    if key not in _CACHE:
        _CACHE[key] = _build(ngrp, scale)
    nc = _CACHE[key]

    x = inp["x"].astype(F32)
    U = inp["U"].astype(F32)
    s_lam = np.exp(-float(inp["gamma"].reshape(-1)[0]) *
                   inp["Lambda"].astype(np.float64) ** 2).astype(F32)

    wqkv = inp["w_qkv"].astype(F32)
    bqkv = inp["b_qkv"].astype(F32)
    wq = (wqkv[:, :C] / np.sqrt(DH)).astype(F32)
    bq = (bqkv[:C] / np.sqrt(DH)).astype(F32)
    wk, bk = wqkv[:, C:2 * C].copy(), bqkv[C:2 * C]
    wv, bv = wqkv[:, 2 * C:].copy(), bqkv[2 * C:]
    b_out_p = (bv @ inp["w_out"] + inp["b_out"]).astype(F32)

    biasp = np.zeros((128, NBIAS), F32)
    for i, vec in [(BI_SPA1, inp["b_spa1"]), (BI_SPA2, inp["b_spa2"]),
                   (BI_SPE1, inp["b_spe1"]), (BI_SPE2, inp["b_spe2"]),
                   (BI_Q, bq), (BI_K, bk), (BI_OUTP, b_out_p),
                   (BI_CHEB, inp["cheb_b"]),
                   (BI_M1A, inp["mlp_b1"][:128]), (BI_M1B, inp["mlp_b1"][128:]),
                   (BI_M2, inp["mlp_b2"]),
                   (BI_BN1W, inp["bn1_w"]), (BI_BN1B, inp["bn1_b"]),
                   (BI_BN2W, inp["bn2_w"]), (BI_BN2B, inp["bn2_b"]),
                   (BI_BN3W, inp["bn3_w"]), (BI_BN3B, inp["bn3_b"]),
                   (BI_EPSC, np.full(128, EPS, F32))]:
        biasp[:, i] = vec.astype(F32)

    chebw_cols = np.concatenate([inp["cheb_w"][k].astype(F32) for k in range(K)],
                                axis=1)  # [128, 5*128]

    common = dict(
        wspa1=inp["w_spa1"].astype(BF16), wspa2=inp["w_spa2"].astype(BF16),
        wspe1=inp["w_spe1"].astype(BF16), wspe2=inp["w_spe2"].astype(BF16),
        wproj=inp["w_proj"].astype(BF16), chebw=chebw_cols.astype(BF16),
        wq=wq.astype(BF16), wk=wk.astype(BF16), wv=wv.astype(BF16),
        wout=inp["w_out"].astype(BF16),
        mw1=inp["mlp_w1"].astype(BF16),
        mw2a=inp["mlp_w2"][:128].astype(BF16), mw2b=inp["mlp_w2"][128:].astype(BF16),
        biasp=biasp,
        bqh=np.ascontiguousarray(bq.reshape(H, DH).T),
        bkh=np.ascontiguousarray(bk.astype(F32).reshape(H, DH).T),
        idm=np.eye(128, dtype=F32), idmb=np.eye(128, dtype=BF16),
        gmat=None, u1col=None, onesf=np.ones((128, 1), F32),
        one1=np.ones((1, 1), F32),
    )

    vs = VSCALE if FP8V else 1.0
    Us_full = (U * s_lam[None, :]).astype(F32)
    dis_cl = np.where(dis > 0, dis, 1.0).astype(F32)
    xfT_np = np.ascontiguousarray(x.T).astype(BF16)
    discf_np = np.ascontiguousarray((vs * dis_cl).reshape(N // 128, 128).T)
    gmat_np = (Us_full.T @ Us_full).astype(F32)
    u1_np = np.ascontiguousarray(Us_full.sum(0).astype(F32)[:, None])
    in_maps = []
    for c in range(NCORES):
        sl = slice(c * NLOC, (c + 1) * NLOC)
        dis_c = dis[sl]
        m = dict(common)
        m["xT"] = np.ascontiguousarray(x[sl].T).astype(BF16)
        m["u_nm"] = np.ascontiguousarray(U[sl]).astype(BF16)
        m["usT"] = np.ascontiguousarray((U[sl] * s_lam[None, :]).T).astype(BF16)
        m["usd"] = np.ascontiguousarray(
            Us_full[sl] / dis_cl[sl][:, None]).astype(BF16)
        m["gmat"] = gmat_np
        m["u1col"] = u1_np
        m["xfT"] = xfT_np
        m["discf"] = discf_np
        us_loc = Us_full[sl]
        m["m1x"] = np.ascontiguousarray(
            x[sl].T @ us_loc
            + np.outer(inp["cheb_b"].astype(F32), us_loc.sum(0)))
        m["disc"] = np.ascontiguousarray(
            (vs * dis_cl[sl]).reshape(NT, 128).T)
        m["disc_m"] = np.ascontiguousarray(
            (-scale / vs * dis_c).reshape(NT, 128).T)
        m["gidx"] = cores[c]["gidx"]
        m["s8"] = cores[c]["s8"]
        in_maps.append(m)

    import os
    global LAST_NC, LAST_IN_MAPS
    LAST_NC = nc
    LAST_IN_MAPS = in_maps
    trace = os.environ.get("KERNEL_TRACE", "0") == "1"
    res = run_bass_kernel_spmd(nc, in_maps, core_ids=list(range(NCORES)),
                               trace=trace)
    global LAST_EXEC_NS, LAST_RESULT
    LAST_EXEC_NS = res.exec_time_ns
    LAST_RESULT = res
    out = np.concatenate([res.results[c]["out_nm"] for c in range(NCORES)], axis=0)
    return out.astype(inp["x"].dtype)



# revision 41
# speedup vs baseline: 1.4974x; 1.1198x over previous
"""Trainium2 Bass kernel for nn_CachedSpectralGPSLayer (8-core SPMD).

Self-contained: takes FULL inputs, shards per-core internally, runs one
Bass/Tile program SPMD on 8 NeuronCores, gathers the full output.
"""
import os
import sys

sys.path.insert(0, "/opt/trn_rl_repo")

import numpy as np
import ml_dtypes

import concourse.bacc as bacc
import concourse.bass as bass
import concourse.mybir as mybir
import concourse.tile as tile
from concourse import library_config
from concourse.bass_utils import run_bass_kernel_spmd

BF16 = ml_dtypes.bfloat16
FP8 = ml_dtypes.float8_e4m3
F32 = np.float32

N, C, K, KEIG, B, NG, H = 32768, 128, 5, 128, 64, 512, 4
NCORES = 8
NLOC = N // NCORES          # 4096
NT = NLOC // 128            # 32 node tiles per core
Bd = 64                     # dst nodes per block
NBLK = NLOC // Bd           # 64 blocks per core
GPC = B // NCORES           # 8 graphs per core
DH = C // H                 # 32
EPS = 1e-5
BPG = 4                     # blocks per dma_gather call
NCALL = NBLK // BPG         # 16 gather calls per hop
FP8V = False                # gather/AllGather payload (v~) in fp8e4m3
VSCALE = 16.0               # v~ pre-scale (keeps fp8 out of subnormals)
# virtual-schedule stamps (ms units): hop start = ST0 + STH*(h-1),
# window = hop + STW, tail = ST0 + STH*hops. Scheduler ordering hints only.
ST0 = float(os.environ.get("K_ST0", "0.45"))
STH = float(os.environ.get("K_STH", "0.40"))
STW = float(os.environ.get("K_STW", "0.20"))

fp32 = mybir.dt.float32
f32r = mybir.dt.float32r
bf16 = mybir.dt.bfloat16
fp8 = mybir.dt.float8e4
i16 = mybir.dt.int16


def R(ap):
    """Bitcast an fp32 AP to float32r: bit-identical fp32 data, but the PE
    runs replicated mode (1 cyc/row when moving dim >=256 vs 4 for fp32)."""
    return ap.bitcast(f32r)

# bias-pack column indices
(BI_SPA1, BI_SPA2, BI_SPE1, BI_SPE2, BI_Q, BI_K, BI_OUTP, BI_CHEB,
 BI_M1A, BI_M1B, BI_M2, BI_BN1W, BI_BN1B, BI_BN2W, BI_BN2B, BI_BN3W,
 BI_BN3B, BI_EPSC) = range(18)
NBIAS = 18

_CACHE = {}


def _wrap_idx(idx_flat):
    """dma_gather wrapped layout per call: idx i -> [i%16, i//16], replicated
    to all 8 groups of 16 partitions. idx_flat: [ncalls, n_per_call]."""
    ncalls, npc = idx_flat.shape
    base = idx_flat.reshape(ncalls, npc // 16, 16).transpose(0, 2, 1)  # [ncalls,16,npc/16]
    out = np.tile(base, (1, 8, 1))                                     # [ncalls,128,npc/16]
    return np.concatenate(list(out), axis=1)                           # [128, ncalls*npc/16]


def _preprocess(inputs):
    src = np.asarray(inputs["edge_index"][0]).astype(np.int64)
    dst = np.asarray(inputs["edge_index"][1]).astype(np.int64)
    deg = np.bincount(src, minlength=N).astype(np.float64)
    dis = np.where(deg > 0, 1.0 / np.sqrt(deg), 0.0).astype(F32)
    lam = float(np.asarray(inputs["lambda_max"]).reshape(-1)[0])
    scale = 2.0 / lam

    order = np.argsort(dst, kind="stable")
    srcs, dsts = src[order], dst[order]
    counts = np.bincount(dst // Bd, minlength=N // Bd)
    ngrp = int(np.ceil(counts.max() / 128))
    epb = ngrp * 128                    # padded edges per block
    epad = NBLK * epb                   # per core

    cores = []
    bounds = np.searchsorted(dsts, np.arange(0, N + 1, NLOC))
    for c in range(NCORES):
        lo = c * NLOC
        sl = slice(bounds[c], bounds[c + 1])
        sc, dc = srcs[sl], dsts[sl] - lo
        blk = dc // Bd
        ord2 = np.lexsort((sc, blk))  # sort by src within each dst block
        sc, dc, blk = sc[ord2], dc[ord2], blk[ord2]
        cnt = np.bincount(blk, minlength=NBLK)
        csum = np.concatenate([[0], np.cumsum(cnt)])
        pos_in_blk = np.arange(len(sc)) - csum[blk]
        slot = blk * epb + pos_in_blk
        src_pad = np.zeros(epad, np.int64)
        src_pad[slot] = sc
        # S one-hot fp8 bytes [128, ngroups*Bd]
        ngroups = NBLK * ngrp
        s8 = np.zeros((128, ngroups * Bd), np.uint8)
        g = slot // 128
        p = slot % 128
        s8[p, g * Bd + (dc - blk * Bd)] = 0x38  # fp8e4m3 1.0
        assert src_pad.max() < 2 ** 15
        gidx = _wrap_idx(src_pad.astype(np.int16).reshape(NCALL, BPG * epb))
        cores.append(dict(s8=s8.view(FP8), gidx=gidx))
    return cores, dis, scale, ngrp


def _build(ngrp, scale, hops=K - 1, do_mha=True, do_spec=True):
    do_spec = True
    """Build + compile the SPMD Bass program. Returns (nc, input names)."""
    epb = ngrp * 128
    epad = NBLK * epb
    ngroups = NBLK * ngrp

    nc = bacc.Bacc("TRN2", target_bir_lowering=False, debug=False,
                   enable_asserts=True, num_devices=NCORES,
                   num_swdge_queues=4)

    def din(name, shape, dt):
        return nc.dram_tensor(name, shape, dt, kind="ExternalInput").ap()

    xT = din("xT", [128, NLOC], bf16)
    u_nm = din("u_nm", [NLOC, 128], bf16)
    usT = din("usT", [128, NLOC], bf16)
    disc = din("disc", [128, NT], fp32)
    disc_m = din("disc_m", [128, NT], fp32)
    gidx = din("gidx", [128, epad // 16], i16)
    s8 = din("s8", [128, ngroups * Bd], fp8)
    wspa1 = din("wspa1", [128, 128], bf16)
    wspa2 = din("wspa2", [128, 128], bf16)
    wspe1 = din("wspe1", [128, 128], bf16)
    wspe2 = din("wspe2", [128, 128], bf16)
    wproj = din("wproj", [128, 128], bf16)
    chebw = din("chebw", [128, K * 128], bf16)
    wq = din("wq", [128, 128], bf16)
    wk = din("wk", [128, 128], bf16)
    wv = din("wv", [128, 128], bf16)
    wout = din("wout", [128, 128], bf16)
    mw1 = din("mw1", [128, 256], bf16)
    mw2a = din("mw2a", [128, 128], bf16)
    mw2b = din("mw2b", [128, 128], bf16)
    biasp = din("biasp", [128, NBIAS], fp32)
    bqh = din("bqh", [32, H], fp32)
    bkh = din("bkh", [32, H], fp32)
    idm = din("idm", [128, 128], fp32)
    gmat = din("gmat", [128, 128], fp32)
    u1col = din("u1col", [128, 1], fp32)
    usd = din("usd", [NLOC, 128], bf16)
    idmb = din("idmb", [128, 128], bf16)
    onesf = din("onesf", [128, 1], fp32)
    one1 = din("one1", [1, 1], fp32)
    m1x = din("m1x", [128, 128], fp32)
    xfT = din("xfT", [128, N], bf16)
    discf = din("discf", [128, N // 128], fp32)

    out_nm = nc.dram_tensor("out_nm", [NLOC, 128], fp32, kind="ExternalOutput").ap()

    AF = mybir.ActivationFunctionType
    OP = mybir.AluOpType

    with tile.TileContext(nc) as tc:
        with tc.tile_pool(name="const", bufs=1) as cp, \
             tc.tile_pool(name="big", bufs=1) as bp, \
             tc.tile_pool(name="work", bufs=2) as wp, \
             tc.tile_pool(name="psmm", bufs=2, space="PSUM") as psmm, \
             tc.tile_pool(name="psat", bufs=2, space="PSUM") as psatp, \
             tc.tile_pool(name="pspt", bufs=2, space="PSUM") as psptp, \
             tc.tile_pool(name="psagg", bufs=2, space="PSUM") as psaggp, \
             tc.tile_pool(name="dram", bufs=1, space="DRAM") as dp:

            # uniform-tag psum allocators (PSUM = 8 banks total: 2+2+2+2)
            def PS_MM():   # transient [128,512] matmul outputs
                return psmm.tile([128, 512], fp32, tag="mm", name="psmm_t")

            def PS_AT(shape):  # long-lived accumulators / phase-2 partial
                return psatp.tile(shape, fp32, tag="at", name="psat_t")

            def PS_ATW():  # MHA attn+denom accumulator [128, 132]
                return psatp.tile([128, 33 * H], fp32, tag="at", name="psatw_t")

            def PS_PT2():  # [32, 512] head q/k psum
                return psptp.tile([32, 512], fp32, tag="pt", name="pspt2_t")

            def PS_PT():   # [128,128] transposes / small matmuls
                return psptp.tile([128, 128], fp32, tag="pt", name="pspt_t")

            def PS_AGG():  # [128,128] cheb aggregation
                return psaggp.tile([128, 128], fp32, tag="agg", name="psagg_t")

            nc.gpsimd.load_library(library_config.mlp)

            # ---- load constants ----
            def ld(ap_in, shape, dt, name, eng=None):
                t = cp.tile(shape, dt, tag=name, name=name)
                (eng or nc.sync).dma_start(t[:], ap_in[:])
                return t

            xT_s = bp.tile([128, NLOC], bf16, tag="xT")
            nc.sync.dma_start(xT_s[:], xT[:])
            s8_s = ld(s8, [128, ngroups * Bd], fp8, "s8")
            wspa1_s = ld(wspa1, [128, 128], bf16, "wspa1")
            wspa2_s = ld(wspa2, [128, 128], bf16, "wspa2")
            wspe1_s = ld(wspe1, [128, 128], bf16, "wspe1")
            wspe2_s = ld(wspe2, [128, 128], bf16, "wspe2")
            wproj_s = ld(wproj, [128, 128], bf16, "wproj")
            chebw_s = ld(chebw, [128, K * 128], bf16, "chebw")
            wq_s = ld(wq, [128, 128], bf16, "wq")
            wk_s = ld(wk, [128, 128], bf16, "wk")
            wv_s = ld(wv, [128, 128], bf16, "wv")
            wout_s = ld(wout, [128, 128], bf16, "wout")
            mw1_s = ld(mw1, [128, 256], bf16, "mw1")
            mw2a_s = ld(mw2a, [128, 128], bf16, "mw2a")
            mw2b_s = ld(mw2b, [128, 128], bf16, "mw2b")
            biasp_s = ld(biasp, [128, NBIAS], fp32, "biasp")
            bqh_s = ld(bqh, [32, H], fp32, "bqh")
            bkh_s = ld(bkh, [32, H], fp32, "bkh")
            idm_s = ld(idm, [128, 128], fp32, "idm")
            idmb_s = ld(idmb, [128, 128], bf16, "idmb")
            vbuf = bp.tile([128, NT, 128], bf16, tag="vbuf")

            def th_accum(t_ps):
                # T_h = v~^T (Us/dis): stream usd in 8-tile chunks (window
                # work, off the gather critical path; scalar-queue DMA)
                for t0 in range(0, NT, 8):
                    ub8 = wp.tile([128, 8, 128], bf16, tag="ut8", bufs=1)
                    nc.scalar.dma_start(
                        ub8[:], usd[t0 * 128:(t0 + 8) * 128, :]
                        .rearrange("(t p) c -> p t c", p=128))
                    for j in range(8):
                        t = t0 + j
                        nc.tensor.matmul(t_ps[:], lhsT=vbuf[:, t, :],
                                         rhs=ub8[:, j, :],
                                         start=(t == 0), stop=(t == NT - 1))
            gmat_s = ld(gmat, [128, 128], fp32, "gmat")
            u1col_s = ld(u1col, [128, 1], fp32, "u1col")
            onesf_s = ld(onesf, [128, 1], fp32, "onesf")
            one1_s = ld(one1, [1, 1], fp32, "one1")
            m1x_s = ld(m1x, [128, 128], fp32, "m1x")
            m1sb = cp.tile([128, 128], fp32, tag="m1sb", name="m1sb")
            discf_s = ld(discf, [128, N // 128], fp32, "discf")
            disc_s = ld(disc, [128, NT], fp32, "disc")
            discm_s = ld(disc_m, [128, 2 * NT], fp32, "discm")
            discm2_s = discm_s[:, NT:2 * NT]

            def bcol(i):
                return biasp_s[:, i:i + 1]

            # persistent big buffers
            TxA = bp.tile([128, NT, 128], fp32, tag="TxA")   # node-major
            TxB = bp.tile([128, NT, 128], fp32, tag="TxB")
            chebT = bp.tile([128, NLOC], fp32, tag="chebT")  # later: pre1, outT
            h2T = bp.tile([128, NLOC], fp32, tag="h2T")      # later: pre2, out2T
            if not do_mha:
                nc.vector.memset(h2T[:], 0.0)
            statc = cp.tile([128, 32], fp32, tag="statc")    # stats/affine cols
            statc2 = cp.tile([128, 16], fp32, tag="statc2")  # per-graph h2 stats
            outnm = TxA  # reuse (dead after cheb)

            # DRAM bounce buffers
            vdt = fp8 if FP8V else bf16
            ag_in = dp.tile([NLOC, 128], vdt, tag="ag_in")
            ag_outs = [None] + [dp.tile([N, 128], vdt, tag=f"ag_out{h}", name=f"ag_out{h}", addr_space="Shared") for h in range(1, 4)]
            vfull0 = dp.tile([N, 128], vdt, tag="vfull0", name="vfull0")
            bn12_in = dp.tile([128, 260], fp32, tag="bn12_in")
            bn12_out = dp.tile([128, 260], fp32, tag="bn12_out", addr_space="Shared")
            bn3_in = dp.tile([128, 2], fp32, tag="bn3_in")
            bn3_out = dp.tile([128, 2], fp32, tag="bn3_out", addr_space="Shared")

            CH = NLOC // 512  # 8 chunks of 512

            from contextlib import ExitStack
            ep_stack = ExitStack()
            ep = ep_stack.enter_context(tc.tile_pool(name="early", bufs=1))
            xspT = ep.tile([128, NLOC], bf16, tag="xspT", name="xspT")

            # ================= Phase 1: local spa MLP (feature-major) =======
            # (the spe MLP + spectral partial run inside AllGather window 1,
            # recomputed from the resident xT_s, off the pre-hop critical path)
            for ch in range(CH):
                sl = slice(ch * 512, (ch + 1) * 512)
                p1 = PS_MM()
                nc.tensor.matmul(p1[:], lhsT=wspa1_s[:], rhs=xT_s[:, sl],
                                 start=True, stop=True)
                t1 = wp.tile([128, 512], bf16, tag="t1b")
                nc.scalar.activation(t1[:], p1[:], AF.Relu, bias=bcol(BI_SPA1))
                p2 = PS_MM()
                nc.tensor.matmul(p2[:], lhsT=wspa2_s[:], rhs=t1[:],
                                 start=True, stop=True)
                nc.scalar.activation(xspT[:, sl], p2[:], AF.Identity,
                                     bias=bcol(BI_SPA2))

            # Tx0 node-major (local shard, for recurrence) + v~0 into vbuf
            for t in range(NT):
                tsl = slice(t * 128, (t + 1) * 128)
                pt = PS_PT()
                nc.tensor.matmul(pt[:], lhsT=xspT[:, tsl], rhs=idmb_s[:],
                                 start=True, stop=True)
                nc.vector.tensor_copy(TxB[:, t, :], pt[:])
                nc.scalar.activation(vbuf[:, t, :], pt[:], AF.Identity,
                                     scale=disc_s[:, t:t + 1])
            # v~0 for ALL nodes computed locally (replaces hop-1 AllGather):
            # every core redundantly runs the spatial MLP over the full x.
            if hops >= 1:
                for gch in range(N // 512):
                    gsl2 = slice(gch * 512, (gch + 1) * 512)
                    xc = wp.tile([128, 512], bf16, tag="t1b")
                    nc.sync.dma_start(xc[:], xfT[:, gsl2])
                    pf1 = PS_MM()
                    nc.tensor.matmul(pf1[:], lhsT=wspa1_s[:], rhs=xc[:],
                                     start=True, stop=True)
                    tf1 = wp.tile([128, 512], bf16, tag="midab")
                    nc.scalar.activation(tf1[:], pf1[:], AF.Relu,
                                         bias=bcol(BI_SPA1))
                    pf2 = PS_MM()
                    nc.tensor.matmul(pf2[:], lhsT=wspa2_s[:], rhs=tf1[:],
                                     start=True, stop=True)
                    spf = wp.tile([128, 512], bf16, tag="t1b")
                    nc.scalar.activation(spf[:], pf2[:], AF.Identity,
                                         bias=bcol(BI_SPA2))
                    vt4 = wp.tile([128, 4, 128], vdt, tag="vt4", bufs=2)
                    for j in range(4):
                        tg = gch * 4 + j
                        ptf = PS_PT()
                        nc.tensor.matmul(ptf[:], lhsT=spf[:, j * 128:(j + 1) * 128],
                                         rhs=idmb_s[:], start=True, stop=True)
                        nc.scalar.activation(vt4[:, j, :], ptf[:], AF.Identity,
                                             scale=discf_s[:, tg:tg + 1])
                    nc.sync.dma_start(
                        vfull0[gch * 512:(gch + 1) * 512, :]
                        .rearrange("(t p) c -> p t c", p=128), vt4[:])

            for ch in range(CH):
                sl = slice(ch * 512, (ch + 1) * 512)
                pw = PS_MM()
                nc.tensor.matmul(pw[:], lhsT=chebw_s[:, 0:128], rhs=xspT[:, sl],
                                 start=True, stop=True)
                nc.vector.tensor_copy(chebT[:, sl], pw[:])
            # T_0 = v~0^T (Us/dis) (accumulate over tiles); m1sb = m1x + W0^T T_0
            t_ps = PS_AGG()
            th_accum(t_ps)
            tsb = wp.tile([128, 128], bf16, tag="tsb")
            nc.vector.tensor_copy(tsb[:], t_ps[:])
            pWt = PS_PT()
            nc.tensor.matmul(pWt[:], lhsT=chebw_s[:, 0:128], rhs=tsb[:],
                             start=True, stop=True)
            nc.vector.tensor_add(m1sb[:], m1x_s[:], pWt[:])

            # ===== Phase 2 (deferred): spectral partial, run in AG window 1 =
            # Recomputes the spe MLP chunk-wise from resident xT_s so xspecT
            # needs no SBUF residency across the hops.
            part_s = wp.tile([128, 128], fp32, tag="part_s", bufs=1)

            def spectral_partial():
                part_ps = PS_AT([128, 128])
                for ch2 in range(CH):
                    sl2 = slice(ch2 * 512, (ch2 + 1) * 512)
                    p3 = PS_MM()
                    nc.tensor.matmul(p3[:], lhsT=wspe1_s[:], rhs=xT_s[:, sl2],
                                     start=True, stop=True)
                    t2 = wp.tile([128, 512], bf16, tag="t1b")
                    nc.scalar.activation(t2[:], p3[:], AF.Relu,
                                         bias=bcol(BI_SPE1))
                    p4 = PS_MM()
                    nc.tensor.matmul(p4[:], lhsT=wspe2_s[:], rhs=t2[:],
                                     start=True, stop=True)
                    xsp2 = wp.tile([128, 512], bf16, tag="t1b")
                    nc.scalar.activation(xsp2[:], p4[:], AF.Identity,
                                         bias=bcol(BI_SPE2))
                    ub = wp.tile([128, 4, 128], bf16, tag="ut4", bufs=1)
                    nc.sync.dma_start(
                        ub[:], u_nm[ch2 * 512:(ch2 + 1) * 512, :]
                        .rearrange("(t p) c -> p t c", p=128))
                    for j in range(4):
                        t = ch2 * 4 + j
                        ph = PS_PT()
                        nc.tensor.matmul(ph[:], lhsT=xsp2[:, j * 128:(j + 1) * 128],
                                         rhs=wproj_s[:], start=True, stop=True)
                        hp = wp.tile([128, 128], bf16, tag="hp")
                        nc.vector.tensor_copy(hp[:], ph[:])
                        nc.tensor.matmul(part_ps[:], lhsT=ub[:, j, :], rhs=hp[:],
                                         start=(t == 0), stop=(t == NT - 1))
                nc.vector.tensor_copy(part_s[:], part_ps[:])

            ep_stack.close()  # free xspT space for later pools
            late_stack = ExitStack()
            gp = late_stack.enter_context(tc.tile_pool(name="gath", bufs=2))
            mp = late_stack.enter_context(tc.tile_pool(name="mha", bufs=2))

            # ---- MHA for one graph (interleaved into AllGather windows) ----
            def mha_graph(g):
                gsl = slice(g * 512, (g + 1) * 512)
                # head-major q/k: per-head matmuls so all operands are base-0
                qT = mp.tile([32, H * 512], bf16, tag="qT", bufs=1)
                kT = mp.tile([32, H * 512], bf16, tag="kT", bufs=1)
                for hh in range(H):
                    csl = slice(hh * 32, (hh + 1) * 32)
                    pqh = PS_PT2()
                    nc.tensor.matmul(pqh[:], lhsT=wq_s[:, csl],
                                     rhs=xT_s[:, gsl], start=True, stop=True)
                    nc.scalar.activation(qT[:, hh * 512:(hh + 1) * 512], pqh[:],
                                         AF.Identity, bias=bqh_s[:, hh:hh + 1])
                    pkh = PS_PT2()
                    nc.tensor.matmul(pkh[:], lhsT=wk_s[:, csl],
                                     rhs=xT_s[:, gsl], start=True, stop=True)
                    nc.scalar.activation(kT[:, hh * 512:(hh + 1) * 512], pkh[:],
                                         AF.Identity, bias=bkh_s[:, hh:hh + 1])
                # v node-major, augmented per head with a ones column
                vaug = mp.tile([128, 4, 33 * H], bf16, tag="vaug")
                nc.vector.memset(vaug[:, :, 32::33], 1.0)
                for j in range(4):
                    pv = PS_PT()
                    nc.tensor.matmul(pv[:], lhsT=xT_s[:, g * 512 + j * 128:
                                                      g * 512 + (j + 1) * 128],
                                     rhs=wv_s[:], start=True, stop=True)
                    for hh in range(H):
                        nc.vector.tensor_copy(
                            vaug[:, j, hh * 33:hh * 33 + 32],
                            pv[:, hh * 32:(hh + 1) * 32])
                # scores_T + exp, per (head, k-chunk)
                ess = {}
                for hh in range(H):
                    qsl = slice(hh * 512, (hh + 1) * 512)
                    for j in range(4):
                        pss = PS_MM()
                        nc.tensor.matmul(
                            pss[:], lhsT=kT[:, hh * 512 + j * 128:
                                            hh * 512 + (j + 1) * 128],
                            rhs=qT[:, qsl], start=True, stop=True)
                        es = mp.tile([128, 512], bf16, tag="es", bufs=16)
                        nc.scalar.activation(es[:], pss[:], AF.Exp)
                        ess[(hh, j)] = es
                # attn + denom per q-chunk
                for qc in range(4):
                    pat = PS_ATW()
                    for hh in range(H):
                        for j in range(4):
                            nc.tensor.matmul(
                                pat[:, hh * 33:(hh + 1) * 33],
                                lhsT=ess[(hh, j)][:, qc * 128:(qc + 1) * 128],
                                rhs=vaug[:, j, hh * 33:(hh + 1) * 33],
                                start=(j == 0), stop=(j == 3),
                                skip_group_check=True)
                    recip = wp.tile([128, 4], fp32, tag="recip")
                    nc.vector.reciprocal(recip[:], pat[:, 32::33])
                    anm = wp.tile([128, 128], fp32, tag="anm")
                    for hh in range(H):
                        nc.vector.tensor_scalar(
                            out=anm[:, hh * 32:(hh + 1) * 32],
                            in0=pat[:, hh * 33:hh * 33 + 32],
                            scalar1=recip[:, hh:hh + 1], scalar2=None,
                            op0=OP.mult)
                    ptr = PS_PT()
                    nc.tensor.transpose(ptr[:], anm[:], idm_s[:])
                    attnT = wp.tile([128, 128], bf16, tag="attnT", bufs=1)
                    nc.vector.tensor_copy(attnT[:], ptr[:])
                    ph2 = PS_PT()
                    nc.tensor.matmul(ph2[:], lhsT=wout_s[:], rhs=attnT[:],
                                     start=True, stop=True)
                    osl = slice(g * 512 + qc * 128, g * 512 + (qc + 1) * 128)
                    # pre2 = h2 + b_out' + x
                    nc.vector.scalar_tensor_tensor(
                        out=h2T[:, osl], in0=ph2[:], scalar=bcol(BI_OUTP),
                        in1=xT_s[:, osl], op0=OP.add, op1=OP.add)
                # incremental BN2 stats for this graph's 512 columns
                nc.vector.tensor_reduce(statc2[:, g:g + 1], h2T[:, gsl],
                                        mybir.AxisListType.X, OP.add)
                tsq = wp.tile([128, 512], fp32, tag="sqt", bufs=1)
                nc.vector.scalar_tensor_tensor(
                    out=tsq[:], in0=h2T[:, gsl], scalar=1.0, in1=h2T[:, gsl],
                    op0=OP.mult, op1=OP.mult,
                    accum_out=statc2[:, 8 + g:9 + g])

            # graphs run inside AllGather wait windows (PE idle otherwise)
            mha_sched = {1: [0, 1, 2], 2: [3, 4, 5], 3: [6, 7]} \
                if (do_mha and hops == K - 1) else {}
            mha_left = [g for g in range(GPC if do_mha else 0)
                        if not any(g in v for v in mha_sched.values())]

            # ================= Phase 3: cheb hops ===========================
            # tile_wait_until stamps are scheduler-only hints (virtual
            # earliest-start): they stop the list scheduler from hoisting
            # hop h+1's recurrence ops ahead of the window-h MHA work in
            # the in-order DVE queue (head-of-line blocking during the
            # AllGather). They emit no HW waits.
            cur, prev = TxB, TxA  # cur holds Tx_{h-1}; prev gets Tx_h
            for h in range(1, 1 + hops):
                t_hop = ST0 + STH * (h - 1)
                ag_src = vfull0 if h == 1 else ag_outs[h - 1]
                hop_stack = ExitStack()
                hop_stack.enter_context(tc.tile_wait_until(t_hop))
                for q in range(NCALL):
                    gt = gp.tile([128, BPG * ngrp, 128], vdt, tag="gt", bufs=3)
                    isl = slice(q * BPG * epb // 16, (q + 1) * BPG * epb // 16)
                    gix = wp.tile([128, BPG * epb // 16], i16, tag="gix", bufs=4)
                    nc.sync.dma_start(gix[:], gidx[:, isl])
                    nc.gpsimd.dma_gather(gt[:], ag_src[:], gix[:],
                                         BPG * epb, BPG * epb, 128,
                                         single_packet=False,
                                         queue_num=q % 4)
                    for r in range(BPG):
                        b = q * BPG + r
                        t, half = b // 2, b % 2
                        if half == 0:
                            aps = PS_AGG()
                        for j in range(ngrp):
                            gcol = b * ngrp + j
                            nc.tensor.matmul(
                                aps[half * 64:(half + 1) * 64, :],
                                lhsT=s8_s[:, gcol * Bd:(gcol + 1) * Bd],
                                rhs=gt[:, r * ngrp + j, :],
                                start=(j == 0), stop=(j == ngrp - 1))
                        if half == 1:
                            # recurrence for tile t
                            tmp = wp.tile([128, 128], fp32, tag="rectmp")
                            if h == 1:
                                nc.vector.tensor_scalar(
                                    out=tmp[:], in0=aps[:],
                                    scalar1=discm_s[:, t:t + 1], scalar2=None,
                                    op0=OP.mult)
                                # Tx1 = (scale-1)*Tx0 + tmp
                                nc.vector.scalar_tensor_tensor(
                                    out=prev[:, t, :], in0=cur[:, t, :],
                                    scalar=float(scale - 1.0), in1=tmp[:],
                                    op0=OP.mult, op1=OP.add)
                            else:
                                nc.vector.tensor_scalar(
                                    out=tmp[:], in0=aps[:],
                                    scalar1=discm_s[:, t:t + 1], scalar2=2.0,
                                    op0=OP.mult, op1=OP.mult)
                                # tmp2 = tmp - Tx_{h-2}
                                tmp2 = wp.tile([128, 128], fp32, tag="rectmp2")
                                nc.vector.scalar_tensor_tensor(
                                    out=tmp2[:], in0=prev[:, t, :],
                                    scalar=-1.0, in1=tmp[:],
                                    op0=OP.mult, op1=OP.add)
                                # Tx_h = 2(scale-1)*Tx_{h-1} + tmp2
                                nc.vector.scalar_tensor_tensor(
                                    out=prev[:, t, :], in0=cur[:, t, :],
                                    scalar=float(2.0 * (scale - 1.0)),
                                    in1=tmp2[:], op0=OP.mult, op1=OP.add)
                            nc.scalar.activation(
                                vbuf[:, t, :], prev[:, t, :], AF.Identity,
                                scale=disc_s[:, t:t + 1])
                            if h < hops:
                                nc.sync.dma_start(
                                    ag_in[t * 128:(t + 1) * 128, :],
                                    vbuf[:, t, :])
                # launch AG for next hop once all v~ tiles written
                if h < hops:
                    nc.gpsimd.collective_compute(
                        "AllGather", OP.bypass,
                        replica_groups=[list(range(NCORES))],
                        ins=[ag_in.opt()], outs=[ag_outs[h].opt()])
                hop_stack.close()
                win_stack = ExitStack()
                win_stack.enter_context(tc.tile_wait_until(t_hop + STW))
                # out_cheb += Tx_h @ W_h  (transpose tiles chunk-wise)
                for ch in range(CH):
                    tpb = wp.tile([128, 512], bf16, tag="tpb", bufs=1)
                    for j in range(4):
                        t = ch * 4 + j
                        pt = PS_PT()
                        nc.tensor.transpose(pt[:], prev[:, t, :], idm_s[:])
                        nc.vector.tensor_copy(tpb[:, j * 128:(j + 1) * 128], pt[:])
                    sl = slice(ch * 512, (ch + 1) * 512)
                    pw = PS_MM()
                    nc.tensor.matmul(pw[:], lhsT=chebw_s[:, h * 128:(h + 1) * 128],
                                     rhs=tpb[:], start=True, stop=True)
                    nc.vector.tensor_add(chebT[:, sl], chebT[:, sl], pw[:])
                # T_h = v~_h^T (Us/dis) ; m1sb += W_h^T T_h  (off the tail)
                t_ps = PS_AGG()
                th_accum(t_ps)
                tsb = wp.tile([128, 128], bf16, tag="tsb")
                nc.vector.tensor_copy(tsb[:], t_ps[:])
                pWt = PS_PT()
                nc.tensor.matmul(pWt[:], lhsT=chebw_s[:, h * 128:(h + 1) * 128],
                                 rhs=tsb[:], start=True, stop=True)
                nc.vector.tensor_add(m1sb[:], m1sb[:], pWt[:])
                if h == 1:
                    spectral_partial()
                for g in mha_sched.get(h, []):
                    mha_graph(g)
                win_stack.close()
                cur, prev = prev, cur

            # ================= Phase 4: MHA (remaining graphs) ==============
            for g in mha_left:
                mha_graph(g)

            tail_stack = ExitStack()
            tail_stack.enter_context(tc.tile_wait_until(ST0 + STH * hops))
            # ===== Phase 5: pre1' (no spec) + BN stats + M1 + joint AR ======
            for ch in range(CH):
                sl = slice(ch * 512, (ch + 1) * 512)
                # pre1' = chebT + cheb_b + x   (overwrite chebT)
                nc.vector.scalar_tensor_tensor(
                    out=chebT[:, sl], in0=chebT[:, sl], scalar=bcol(BI_CHEB),
                    in1=xT_s[:, sl], op0=OP.add, op1=OP.add)
            # BN1/BN2 stats
            def sumsq(buf, out_col):
                for c2 in range(CH):
                    s2 = slice(c2 * 512, (c2 + 1) * 512)
                    tt = wp.tile([128, 512], fp32, tag="sqt", bufs=1)
                    nc.vector.scalar_tensor_tensor(
                        out=tt[:], in0=buf[:, s2], scalar=1.0, in1=buf[:, s2],
                        op0=OP.mult, op1=OP.mult,
                        accum_out=statc[:, 24 + c2:25 + c2])
                nc.vector.tensor_reduce(out_col, statc[:, 24:32],
                                        mybir.AxisListType.X, OP.add)

            nc.vector.tensor_reduce(statc[:, 0:1], chebT[:], mybir.AxisListType.X, OP.add)
            sumsq(chebT, statc[:, 1:2])
            if do_mha:
                nc.vector.tensor_reduce(statc[:, 2:3], statc2[:, 0:8],
                                        mybir.AxisListType.X, OP.add)
                nc.vector.tensor_reduce(statc[:, 3:4], statc2[:, 8:16],
                                        mybir.AxisListType.X, OP.add)
            else:
                nc.vector.tensor_reduce(statc[:, 2:3], h2T[:], mybir.AxisListType.X, OP.add)
                sumsq(h2T, statc[:, 3:4])
            st12 = wp.tile([128, 260], fp32, tag="st12", bufs=1)
            nc.vector.tensor_copy(st12[:, 0:4], statc[:, 0:4])
            nc.vector.tensor_copy(st12[:, 4:132], m1sb[:])
            nc.vector.tensor_copy(st12[:, 132:260], part_s[:])
            nc.sync.dma_start(bn12_in[:], st12[:])
            nc.gpsimd.collective_compute(
                "AllReduce", OP.add, replica_groups=[list(range(NCORES))],
                ins=[bn12_in.opt()], outs=[bn12_out.opt()])
            sr12 = wp.tile([128, 260], fp32, tag="sr12", bufs=1)
            nc.sync.dma_start(sr12[:], bn12_out[:])
            Pm = sr12[:, 132:260]   # AR'd spectral partial [keig, C]
            M1g = sr12[:, 4:132]    # [C, keig]
            # spectral stat terms:
            # s_sum[c] = sum_k u1[k] P[k,c];  s_sq[c] = sum_k P[k,c](G P)[k,c]
            # cross[c] = sum_k M1g[c,k] P[k,c]
            w12 = wp.tile([128, 256], fp32, tag="w12", bufs=1)
            nc.vector.tensor_scalar(out=w12[:, 0:128], in0=Pm,
                                    scalar1=u1col_s[:], scalar2=None,
                                    op0=OP.mult)
            t1_ps = PS_PT()
            nc.tensor.matmul(t1_ps[:], lhsT=gmat_s[:], rhs=Pm,
                             start=True, stop=True)
            nc.vector.tensor_tensor(w12[:, 128:256], t1_ps[:], Pm, OP.mult)
            # column sums: ones-matmul -> [1,256] row, then row -> two cols
            r_ps = PS_MM()
            nc.tensor.matmul(r_ps[0:1, 0:256], lhsT=onesf_s[:], rhs=w12[:],
                             start=True, stop=True)
            rowbuf = wp.tile([1, 256], fp32, tag="rowbuf")
            nc.vector.tensor_copy(rowbuf[:], r_ps[0:1, 0:256])
            c_ps = PS_PT()
            nc.tensor.matmul(c_ps[:, 0:1], lhsT=rowbuf[:, 0:128], rhs=one1_s[:],
                             start=True, stop=True, skip_group_check=True)
            nc.tensor.matmul(c_ps[:, 1:2], lhsT=rowbuf[:, 128:256], rhs=one1_s[:],
                             start=True, stop=True, skip_group_check=True)
            s_cols = wp.tile([128, 2], fp32, tag="s_cols")
            nc.vector.tensor_copy(s_cols[:], c_ps[:, 0:2])
            # cross: transpose P, multiply with M1g, reduce
            pt_ps = PS_PT()
            nc.tensor.transpose(pt_ps[:], sr12[:, 132:260], idm_s[:])
            ptm = wp.tile([128, 128], fp32, tag="w1t")
            nc.vector.tensor_tensor(ptm[:], pt_ps[:], M1g, OP.mult)
            crossc = statc[:, 14:15]
            nc.vector.tensor_reduce(crossc, ptm[:], mybir.AxisListType.X, OP.add)
            # BN1 totals: sum1 = sr12[:,0] + s_sum ; sq1 = sr12[:,1] + 2*cross + s_sq
            sum1c = statc[:, 15:16]
            nc.vector.tensor_tensor(sum1c, sr12[:, 0:1], s_cols[:, 0:1], OP.add)
            sq1c = statc[:, 16:17]
            nc.vector.scalar_tensor_tensor(out=sq1c, in0=crossc, scalar=2.0,
                                           in1=sr12[:, 1:2], op0=OP.mult,
                                           op1=OP.add)
            nc.vector.tensor_tensor(sq1c, sq1c, s_cols[:, 1:2], OP.add)

            # affine coefs: A = w/sqrt(var+eps), Bc = b - mu*A
            def bn_affine(sum_col, sq_col, w_col, b_col, a_out, b_out_col):
                mu = statc[:, 8:9]
                nc.vector.tensor_scalar(out=mu, in0=sum_col, scalar1=1.0 / N,
                                        scalar2=None, op0=OP.mult)
                msq = statc[:, 9:10]
                nc.vector.tensor_scalar(out=msq, in0=sq_col, scalar1=1.0 / N,
                                        scalar2=None, op0=OP.mult)
                nvar = statc[:, 10:11]
                nc.vector.scalar_tensor_tensor(out=nvar, in0=mu, scalar=mu,
                                               in1=msq, op0=OP.mult,
                                               op1=OP.subtract)  # mu^2 - msq
                sd = statc[:, 11:12]
                nc.scalar.activation(sd, nvar, AF.Sqrt, bias=bcol(BI_EPSC),
                                     scale=-1.0)
                rsd = statc[:, 12:13]
                nc.vector.reciprocal(rsd, sd)
                nc.vector.tensor_tensor(a_out, rsd, w_col, OP.mult)
                nbc = statc[:, 13:14]
                nc.vector.scalar_tensor_tensor(out=nbc, in0=mu, scalar=a_out,
                                               in1=b_col, op0=OP.mult,
                                               op1=OP.subtract)  # mu*A - b
                nc.vector.tensor_scalar(out=b_out_col, in0=nbc, scalar1=-1.0,
                                        scalar2=None, op0=OP.mult)

            A1, B1 = statc[:, 4:5], statc[:, 5:6]
            A2, B2 = statc[:, 6:7], statc[:, 7:8]
            bn_affine(statc[:, 15:16], statc[:, 16:17], bcol(BI_BN1W), bcol(BI_BN1B), A1, B1)
            bn_affine(sr12[:, 2:3], sr12[:, 3:4], bcol(BI_BN2W), bcol(BI_BN2B), A2, B2)
            B12 = statc[:, 5:6]
            nc.vector.tensor_tensor(B12, B1, B2, OP.add)  # B1 += B2 (in place)

            # ============ Phase 6: out = h1 + h2n; MLP2; BN3 ================
            out2T = h2T   # overwrite pre2 per chunk
            Pmb = wp.tile([128, 128], bf16, tag="pmb16", bufs=1)
            nc.vector.tensor_copy(Pmb[:], sr12[:, 132:260])
            for ch in range(CH):
                sl = slice(ch * 512, (ch + 1) * 512)
                ust = wp.tile([128, 512], bf16, tag="ust")
                nc.scalar.dma_start(ust[:], usT[:, sl])
                pso = PS_MM()
                nc.tensor.matmul(pso[:], lhsT=Pmb[:], rhs=ust[:],
                                 start=True, stop=True)
                t1 = wp.tile([128, 512], fp32, tag="t1")
                nc.scalar.activation(t1[:], chebT[:, sl], AF.Identity,
                                     bias=B12, scale=A1)
                tsp = wp.tile([128, 512], fp32, tag="t1")
                nc.vector.scalar_tensor_tensor(
                    out=tsp[:], in0=pso[:], scalar=A1, in1=t1[:],
                    op0=OP.mult, op1=OP.add)
                outT = wp.tile([128, 512], bf16, tag="outTb", bufs=2)
                nc.vector.scalar_tensor_tensor(
                    out=outT[:], in0=h2T[:, sl], scalar=A2, in1=tsp[:],
                    op0=OP.mult, op1=OP.add)
                pma = PS_MM()
                nc.tensor.matmul(pma[:], lhsT=mw1_s[:, 0:128], rhs=outT[:],
                                 start=True, stop=True)
                mida = wp.tile([128, 512], bf16, tag="midab")
                nc.scalar.activation(mida[:], pma[:], AF.Relu, bias=bcol(BI_M1A))
                pmb = PS_MM()
                nc.tensor.matmul(pmb[:], lhsT=mw1_s[:, 128:256], rhs=outT[:],
                                 start=True, stop=True)
                midb = wp.tile([128, 512], bf16, tag="midab")
                nc.scalar.activation(midb[:], pmb[:], AF.Relu, bias=bcol(BI_M1B))
                pmo = PS_MM()
                nc.tensor.matmul(pmo[:], lhsT=mw2a_s[:], rhs=mida[:],
                                 start=True, stop=False)
                nc.tensor.matmul(pmo[:], lhsT=mw2b_s[:], rhs=midb[:],
                                 start=False, stop=True)
                # out2 = out + mlp_b2 + psum
                nc.vector.scalar_tensor_tensor(
                    out=out2T[:, sl], in0=outT[:], scalar=bcol(BI_M2),
                    in1=pmo[:], op0=OP.add, op1=OP.add)
            # BN3 stats
            nc.vector.tensor_reduce(statc[:, 0:1], out2T[:], mybir.AxisListType.X, OP.add)
            sumsq(out2T, statc[:, 1:2])
            st3 = wp.tile([128, 2], fp32, tag="st")
            nc.vector.tensor_copy(st3[:], statc[:, 0:2])
            nc.sync.dma_start(bn3_in[:], st3[:])
            nc.gpsimd.collective_compute(
                "AllReduce", OP.add, replica_groups=[list(range(NCORES))],
                ins=[bn3_in.opt()], outs=[bn3_out.opt()])
            sr3 = wp.tile([128, 2], fp32, tag="st")
            nc.sync.dma_start(sr3[:], bn3_out[:])
            A3, B3 = statc[:, 4:5], statc[:, 5:6]
            bn_affine(sr3[:, 0:1], sr3[:, 1:2], bcol(BI_BN3W), bcol(BI_BN3B), A3, B3)

            # apply BN3, transpose to node-major, write out
            for ch in range(CH):
                sl = slice(ch * 512, (ch + 1) * 512)
                bn3b = wp.tile([128, 512], fp32, tag="t1")
                nc.scalar.activation(bn3b[:], out2T[:, sl], AF.Identity,
                                     bias=B3, scale=A3)
                for j in range(4):
                    t = ch * 4 + j
                    pt = PS_PT()
                    nc.tensor.transpose(pt[:], bn3b[:, j * 128:(j + 1) * 128],
                                        idm_s[:])
                    nc.vector.tensor_copy(outnm[:, t, :], pt[:])
            nc.sync.dma_start(
                out_nm[:].rearrange("(t p) c -> p t c", p=128), outnm[:])
            tail_stack.close()
            late_stack.close()

    nc.compile()
    return nc


def kernel(**inputs):
    inp = {k: np.asarray(v) for k, v in inputs.items()}
    cores, dis, scale, ngrp = _preprocess(inp)

    key = (ngrp, float(scale))
    if key not in _CACHE:
        _CACHE[key] = _build(ngrp, scale)
    nc = _CACHE[key]

    x = inp["x"].astype(F32)
    U = inp["U"].astype(F32)
    s_lam = np.exp(-float(inp["gamma"].reshape(-1)[0]) *
                   inp["Lambda"].astype(np.float64) ** 2).astype(F32)

    wqkv = inp["w_qkv"].astype(F32)
    bqkv = inp["b_qkv"].astype(F32)
    wq = (wqkv[:, :C] / np.sqrt(DH)).astype(F32)
    bq = (bqkv[:C] / np.sqrt(DH)).astype(F32)
    wk, bk = wqkv[:, C:2 * C].copy(), bqkv[C:2 * C]
    wv, bv = wqkv[:, 2 * C:].copy(), bqkv[2 * C:]
    b_out_p = (bv @ inp["w_out"] + inp["b_out"]).astype(F32)

    biasp = np.zeros((128, NBIAS), F32)
    for i, vec in [(BI_SPA1, inp["b_spa1"]), (BI_SPA2, inp["b_spa2"]),
                   (BI_SPE1, inp["b_spe1"]), (BI_SPE2, inp["b_spe2"]),
                   (BI_Q, bq), (BI_K, bk), (BI_OUTP, b_out_p),
                   (BI_CHEB, inp["cheb_b"]),
                   (BI_M1A, inp["mlp_b1"][:128]), (BI_M1B, inp["mlp_b1"][128:]),
                   (BI_M2, inp["mlp_b2"]),
                   (BI_BN1W, inp["bn1_w"]), (BI_BN1B, inp["bn1_b"]),
                   (BI_BN2W, inp["bn2_w"]), (BI_BN2B, inp["bn2_b"]),
                   (BI_BN3W, inp["bn3_w"]), (BI_BN3B, inp["bn3_b"]),
                   (BI_EPSC, np.full(128, EPS, F32))]:
        biasp[:, i] = vec.astype(F32)

    chebw_cols = np.concatenate([inp["cheb_w"][k].astype(F32) for k in range(K)],
                                axis=1)  # [128, 5*128]

    common = dict(
        wspa1=inp["w_spa1"].astype(BF16), wspa2=inp["w_spa2"].astype(BF16),
        wspe1=inp["w_spe1"].astype(BF16), wspe2=inp["w_spe2"].astype(BF16),
        wproj=inp["w_proj"].astype(BF16), chebw=chebw_cols.astype(BF16),
        wq=wq.astype(BF16), wk=wk.astype(BF16), wv=wv.astype(BF16),
        wout=inp["w_out"].astype(BF16),
        mw1=inp["mlp_w1"].astype(BF16),
        mw2a=inp["mlp_w2"][:128].astype(BF16), mw2b=inp["mlp_w2"][128:].astype(BF16),
        biasp=biasp,
        bqh=np.ascontiguousarray(bq.reshape(H, DH).T),
        bkh=np.ascontiguousarray(bk.astype(F32).reshape(H, DH).T),
        idm=np.eye(128, dtype=F32), idmb=np.eye(128, dtype=BF16),
        gmat=None, u1col=None, onesf=np.ones((128, 1), F32),
        one1=np.ones((1, 1), F32),
    )

    vs = VSCALE if FP8V else 1.0
    Us_full = (U * s_lam[None, :]).astype(F32)
    dis_cl = np.where(dis > 0, dis, 1.0).astype(F32)
    xfT_np = np.ascontiguousarray(x.T).astype(BF16)
    discf_np = np.ascontiguousarray((vs * dis_cl).reshape(N // 128, 128).T)
    gmat_np = (Us_full.T @ Us_full).astype(F32)
    u1_np = np.ascontiguousarray(Us_full.sum(0).astype(F32)[:, None])
    in_maps = []
    for c in range(NCORES):
        sl = slice(c * NLOC, (c + 1) * NLOC)
        dis_c = dis[sl]
        m = dict(common)
        m["xT"] = np.ascontiguousarray(x[sl].T).astype(BF16)
        m["u_nm"] = np.ascontiguousarray(U[sl]).astype(BF16)
        m["usT"] = np.ascontiguousarray((U[sl] * s_lam[None, :]).T).astype(BF16)
        m["usd"] = np.ascontiguousarray(
            Us_full[sl] / dis_cl[sl][:, None]).astype(BF16)
        m["gmat"] = gmat_np
        m["u1col"] = u1_np
        m["xfT"] = xfT_np
        m["discf"] = discf_np
        us_loc = Us_full[sl]
        m["m1x"] = np.ascontiguousarray(
            x[sl].T @ us_loc
            + np.outer(inp["cheb_b"].astype(F32), us_loc.sum(0)))
        m["disc"] = np.ascontiguousarray(
            (vs * dis_cl[sl]).reshape(NT, 128).T)
        m["disc_m"] = np.ascontiguousarray(
            (-scale / vs * dis_c).reshape(NT, 128).T)
        m["gidx"] = cores[c]["gidx"]
        m["s8"] = cores[c]["s8"]
        in_maps.append(m)

    import os
    global LAST_NC, LAST_IN_MAPS
    LAST_NC = nc
    LAST_IN_MAPS = in_maps
    trace = os.environ.get("KERNEL_TRACE", "0") == "1"
    res = run_bass_kernel_spmd(nc, in_maps, core_ids=list(range(NCORES)),
                               trace=trace)
    global LAST_EXEC_NS, LAST_RESULT
    LAST_EXEC_NS = res.exec_time_ns
    LAST_RESULT = res
    out = np.concatenate([res.results[c]["out_nm"] for c in range(NCORES)], axis=0)
    return out.astype(inp["x"].dtype)



# revision 42
# speedup vs baseline: 1.8174x; 1.2137x over previous
"""Trainium2 Bass kernel for nn_CachedSpectralGPSLayer (8-core SPMD).

Self-contained: takes FULL inputs, shards per-core internally, runs one
Bass/Tile program SPMD on 8 NeuronCores, gathers the full output.
"""
import os
import sys

sys.path.insert(0, "/opt/trn_rl_repo")

import numpy as np
import ml_dtypes

import concourse.bacc as bacc
import concourse.bass as bass
import concourse.mybir as mybir
import concourse.tile as tile
from concourse import library_config
from concourse.bass_utils import run_bass_kernel_spmd

BF16 = ml_dtypes.bfloat16
FP8 = ml_dtypes.float8_e4m3
F32 = np.float32

N, C, K, KEIG, B, NG, H = 32768, 128, 5, 128, 64, 512, 4
NCORES = 8
NLOC = N // NCORES          # 4096
NT = NLOC // 128            # 32 node tiles per core
Bd = 64                     # dst nodes per block
NBLK = NLOC // Bd           # 64 blocks per core
GPC = B // NCORES           # 8 graphs per core
DH = C // H                 # 32
EPS = 1e-5
BPG = 2                     # blocks per dma_gather call
NCALL = NBLK // BPG         # 16 gather calls per hop
FP8V = False                # gather/AllGather payload (v~) in fp8e4m3
VSCALE = 16.0               # v~ pre-scale (keeps fp8 out of subnormals)
# virtual-schedule stamps (ms units): hop start = ST0 + STH*(h-1),
# window = hop + STW, tail = ST0 + STH*hops. Scheduler ordering hints only.
ST0 = float(os.environ.get("K_ST0", "0.45"))
STH = float(os.environ.get("K_STH", "0.40"))
STW = float(os.environ.get("K_STW", "0.20"))

fp32 = mybir.dt.float32
f32r = mybir.dt.float32r
bf16 = mybir.dt.bfloat16
fp8 = mybir.dt.float8e4
i16 = mybir.dt.int16


def R(ap):
    """Bitcast an fp32 AP to float32r: bit-identical fp32 data, but the PE
    runs replicated mode (1 cyc/row when moving dim >=256 vs 4 for fp32)."""
    return ap.bitcast(f32r)

# bias-pack column indices
(BI_SPA1, BI_SPA2, BI_SPE1, BI_SPE2, BI_Q, BI_K, BI_OUTP, BI_CHEB,
 BI_M1A, BI_M1B, BI_M2, BI_BN1W, BI_BN1B, BI_BN2W, BI_BN2B, BI_BN3W,
 BI_BN3B, BI_EPSC) = range(18)
NBIAS = 18

_CACHE = {}


def _wrap_idx(idx_flat):
    """dma_gather wrapped layout per call: idx i -> [i%16, i//16], replicated
    to all 8 groups of 16 partitions. idx_flat: [ncalls, n_per_call]."""
    ncalls, npc = idx_flat.shape
    base = idx_flat.reshape(ncalls, npc // 16, 16).transpose(0, 2, 1)  # [ncalls,16,npc/16]
    out = np.tile(base, (1, 8, 1))                                     # [ncalls,128,npc/16]
    return np.concatenate(list(out), axis=1)                           # [128, ncalls*npc/16]


def _preprocess(inputs):
    src = np.asarray(inputs["edge_index"][0]).astype(np.int64)
    dst = np.asarray(inputs["edge_index"][1]).astype(np.int64)
    deg = np.bincount(src, minlength=N).astype(np.float64)
    dis = np.where(deg > 0, 1.0 / np.sqrt(deg), 0.0).astype(F32)
    lam = float(np.asarray(inputs["lambda_max"]).reshape(-1)[0])
    scale = 2.0 / lam

    order = np.argsort(dst, kind="stable")
    srcs, dsts = src[order], dst[order]
    counts = np.bincount(dst // Bd, minlength=N // Bd)
    ngrp = int(np.ceil(counts.max() / 128))
    epb = ngrp * 128                    # padded edges per block
    epad = NBLK * epb                   # per core

    cores = []
    bounds = np.searchsorted(dsts, np.arange(0, N + 1, NLOC))
    for c in range(NCORES):
        lo = c * NLOC
        sl = slice(bounds[c], bounds[c + 1])
        sc, dc = srcs[sl], dsts[sl] - lo
        blk = dc // Bd
        ord2 = np.lexsort((sc, blk))  # sort by src within each dst block
        sc, dc, blk = sc[ord2], dc[ord2], blk[ord2]
        cnt = np.bincount(blk, minlength=NBLK)
        csum = np.concatenate([[0], np.cumsum(cnt)])
        pos_in_blk = np.arange(len(sc)) - csum[blk]
        slot = blk * epb + pos_in_blk
        src_pad = np.zeros(epad, np.int64)
        src_pad[slot] = sc
        # S one-hot fp8 bytes [128, ngroups*Bd]
        ngroups = NBLK * ngrp
        s8 = np.zeros((128, ngroups * Bd), np.uint8)
        g = slot // 128
        p = slot % 128
        s8[p, g * Bd + (dc - blk * Bd)] = 0x38  # fp8e4m3 1.0
        assert src_pad.max() < 2 ** 15
        gidx = _wrap_idx(src_pad.astype(np.int16).reshape(NCALL, BPG * epb))
        cores.append(dict(s8=s8.view(FP8), gidx=gidx))
    return cores, dis, scale, ngrp


def _build(ngrp, scale, hops=K - 1, do_mha=True, do_spec=True):
    do_spec = True
    """Build + compile the SPMD Bass program. Returns (nc, input names)."""
    epb = ngrp * 128
    epad = NBLK * epb
    ngroups = NBLK * ngrp

    nc = bacc.Bacc("TRN2", target_bir_lowering=False, debug=False,
                   enable_asserts=True, num_devices=NCORES,
                   num_swdge_queues=4)

    def din(name, shape, dt):
        return nc.dram_tensor(name, shape, dt, kind="ExternalInput").ap()

    xT = din("xT", [128, NLOC], bf16)
    u_nm = din("u_nm", [NLOC, 128], bf16)
    usT = din("usT", [128, NLOC], bf16)
    disc = din("disc", [128, NT], fp32)
    disc_m = din("disc_m", [128, NT], fp32)
    gidx = din("gidx", [128, epad // 16], i16)
    s8 = din("s8", [128, ngroups * Bd], fp8)
    wspa1 = din("wspa1", [128, 128], bf16)
    wspa2 = din("wspa2", [128, 128], bf16)
    wspe1 = din("wspe1", [128, 128], bf16)
    wspe2 = din("wspe2", [128, 128], bf16)
    wproj = din("wproj", [128, 128], bf16)
    chebw = din("chebw", [128, K * 128], bf16)
    wq = din("wq", [128, 128], bf16)
    wk = din("wk", [128, 128], bf16)
    wv = din("wv", [128, 128], bf16)
    wout = din("wout", [128, 128], bf16)
    mw1 = din("mw1", [128, 256], bf16)
    mw2a = din("mw2a", [128, 128], bf16)
    mw2b = din("mw2b", [128, 128], bf16)
    biasp = din("biasp", [128, NBIAS], fp32)
    bqh = din("bqh", [32, H], fp32)
    bkh = din("bkh", [32, H], fp32)
    idm = din("idm", [128, 128], fp32)
    gmat = din("gmat", [128, 128], fp32)
    u1col = din("u1col", [128, 1], fp32)
    usd = din("usd", [NLOC, 128], bf16)
    idmb = din("idmb", [128, 128], bf16)
    onesf = din("onesf", [128, 1], fp32)
    one1 = din("one1", [1, 1], fp32)
    m1x = din("m1x", [128, 128], fp32)
    xfT = din("xfT", [128, N], bf16)
    discf = din("discf", [128, N // 128], fp32)

    out_nm = nc.dram_tensor("out_nm", [NLOC, 128], fp32, kind="ExternalOutput").ap()

    AF = mybir.ActivationFunctionType
    OP = mybir.AluOpType

    with tile.TileContext(nc) as tc:
        with tc.tile_pool(name="const", bufs=1) as cp, \
             tc.tile_pool(name="big", bufs=1) as bp, \
             tc.tile_pool(name="work", bufs=2) as wp, \
             tc.tile_pool(name="psmm", bufs=2, space="PSUM") as psmm, \
             tc.tile_pool(name="psat", bufs=2, space="PSUM") as psatp, \
             tc.tile_pool(name="pspt", bufs=2, space="PSUM") as psptp, \
             tc.tile_pool(name="psagg", bufs=2, space="PSUM") as psaggp, \
             tc.tile_pool(name="dram", bufs=1, space="DRAM") as dp:

            # uniform-tag psum allocators (PSUM = 8 banks total: 2+2+2+2)
            def PS_MM():   # transient [128,512] matmul outputs
                return psmm.tile([128, 512], fp32, tag="mm", name="psmm_t")

            def PS_AT(shape):  # long-lived accumulators / phase-2 partial
                return psatp.tile(shape, fp32, tag="at", name="psat_t")

            def PS_ATW():  # MHA attn+denom accumulator [128, 132]
                return psatp.tile([128, 33 * H], fp32, tag="at", name="psatw_t")

            def PS_PT2():  # [32, 512] head q/k psum
                return psptp.tile([32, 512], fp32, tag="pt", name="pspt2_t")

            def PS_PT():   # [128,128] transposes / small matmuls
                return psptp.tile([128, 128], fp32, tag="pt", name="pspt_t")

            def PS_AGG():  # [128,128] cheb aggregation
                return psaggp.tile([128, 128], fp32, tag="agg", name="psagg_t")

            nc.gpsimd.load_library(library_config.mlp)

            # ---- load constants ----
            def ld(ap_in, shape, dt, name, eng=None):
                t = cp.tile(shape, dt, tag=name, name=name)
                (eng or nc.sync).dma_start(t[:], ap_in[:])
                return t

            xT_s = bp.tile([128, NLOC], bf16, tag="xT")
            nc.sync.dma_start(xT_s[:], xT[:])
            s8_s = ld(s8, [128, ngroups * Bd], fp8, "s8")
            wspa1_s = ld(wspa1, [128, 128], bf16, "wspa1")
            wspa2_s = ld(wspa2, [128, 128], bf16, "wspa2")
            wspe1_s = ld(wspe1, [128, 128], bf16, "wspe1")
            wspe2_s = ld(wspe2, [128, 128], bf16, "wspe2")
            wproj_s = ld(wproj, [128, 128], bf16, "wproj")
            chebw_s = ld(chebw, [128, K * 128], bf16, "chebw")
            wq_s = ld(wq, [128, 128], bf16, "wq")
            wk_s = ld(wk, [128, 128], bf16, "wk")
            wv_s = ld(wv, [128, 128], bf16, "wv")
            wout_s = ld(wout, [128, 128], bf16, "wout")
            mw1_s = ld(mw1, [128, 256], bf16, "mw1")
            mw2a_s = ld(mw2a, [128, 128], bf16, "mw2a")
            mw2b_s = ld(mw2b, [128, 128], bf16, "mw2b")
            biasp_s = ld(biasp, [128, NBIAS], fp32, "biasp")
            bqh_s = ld(bqh, [32, H], fp32, "bqh")
            bkh_s = ld(bkh, [32, H], fp32, "bkh")
            idm_s = ld(idm, [128, 128], fp32, "idm")
            idmb_s = ld(idmb, [128, 128], bf16, "idmb")
            vbuf = bp.tile([128, NT, 128], bf16, tag="vbuf")

            def th_accum(t_ps):
                # T_h = v~^T (Us/dis): stream usd in 8-tile chunks (window
                # work, off the gather critical path; scalar-queue DMA)
                for t0 in range(0, NT, 8):
                    ub8 = wp.tile([128, 8, 128], bf16, tag="ut8", bufs=1)
                    nc.scalar.dma_start(
                        ub8[:], usd[t0 * 128:(t0 + 8) * 128, :]
                        .rearrange("(t p) c -> p t c", p=128))
                    for j in range(8):
                        t = t0 + j
                        nc.tensor.matmul(t_ps[:], lhsT=vbuf[:, t, :],
                                         rhs=ub8[:, j, :],
                                         start=(t == 0), stop=(t == NT - 1))
            gmat_s = ld(gmat, [128, 128], fp32, "gmat")
            u1col_s = ld(u1col, [128, 1], fp32, "u1col")
            onesf_s = ld(onesf, [128, 1], fp32, "onesf")
            one1_s = ld(one1, [1, 1], fp32, "one1")
            m1x_s = ld(m1x, [128, 128], fp32, "m1x")
            m1sb = cp.tile([128, 128], fp32, tag="m1sb", name="m1sb")
            discf_s = ld(discf, [128, N // 128], fp32, "discf")
            disc_s = ld(disc, [128, NT], fp32, "disc")
            discm_s = ld(disc_m, [128, 2 * NT], fp32, "discm")
            discm2_s = discm_s[:, NT:2 * NT]

            def bcol(i):
                return biasp_s[:, i:i + 1]

            # persistent big buffers
            TxA = bp.tile([128, NT, 128], fp32, tag="TxA")   # node-major
            TxB = bp.tile([128, NT, 128], fp32, tag="TxB")
            chebT = bp.tile([128, NLOC], fp32, tag="chebT")  # later: pre1, outT
            h2T = bp.tile([128, NLOC], fp32, tag="h2T")      # later: pre2, out2T
            if not do_mha:
                nc.vector.memset(h2T[:], 0.0)
            statc = cp.tile([128, 32], fp32, tag="statc")    # stats/affine cols
            statc2 = cp.tile([128, 16], fp32, tag="statc2")  # per-graph h2 stats
            outnm = TxA  # reuse (dead after cheb)

            # DRAM bounce buffers
            vdt = fp8 if FP8V else bf16
            ag_in = dp.tile([NLOC, 128], vdt, tag="ag_in")
            ag_outs = [None] + [dp.tile([N, 128], vdt, tag=f"ag_out{h}", name=f"ag_out{h}", addr_space="Shared") for h in range(1, 4)]
            vfull0 = dp.tile([N, 128], vdt, tag="vfull0", name="vfull0")
            bn12_in = dp.tile([128, 260], fp32, tag="bn12_in")
            bn12_out = dp.tile([128, 260], fp32, tag="bn12_out", addr_space="Shared")
            bn3_in = dp.tile([128, 2], fp32, tag="bn3_in")
            bn3_out = dp.tile([128, 2], fp32, tag="bn3_out", addr_space="Shared")

            CH = NLOC // 512  # 8 chunks of 512

            from contextlib import ExitStack
            ep_stack = ExitStack()
            ep = ep_stack.enter_context(tc.tile_pool(name="early", bufs=1))
            xspT = ep.tile([128, NLOC], bf16, tag="xspT", name="xspT")

            # ================= Phase 1: local spa MLP (feature-major) =======
            # (the spe MLP + spectral partial run inside AllGather window 1,
            # recomputed from the resident xT_s, off the pre-hop critical path)
            for ch in range(CH):
                sl = slice(ch * 512, (ch + 1) * 512)
                p1 = PS_MM()
                nc.tensor.matmul(p1[:], lhsT=wspa1_s[:], rhs=xT_s[:, sl],
                                 start=True, stop=True)
                t1 = wp.tile([128, 512], bf16, tag="t1b")
                nc.scalar.activation(t1[:], p1[:], AF.Relu, bias=bcol(BI_SPA1))
                p2 = PS_MM()
                nc.tensor.matmul(p2[:], lhsT=wspa2_s[:], rhs=t1[:],
                                 start=True, stop=True)
                nc.scalar.activation(xspT[:, sl], p2[:], AF.Identity,
                                     bias=bcol(BI_SPA2))

            # Tx0 node-major (local shard, for recurrence) + v~0 into vbuf
            for t in range(NT):
                tsl = slice(t * 128, (t + 1) * 128)
                pt = PS_PT()
                nc.tensor.matmul(pt[:], lhsT=xspT[:, tsl], rhs=idmb_s[:],
                                 start=True, stop=True)
                nc.vector.tensor_copy(TxB[:, t, :], pt[:])
                nc.scalar.activation(vbuf[:, t, :], pt[:], AF.Identity,
                                     scale=disc_s[:, t:t + 1])
            # v~0 for ALL nodes computed locally (replaces hop-1 AllGather):
            # every core redundantly runs the spatial MLP over the full x.
            if hops >= 1:
                for gch in range(N // 512):
                    gsl2 = slice(gch * 512, (gch + 1) * 512)
                    xc = wp.tile([128, 512], bf16, tag="t1b")
                    nc.sync.dma_start(xc[:], xfT[:, gsl2])
                    pf1 = PS_MM()
                    nc.tensor.matmul(pf1[:], lhsT=wspa1_s[:], rhs=xc[:],
                                     start=True, stop=True)
                    tf1 = wp.tile([128, 512], bf16, tag="midab")
                    nc.scalar.activation(tf1[:], pf1[:], AF.Relu,
                                         bias=bcol(BI_SPA1))
                    pf2 = PS_MM()
                    nc.tensor.matmul(pf2[:], lhsT=wspa2_s[:], rhs=tf1[:],
                                     start=True, stop=True)
                    spf = wp.tile([128, 512], bf16, tag="t1b")
                    nc.scalar.activation(spf[:], pf2[:], AF.Identity,
                                         bias=bcol(BI_SPA2))
                    vt4 = wp.tile([128, 4, 128], vdt, tag="vt4", bufs=2)
                    for j in range(4):
                        tg = gch * 4 + j
                        ptf = PS_PT()
                        nc.tensor.matmul(ptf[:], lhsT=spf[:, j * 128:(j + 1) * 128],
                                         rhs=idmb_s[:], start=True, stop=True)
                        nc.scalar.activation(vt4[:, j, :], ptf[:], AF.Identity,
                                             scale=discf_s[:, tg:tg + 1])
                    nc.sync.dma_start(
                        vfull0[gch * 512:(gch + 1) * 512, :]
                        .rearrange("(t p) c -> p t c", p=128), vt4[:])

            for ch in range(CH):
                sl = slice(ch * 512, (ch + 1) * 512)
                pw = PS_MM()
                nc.tensor.matmul(pw[:], lhsT=chebw_s[:, 0:128], rhs=xspT[:, sl],
                                 start=True, stop=True)
                nc.vector.tensor_copy(chebT[:, sl], pw[:])
            # T_0 = v~0^T (Us/dis) (accumulate over tiles); m1sb = m1x + W0^T T_0
            t_ps = PS_AGG()
            th_accum(t_ps)
            tsb = wp.tile([128, 128], bf16, tag="tsb")
            nc.vector.tensor_copy(tsb[:], t_ps[:])
            pWt = PS_PT()
            nc.tensor.matmul(pWt[:], lhsT=chebw_s[:, 0:128], rhs=tsb[:],
                             start=True, stop=True)
            nc.vector.tensor_add(m1sb[:], m1x_s[:], pWt[:])

            # ===== Phase 2 (deferred): spectral partial, run in AG window 1 =
            # Recomputes the spe MLP chunk-wise from resident xT_s so xspecT
            # needs no SBUF residency across the hops.
            part_s = wp.tile([128, 128], fp32, tag="part_s", bufs=1)

            def spectral_partial():
                part_ps = PS_AT([128, 128])
                for ch2 in range(CH):
                    sl2 = slice(ch2 * 512, (ch2 + 1) * 512)
                    p3 = PS_MM()
                    nc.tensor.matmul(p3[:], lhsT=wspe1_s[:], rhs=xT_s[:, sl2],
                                     start=True, stop=True)
                    t2 = wp.tile([128, 512], bf16, tag="t1b")
                    nc.scalar.activation(t2[:], p3[:], AF.Relu,
                                         bias=bcol(BI_SPE1))
                    p4 = PS_MM()
                    nc.tensor.matmul(p4[:], lhsT=wspe2_s[:], rhs=t2[:],
                                     start=True, stop=True)
                    xsp2 = wp.tile([128, 512], bf16, tag="t1b")
                    nc.scalar.activation(xsp2[:], p4[:], AF.Identity,
                                         bias=bcol(BI_SPE2))
                    ub = wp.tile([128, 4, 128], bf16, tag="ut4", bufs=1)
                    nc.sync.dma_start(
                        ub[:], u_nm[ch2 * 512:(ch2 + 1) * 512, :]
                        .rearrange("(t p) c -> p t c", p=128))
                    for j in range(4):
                        t = ch2 * 4 + j
                        ph = PS_PT()
                        nc.tensor.matmul(ph[:], lhsT=xsp2[:, j * 128:(j + 1) * 128],
                                         rhs=wproj_s[:], start=True, stop=True)
                        hp = wp.tile([128, 128], bf16, tag="hp")
                        nc.vector.tensor_copy(hp[:], ph[:])
                        nc.tensor.matmul(part_ps[:], lhsT=ub[:, j, :], rhs=hp[:],
                                         start=(t == 0), stop=(t == NT - 1))
                nc.vector.tensor_copy(part_s[:], part_ps[:])

            ep_stack.close()  # free xspT space for later pools
            late_stack = ExitStack()
            gp = late_stack.enter_context(tc.tile_pool(name="gath", bufs=2))
            mp = late_stack.enter_context(tc.tile_pool(name="mha", bufs=2))

            # ---- MHA for one graph (interleaved into AllGather windows) ----
            def mha_graph(g):
                gsl = slice(g * 512, (g + 1) * 512)
                # head-major q/k: per-head matmuls so all operands are base-0
                qT = mp.tile([32, H * 512], bf16, tag="qT", bufs=1)
                kT = mp.tile([32, H * 512], bf16, tag="kT", bufs=1)
                for hh in range(H):
                    csl = slice(hh * 32, (hh + 1) * 32)
                    pqh = PS_PT2()
                    nc.tensor.matmul(pqh[:], lhsT=wq_s[:, csl],
                                     rhs=xT_s[:, gsl], start=True, stop=True)
                    nc.scalar.activation(qT[:, hh * 512:(hh + 1) * 512], pqh[:],
                                         AF.Identity, bias=bqh_s[:, hh:hh + 1])
                    pkh = PS_PT2()
                    nc.tensor.matmul(pkh[:], lhsT=wk_s[:, csl],
                                     rhs=xT_s[:, gsl], start=True, stop=True)
                    nc.scalar.activation(kT[:, hh * 512:(hh + 1) * 512], pkh[:],
                                         AF.Identity, bias=bkh_s[:, hh:hh + 1])
                # v node-major, augmented per head with a ones column
                vaug = mp.tile([128, 4, 33 * H], bf16, tag="vaug")
                nc.vector.memset(vaug[:, :, 32::33], 1.0)
                for j in range(4):
                    pv = PS_PT()
                    nc.tensor.matmul(pv[:], lhsT=xT_s[:, g * 512 + j * 128:
                                                      g * 512 + (j + 1) * 128],
                                     rhs=wv_s[:], start=True, stop=True)
                    for hh in range(H):
                        nc.vector.tensor_copy(
                            vaug[:, j, hh * 33:hh * 33 + 32],
                            pv[:, hh * 32:(hh + 1) * 32])
                # scores_T + exp, per (head, k-chunk)
                ess = {}
                for hh in range(H):
                    qsl = slice(hh * 512, (hh + 1) * 512)
                    for j in range(4):
                        pss = PS_MM()
                        nc.tensor.matmul(
                            pss[:], lhsT=kT[:, hh * 512 + j * 128:
                                            hh * 512 + (j + 1) * 128],
                            rhs=qT[:, qsl], start=True, stop=True)
                        es = mp.tile([128, 512], bf16, tag="es", bufs=16)
                        nc.scalar.activation(es[:], pss[:], AF.Exp)
                        ess[(hh, j)] = es
                # attn + denom per q-chunk
                for qc in range(4):
                    pat = PS_ATW()
                    for hh in range(H):
                        for j in range(4):
                            nc.tensor.matmul(
                                pat[:, hh * 33:(hh + 1) * 33],
                                lhsT=ess[(hh, j)][:, qc * 128:(qc + 1) * 128],
                                rhs=vaug[:, j, hh * 33:(hh + 1) * 33],
                                start=(j == 0), stop=(j == 3),
                                skip_group_check=True)
                    recip = wp.tile([128, 4], fp32, tag="recip")
                    nc.vector.reciprocal(recip[:], pat[:, 32::33])
                    anm = wp.tile([128, 128], fp32, tag="anm")
                    for hh in range(H):
                        nc.vector.tensor_scalar(
                            out=anm[:, hh * 32:(hh + 1) * 32],
                            in0=pat[:, hh * 33:hh * 33 + 32],
                            scalar1=recip[:, hh:hh + 1], scalar2=None,
                            op0=OP.mult)
                    ptr = PS_PT()
                    nc.tensor.transpose(ptr[:], anm[:], idm_s[:])
                    attnT = wp.tile([128, 128], bf16, tag="attnT", bufs=1)
                    nc.vector.tensor_copy(attnT[:], ptr[:])
                    ph2 = PS_PT()
                    nc.tensor.matmul(ph2[:], lhsT=wout_s[:], rhs=attnT[:],
                                     start=True, stop=True)
                    osl = slice(g * 512 + qc * 128, g * 512 + (qc + 1) * 128)
                    # pre2 = h2 + b_out' + x
                    nc.vector.scalar_tensor_tensor(
                        out=h2T[:, osl], in0=ph2[:], scalar=bcol(BI_OUTP),
                        in1=xT_s[:, osl], op0=OP.add, op1=OP.add)
                # incremental BN2 stats for this graph's 512 columns
                nc.vector.tensor_reduce(statc2[:, g:g + 1], h2T[:, gsl],
                                        mybir.AxisListType.X, OP.add)
                tsq = wp.tile([128, 512], fp32, tag="sqt", bufs=1)
                nc.vector.scalar_tensor_tensor(
                    out=tsq[:], in0=h2T[:, gsl], scalar=1.0, in1=h2T[:, gsl],
                    op0=OP.mult, op1=OP.mult,
                    accum_out=statc2[:, 8 + g:9 + g])

            # graphs run inside AllGather wait windows (PE idle otherwise)
            mha_sched = {1: [0, 1, 2], 2: [3, 4, 5], 3: [6, 7]} \
                if (do_mha and hops == K - 1) else {}
            mha_left = [g for g in range(GPC if do_mha else 0)
                        if not any(g in v for v in mha_sched.values())]

            # ================= Phase 3: cheb hops ===========================
            # tile_wait_until stamps are scheduler-only hints (virtual
            # earliest-start): they stop the list scheduler from hoisting
            # hop h+1's recurrence ops ahead of the window-h MHA work in
            # the in-order DVE queue (head-of-line blocking during the
            # AllGather). They emit no HW waits.
            cur, prev = TxB, TxA  # cur holds Tx_{h-1}; prev gets Tx_h
            for h in range(1, 1 + hops):
                t_hop = ST0 + STH * (h - 1)
                ag_src = vfull0 if h == 1 else ag_outs[h - 1]
                hop_stack = ExitStack()
                hop_stack.enter_context(tc.tile_wait_until(t_hop))
                for q in range(NCALL):
                    gt = gp.tile([128, BPG * ngrp, 128], vdt, tag="gt", bufs=6)
                    isl = slice(q * BPG * epb // 16, (q + 1) * BPG * epb // 16)
                    gix = wp.tile([128, BPG * epb // 16], i16, tag="gix", bufs=6)
                    nc.sync.dma_start(gix[:], gidx[:, isl])
                    nc.gpsimd.dma_gather(gt[:], ag_src[:], gix[:],
                                         BPG * epb, BPG * epb, 128,
                                         single_packet=False,
                                         queue_num=q % 4)
                    for r in range(BPG):
                        b = q * BPG + r
                        t, half = b // 2, b % 2
                        if half == 0:
                            aps = PS_AGG()
                        for j in range(ngrp):
                            gcol = b * ngrp + j
                            nc.tensor.matmul(
                                aps[half * 64:(half + 1) * 64, :],
                                lhsT=s8_s[:, gcol * Bd:(gcol + 1) * Bd],
                                rhs=gt[:, r * ngrp + j, :],
                                start=(j == 0), stop=(j == ngrp - 1))
                        if half == 1:
                            # recurrence for tile t
                            tmp = wp.tile([128, 128], fp32, tag="rectmp")
                            if h == 1:
                                nc.vector.tensor_scalar(
                                    out=tmp[:], in0=aps[:],
                                    scalar1=discm_s[:, t:t + 1], scalar2=None,
                                    op0=OP.mult)
                                # Tx1 = (scale-1)*Tx0 + tmp
                                nc.vector.scalar_tensor_tensor(
                                    out=prev[:, t, :], in0=cur[:, t, :],
                                    scalar=float(scale - 1.0), in1=tmp[:],
                                    op0=OP.mult, op1=OP.add)
                            else:
                                nc.vector.tensor_scalar(
                                    out=tmp[:], in0=aps[:],
                                    scalar1=discm_s[:, t:t + 1], scalar2=2.0,
                                    op0=OP.mult, op1=OP.mult)
                                # tmp2 = tmp - Tx_{h-2}
                                tmp2 = wp.tile([128, 128], fp32, tag="rectmp2")
                                nc.vector.scalar_tensor_tensor(
                                    out=tmp2[:], in0=prev[:, t, :],
                                    scalar=-1.0, in1=tmp[:],
                                    op0=OP.mult, op1=OP.add)
                                # Tx_h = 2(scale-1)*Tx_{h-1} + tmp2
                                nc.vector.scalar_tensor_tensor(
                                    out=prev[:, t, :], in0=cur[:, t, :],
                                    scalar=float(2.0 * (scale - 1.0)),
                                    in1=tmp2[:], op0=OP.mult, op1=OP.add)
                            nc.scalar.activation(
                                vbuf[:, t, :], prev[:, t, :], AF.Identity,
                                scale=disc_s[:, t:t + 1])
                            if h < hops:
                                nc.sync.dma_start(
                                    ag_in[t * 128:(t + 1) * 128, :],
                                    vbuf[:, t, :])
                # launch AG for next hop once all v~ tiles written
                if h < hops:
                    nc.gpsimd.collective_compute(
                        "AllGather", OP.bypass,
                        replica_groups=[list(range(NCORES))],
                        ins=[ag_in.opt()], outs=[ag_outs[h].opt()])
                hop_stack.close()
                win_stack = ExitStack()
                win_stack.enter_context(tc.tile_wait_until(t_hop + STW))
                # out_cheb += Tx_h @ W_h  (transpose tiles chunk-wise)
                for ch in range(CH):
                    tpb = wp.tile([128, 512], bf16, tag="tpb", bufs=1)
                    for j in range(4):
                        t = ch * 4 + j
                        pt = PS_PT()
                        nc.tensor.transpose(pt[:], prev[:, t, :], idm_s[:])
                        nc.vector.tensor_copy(tpb[:, j * 128:(j + 1) * 128], pt[:])
                    sl = slice(ch * 512, (ch + 1) * 512)
                    pw = PS_MM()
                    nc.tensor.matmul(pw[:], lhsT=chebw_s[:, h * 128:(h + 1) * 128],
                                     rhs=tpb[:], start=True, stop=True)
                    nc.vector.tensor_add(chebT[:, sl], chebT[:, sl], pw[:])
                # T_h = v~_h^T (Us/dis) ; m1sb += W_h^T T_h  (off the tail)
                t_ps = PS_AGG()
                th_accum(t_ps)
                tsb = wp.tile([128, 128], bf16, tag="tsb")
                nc.vector.tensor_copy(tsb[:], t_ps[:])
                pWt = PS_PT()
                nc.tensor.matmul(pWt[:], lhsT=chebw_s[:, h * 128:(h + 1) * 128],
                                 rhs=tsb[:], start=True, stop=True)
                nc.vector.tensor_add(m1sb[:], m1sb[:], pWt[:])
                if h == 1:
                    spectral_partial()
                for g in mha_sched.get(h, []):
                    mha_graph(g)
                win_stack.close()
                cur, prev = prev, cur

            # ================= Phase 4: MHA (remaining graphs) ==============
            for g in mha_left:
                mha_graph(g)

            tail_stack = ExitStack()
            tail_stack.enter_context(tc.tile_wait_until(ST0 + STH * hops))
            # ===== Phase 5: pre1' (no spec) + BN stats + M1 + joint AR ======
            for ch in range(CH):
                sl = slice(ch * 512, (ch + 1) * 512)
                # pre1' = chebT + cheb_b + x   (overwrite chebT)
                nc.vector.scalar_tensor_tensor(
                    out=chebT[:, sl], in0=chebT[:, sl], scalar=bcol(BI_CHEB),
                    in1=xT_s[:, sl], op0=OP.add, op1=OP.add)
            # BN1/BN2 stats
            def sumsq(buf, out_col):
                for c2 in range(CH):
                    s2 = slice(c2 * 512, (c2 + 1) * 512)
                    tt = wp.tile([128, 512], fp32, tag="sqt", bufs=1)
                    nc.vector.scalar_tensor_tensor(
                        out=tt[:], in0=buf[:, s2], scalar=1.0, in1=buf[:, s2],
                        op0=OP.mult, op1=OP.mult,
                        accum_out=statc[:, 24 + c2:25 + c2])
                nc.vector.tensor_reduce(out_col, statc[:, 24:32],
                                        mybir.AxisListType.X, OP.add)

            nc.vector.tensor_reduce(statc[:, 0:1], chebT[:], mybir.AxisListType.X, OP.add)
            sumsq(chebT, statc[:, 1:2])
            if do_mha:
                nc.vector.tensor_reduce(statc[:, 2:3], statc2[:, 0:8],
                                        mybir.AxisListType.X, OP.add)
                nc.vector.tensor_reduce(statc[:, 3:4], statc2[:, 8:16],
                                        mybir.AxisListType.X, OP.add)
            else:
                nc.vector.tensor_reduce(statc[:, 2:3], h2T[:], mybir.AxisListType.X, OP.add)
                sumsq(h2T, statc[:, 3:4])
            st12 = wp.tile([128, 260], fp32, tag="st12", bufs=1)
            nc.vector.tensor_copy(st12[:, 0:4], statc[:, 0:4])
            nc.vector.tensor_copy(st12[:, 4:132], m1sb[:])
            nc.vector.tensor_copy(st12[:, 132:260], part_s[:])
            nc.sync.dma_start(bn12_in[:], st12[:])
            nc.gpsimd.collective_compute(
                "AllReduce", OP.add, replica_groups=[list(range(NCORES))],
                ins=[bn12_in.opt()], outs=[bn12_out.opt()])
            sr12 = wp.tile([128, 260], fp32, tag="sr12", bufs=1)
            nc.sync.dma_start(sr12[:], bn12_out[:])
            Pm = sr12[:, 132:260]   # AR'd spectral partial [keig, C]
            M1g = sr12[:, 4:132]    # [C, keig]
            # spectral stat terms:
            # s_sum[c] = sum_k u1[k] P[k,c];  s_sq[c] = sum_k P[k,c](G P)[k,c]
            # cross[c] = sum_k M1g[c,k] P[k,c]
            w12 = wp.tile([128, 256], fp32, tag="w12", bufs=1)
            nc.vector.tensor_scalar(out=w12[:, 0:128], in0=Pm,
                                    scalar1=u1col_s[:], scalar2=None,
                                    op0=OP.mult)
            t1_ps = PS_PT()
            nc.tensor.matmul(t1_ps[:], lhsT=gmat_s[:], rhs=Pm,
                             start=True, stop=True)
            nc.vector.tensor_tensor(w12[:, 128:256], t1_ps[:], Pm, OP.mult)
            # column sums: ones-matmul -> [1,256] row, then row -> two cols
            r_ps = PS_MM()
            nc.tensor.matmul(r_ps[0:1, 0:256], lhsT=onesf_s[:], rhs=w12[:],
                             start=True, stop=True)
            rowbuf = wp.tile([1, 256], fp32, tag="rowbuf")
            nc.vector.tensor_copy(rowbuf[:], r_ps[0:1, 0:256])
            c_ps = PS_PT()
            nc.tensor.matmul(c_ps[:, 0:1], lhsT=rowbuf[:, 0:128], rhs=one1_s[:],
                             start=True, stop=True, skip_group_check=True)
            nc.tensor.matmul(c_ps[:, 1:2], lhsT=rowbuf[:, 128:256], rhs=one1_s[:],
                             start=True, stop=True, skip_group_check=True)
            s_cols = wp.tile([128, 2], fp32, tag="s_cols")
            nc.vector.tensor_copy(s_cols[:], c_ps[:, 0:2])
            # cross: transpose P, multiply with M1g, reduce
            pt_ps = PS_PT()
            nc.tensor.transpose(pt_ps[:], sr12[:, 132:260], idm_s[:])
            ptm = wp.tile([128, 128], fp32, tag="w1t")
            nc.vector.tensor_tensor(ptm[:], pt_ps[:], M1g, OP.mult)
            crossc = statc[:, 14:15]
            nc.vector.tensor_reduce(crossc, ptm[:], mybir.AxisListType.X, OP.add)
            # BN1 totals: sum1 = sr12[:,0] + s_sum ; sq1 = sr12[:,1] + 2*cross + s_sq
            sum1c = statc[:, 15:16]
            nc.vector.tensor_tensor(sum1c, sr12[:, 0:1], s_cols[:, 0:1], OP.add)
            sq1c = statc[:, 16:17]
            nc.vector.scalar_tensor_tensor(out=sq1c, in0=crossc, scalar=2.0,
                                           in1=sr12[:, 1:2], op0=OP.mult,
                                           op1=OP.add)
            nc.vector.tensor_tensor(sq1c, sq1c, s_cols[:, 1:2], OP.add)

            # affine coefs: A = w/sqrt(var+eps), Bc = b - mu*A
            def bn_affine(sum_col, sq_col, w_col, b_col, a_out, b_out_col):
                mu = statc[:, 8:9]
                nc.vector.tensor_scalar(out=mu, in0=sum_col, scalar1=1.0 / N,
                                        scalar2=None, op0=OP.mult)
                msq = statc[:, 9:10]
                nc.vector.tensor_scalar(out=msq, in0=sq_col, scalar1=1.0 / N,
                                        scalar2=None, op0=OP.mult)
                nvar = statc[:, 10:11]
                nc.vector.scalar_tensor_tensor(out=nvar, in0=mu, scalar=mu,
                                               in1=msq, op0=OP.mult,
                                               op1=OP.subtract)  # mu^2 - msq
                sd = statc[:, 11:12]
                nc.scalar.activation(sd, nvar, AF.Sqrt, bias=bcol(BI_EPSC),
                                     scale=-1.0)
                rsd = statc[:, 12:13]
                nc.vector.reciprocal(rsd, sd)
                nc.vector.tensor_tensor(a_out, rsd, w_col, OP.mult)
                nbc = statc[:, 13:14]
                nc.vector.scalar_tensor_tensor(out=nbc, in0=mu, scalar=a_out,
                                               in1=b_col, op0=OP.mult,
                                               op1=OP.subtract)  # mu*A - b
                nc.vector.tensor_scalar(out=b_out_col, in0=nbc, scalar1=-1.0,
                                        scalar2=None, op0=OP.mult)

            A1, B1 = statc[:, 4:5], statc[:, 5:6]
            A2, B2 = statc[:, 6:7], statc[:, 7:8]
            bn_affine(statc[:, 15:16], statc[:, 16:17], bcol(BI_BN1W), bcol(BI_BN1B), A1, B1)
            bn_affine(sr12[:, 2:3], sr12[:, 3:4], bcol(BI_BN2W), bcol(BI_BN2B), A2, B2)
            B12 = statc[:, 5:6]
            nc.vector.tensor_tensor(B12, B1, B2, OP.add)  # B1 += B2 (in place)

            # ============ Phase 6: out = h1 + h2n; MLP2; BN3 ================
            out2T = h2T   # overwrite pre2 per chunk
            Pmb = wp.tile([128, 128], bf16, tag="pmb16", bufs=1)
            nc.vector.tensor_copy(Pmb[:], sr12[:, 132:260])
            for ch in range(CH):
                sl = slice(ch * 512, (ch + 1) * 512)
                ust = wp.tile([128, 512], bf16, tag="ust")
                nc.scalar.dma_start(ust[:], usT[:, sl])
                pso = PS_MM()
                nc.tensor.matmul(pso[:], lhsT=Pmb[:], rhs=ust[:],
                                 start=True, stop=True)
                t1 = wp.tile([128, 512], fp32, tag="t1")
                nc.scalar.activation(t1[:], chebT[:, sl], AF.Identity,
                                     bias=B12, scale=A1)
                tsp = wp.tile([128, 512], fp32, tag="t1")
                nc.vector.scalar_tensor_tensor(
                    out=tsp[:], in0=pso[:], scalar=A1, in1=t1[:],
                    op0=OP.mult, op1=OP.add)
                outT = wp.tile([128, 512], bf16, tag="outTb", bufs=2)
                nc.vector.scalar_tensor_tensor(
                    out=outT[:], in0=h2T[:, sl], scalar=A2, in1=tsp[:],
                    op0=OP.mult, op1=OP.add)
                pma = PS_MM()
                nc.tensor.matmul(pma[:], lhsT=mw1_s[:, 0:128], rhs=outT[:],
                                 start=True, stop=True)
                mida = wp.tile([128, 512], bf16, tag="midab")
                nc.scalar.activation(mida[:], pma[:], AF.Relu, bias=bcol(BI_M1A))
                pmb = PS_MM()
                nc.tensor.matmul(pmb[:], lhsT=mw1_s[:, 128:256], rhs=outT[:],
                                 start=True, stop=True)
                midb = wp.tile([128, 512], bf16, tag="midab")
                nc.scalar.activation(midb[:], pmb[:], AF.Relu, bias=bcol(BI_M1B))
                pmo = PS_MM()
                nc.tensor.matmul(pmo[:], lhsT=mw2a_s[:], rhs=mida[:],
                                 start=True, stop=False)
                nc.tensor.matmul(pmo[:], lhsT=mw2b_s[:], rhs=midb[:],
                                 start=False, stop=True)
                # out2 = out + mlp_b2 + psum
                nc.vector.scalar_tensor_tensor(
                    out=out2T[:, sl], in0=outT[:], scalar=bcol(BI_M2),
                    in1=pmo[:], op0=OP.add, op1=OP.add)
            # BN3 stats
            nc.vector.tensor_reduce(statc[:, 0:1], out2T[:], mybir.AxisListType.X, OP.add)
            sumsq(out2T, statc[:, 1:2])
            st3 = wp.tile([128, 2], fp32, tag="st")
            nc.vector.tensor_copy(st3[:], statc[:, 0:2])
            nc.sync.dma_start(bn3_in[:], st3[:])
            nc.gpsimd.collective_compute(
                "AllReduce", OP.add, replica_groups=[list(range(NCORES))],
                ins=[bn3_in.opt()], outs=[bn3_out.opt()])
            sr3 = wp.tile([128, 2], fp32, tag="st")
            nc.sync.dma_start(sr3[:], bn3_out[:])
            A3, B3 = statc[:, 4:5], statc[:, 5:6]
            bn_affine(sr3[:, 0:1], sr3[:, 1:2], bcol(BI_BN3W), bcol(BI_BN3B), A3, B3)

            # apply BN3, transpose to node-major, write out
            for ch in range(CH):
                sl = slice(ch * 512, (ch + 1) * 512)
                bn3b = wp.tile([128, 512], fp32, tag="t1")
                nc.scalar.activation(bn3b[:], out2T[:, sl], AF.Identity,
                                     bias=B3, scale=A3)
                for j in range(4):
                    t = ch * 4 + j
                    pt = PS_PT()
                    nc.tensor.transpose(pt[:], bn3b[:, j * 128:(j + 1) * 128],
                                        idm_s[:])
                    nc.vector.tensor_copy(outnm[:, t, :], pt[:])
            nc.sync.dma_start(
                out_nm[:].rearrange("(t p) c -> p t c", p=128), outnm[:])
            tail_stack.close()
            late_stack.close()

    nc.compile()
    return nc


def kernel(**inputs):
    inp = {k: np.asarray(v) for k, v in inputs.items()}
    cores, dis, scale, ngrp = _preprocess(inp)

    key = (ngrp, float(scale))
    if key not in _CACHE:
        _CACHE[key] = _build(ngrp, scale)
    nc = _CACHE[key]

    x = inp["x"].astype(F32)
    U = inp["U"].astype(F32)
    s_lam = np.exp(-float(inp["gamma"].reshape(-1)[0]) *
                   inp["Lambda"].astype(np.float64) ** 2).astype(F32)

    wqkv = inp["w_qkv"].astype(F32)
    bqkv = inp["b_qkv"].astype(F32)
    wq = (wqkv[:, :C] / np.sqrt(DH)).astype(F32)
    bq = (bqkv[:C] / np.sqrt(DH)).astype(F32)
    wk, bk = wqkv[:, C:2 * C].copy(), bqkv[C:2 * C]
    wv, bv = wqkv[:, 2 * C:].copy(), bqkv[2 * C:]
    b_out_p = (bv @ inp["w_out"] + inp["b_out"]).astype(F32)

    biasp = np.zeros((128, NBIAS), F32)
    for i, vec in [(BI_SPA1, inp["b_spa1"]), (BI_SPA2, inp["b_spa2"]),
                   (BI_SPE1, inp["b_spe1"]), (BI_SPE2, inp["b_spe2"]),
                   (BI_Q, bq), (BI_K, bk), (BI_OUTP, b_out_p),
                   (BI_CHEB, inp["cheb_b"]),
                   (BI_M1A, inp["mlp_b1"][:128]), (BI_M1B, inp["mlp_b1"][128:]),
                   (BI_M2, inp["mlp_b2"]),
                   (BI_BN1W, inp["bn1_w"]), (BI_BN1B, inp["bn1_b"]),
                   (BI_BN2W, inp["bn2_w"]), (BI_BN2B, inp["bn2_b"]),
                   (BI_BN3W, inp["bn3_w"]), (BI_BN3B, inp["bn3_b"]),
                   (BI_EPSC, np.full(128, EPS, F32))]:
        biasp[:, i] = vec.astype(F32)

    chebw_cols = np.concatenate([inp["cheb_w"][k].astype(F32) for k in range(K)],
                                axis=1)  # [128, 5*128]

    common = dict(
        wspa1=inp["w_spa1"].astype(BF16), wspa2=inp["w_spa2"].astype(BF16),
        wspe1=inp["w_spe1"].astype(BF16), wspe2=inp["w_spe2"].astype(BF16),
        wproj=inp["w_proj"].astype(BF16), chebw=chebw_cols.astype(BF16),
        wq=wq.astype(BF16), wk=wk.astype(BF16), wv=wv.astype(BF16),
        wout=inp["w_out"].astype(BF16),
        mw1=inp["mlp_w1"].astype(BF16),
        mw2a=inp["mlp_w2"][:128].astype(BF16), mw2b=inp["mlp_w2"][128:].astype(BF16),
        biasp=biasp,
        bqh=np.ascontiguousarray(bq.reshape(H, DH).T),
        bkh=np.ascontiguousarray(bk.astype(F32).reshape(H, DH).T),
        idm=np.eye(128, dtype=F32), idmb=np.eye(128, dtype=BF16),
        gmat=None, u1col=None, onesf=np.ones((128, 1), F32),
        one1=np.ones((1, 1), F32),
    )

    vs = VSCALE if FP8V else 1.0
    Us_full = (U * s_lam[None, :]).astype(F32)
    dis_cl = np.where(dis > 0, dis, 1.0).astype(F32)
    xfT_np = np.ascontiguousarray(x.T).astype(BF16)
    discf_np = np.ascontiguousarray((vs * dis_cl).reshape(N // 128, 128).T)
    gmat_np = (Us_full.T @ Us_full).astype(F32)
    u1_np = np.ascontiguousarray(Us_full.sum(0).astype(F32)[:, None])
    in_maps = []
    for c in range(NCORES):
        sl = slice(c * NLOC, (c + 1) * NLOC)
        dis_c = dis[sl]
        m = dict(common)
        m["xT"] = np.ascontiguousarray(x[sl].T).astype(BF16)
        m["u_nm"] = np.ascontiguousarray(U[sl]).astype(BF16)
        m["usT"] = np.ascontiguousarray((U[sl] * s_lam[None, :]).T).astype(BF16)
        m["usd"] = np.ascontiguousarray(
            Us_full[sl] / dis_cl[sl][:, None]).astype(BF16)
        m["gmat"] = gmat_np
        m["u1col"] = u1_np
        m["xfT"] = xfT_np
        m["discf"] = discf_np
        us_loc = Us_full[sl]
        m["m1x"] = np.ascontiguousarray(
            x[sl].T @ us_loc
            + np.outer(inp["cheb_b"].astype(F32), us_loc.sum(0)))
        m["disc"] = np.ascontiguousarray(
            (vs * dis_cl[sl]).reshape(NT, 128).T)
        m["disc_m"] = np.ascontiguousarray(
            (-scale / vs * dis_c).reshape(NT, 128).T)
        m["gidx"] = cores[c]["gidx"]
        m["s8"] = cores[c]["s8"]
        in_maps.append(m)

    import os
    global LAST_NC, LAST_IN_MAPS
    LAST_NC = nc
    LAST_IN_MAPS = in_maps
    trace = os.environ.get("KERNEL_TRACE", "0") == "1"
    res = run_bass_kernel_spmd(nc, in_maps, core_ids=list(range(NCORES)),
                               trace=trace)
    global LAST_EXEC_NS, LAST_RESULT
    LAST_EXEC_NS = res.exec_time_ns
    LAST_RESULT = res
    out = np.concatenate([res.results[c]["out_nm"] for c in range(NCORES)], axis=0)
    return out.astype(inp["x"].dtype)



# revision 46
# speedup vs baseline: 1.9370x; 1.0658x over previous
"""Trainium2 Bass kernel for nn_CachedSpectralGPSLayer (8-core SPMD).

Self-contained: takes FULL inputs, shards per-core internally, runs one
Bass/Tile program SPMD on 8 NeuronCores, gathers the full output.
"""
import os
import sys

sys.path.insert(0, "/opt/trn_rl_repo")

import numpy as np
import ml_dtypes

import concourse.bacc as bacc
import concourse.bass as bass
import concourse.mybir as mybir
import concourse.tile as tile
from concourse import library_config
from concourse.bass_utils import run_bass_kernel_spmd

BF16 = ml_dtypes.bfloat16
FP8 = ml_dtypes.float8_e4m3
F32 = np.float32

N, C, K, KEIG, B, NG, H = 32768, 128, 5, 128, 64, 512, 4
NCORES = 8
NLOC = N // NCORES          # 4096
NT = NLOC // 128            # 32 node tiles per core
Bd = 64                     # dst nodes per block
NBLK = NLOC // Bd           # 64 blocks per core
GPC = B // NCORES           # 8 graphs per core
DH = C // H                 # 32
EPS = 1e-5
BPG = 1                     # blocks per dma_gather call
NCALL = NBLK // BPG         # 16 gather calls per hop
FP8V = False                # gather/AllGather payload (v~) in fp8e4m3
VSCALE = 16.0               # v~ pre-scale (keeps fp8 out of subnormals)
# virtual-schedule stamps (ms units): hop start = ST0 + STH*(h-1),
# window = hop + STW, tail = ST0 + STH*hops. Scheduler ordering hints only.
ST0 = float(os.environ.get("K_ST0", "0.45"))
STH = float(os.environ.get("K_STH", "0.40"))
STW = float(os.environ.get("K_STW", "0.20"))

fp32 = mybir.dt.float32
f32r = mybir.dt.float32r
bf16 = mybir.dt.bfloat16
fp8 = mybir.dt.float8e4
i16 = mybir.dt.int16


def R(ap):
    """Bitcast an fp32 AP to float32r: bit-identical fp32 data, but the PE
    runs replicated mode (1 cyc/row when moving dim >=256 vs 4 for fp32)."""
    return ap.bitcast(f32r)

# bias-pack column indices
(BI_SPA1, BI_SPA2, BI_SPE1, BI_SPE2, BI_Q, BI_K, BI_OUTP, BI_CHEB,
 BI_M1A, BI_M1B, BI_M2, BI_BN1W, BI_BN1B, BI_BN2W, BI_BN2B, BI_BN3W,
 BI_BN3B, BI_EPSC) = range(18)
NBIAS = 18

_CACHE = {}


def _wrap_idx(idx_flat):
    """dma_gather wrapped layout per call: idx i -> [i%16, i//16], replicated
    to all 8 groups of 16 partitions. idx_flat: [ncalls, n_per_call]."""
    ncalls, npc = idx_flat.shape
    base = idx_flat.reshape(ncalls, npc // 16, 16).transpose(0, 2, 1)  # [ncalls,16,npc/16]
    out = np.tile(base, (1, 8, 1))                                     # [ncalls,128,npc/16]
    return np.concatenate(list(out), axis=1)                           # [128, ncalls*npc/16]


def _preprocess(inputs):
    src = np.asarray(inputs["edge_index"][0]).astype(np.int64)
    dst = np.asarray(inputs["edge_index"][1]).astype(np.int64)
    deg = np.bincount(src, minlength=N).astype(np.float64)
    dis = np.where(deg > 0, 1.0 / np.sqrt(deg), 0.0).astype(F32)
    lam = float(np.asarray(inputs["lambda_max"]).reshape(-1)[0])
    scale = 2.0 / lam

    order = np.argsort(dst, kind="stable")
    srcs, dsts = src[order], dst[order]
    counts = np.bincount(dst // Bd, minlength=N // Bd)
    ngrp = int(np.ceil(counts.max() / 128))
    epb = ngrp * 128                    # padded edges per block
    epad = NBLK * epb                   # per core

    cores = []
    bounds = np.searchsorted(dsts, np.arange(0, N + 1, NLOC))
    for c in range(NCORES):
        lo = c * NLOC
        sl = slice(bounds[c], bounds[c + 1])
        sc, dc = srcs[sl], dsts[sl] - lo
        blk = dc // Bd
        ord2 = np.lexsort((sc, blk))  # sort by src within each dst block
        sc, dc, blk = sc[ord2], dc[ord2], blk[ord2]
        cnt = np.bincount(blk, minlength=NBLK)
        csum = np.concatenate([[0], np.cumsum(cnt)])
        pos_in_blk = np.arange(len(sc)) - csum[blk]
        slot = blk * epb + pos_in_blk
        src_pad = np.zeros(epad, np.int64)
        src_pad[slot] = sc
        # S one-hot fp8 bytes [128, ngroups*Bd]
        ngroups = NBLK * ngrp
        s8 = np.zeros((128, ngroups * Bd), np.uint8)
        g = slot // 128
        p = slot % 128
        s8[p, g * Bd + (dc - blk * Bd)] = 0x38  # fp8e4m3 1.0
        assert src_pad.max() < 2 ** 15
        gidx = _wrap_idx(src_pad.astype(np.int16).reshape(NCALL, BPG * epb))
        cores.append(dict(s8=s8.view(FP8), gidx=gidx))
    return cores, dis, scale, ngrp


def _build(ngrp, scale, hops=K - 1, do_mha=True, do_spec=True):
    do_spec = True
    """Build + compile the SPMD Bass program. Returns (nc, input names)."""
    epb = ngrp * 128
    epad = NBLK * epb
    ngroups = NBLK * ngrp

    nc = bacc.Bacc("TRN2", target_bir_lowering=False, debug=False,
                   enable_asserts=True, num_devices=NCORES,
                   num_swdge_queues=4)

    def din(name, shape, dt):
        return nc.dram_tensor(name, shape, dt, kind="ExternalInput").ap()

    xT = din("xT", [128, NLOC], bf16)
    u_nm = din("u_nm", [NLOC, 128], bf16)
    usT = din("usT", [128, NLOC], bf16)
    disc = din("disc", [128, NT], fp32)
    disc_m = din("disc_m", [128, NT], fp32)
    gidx = din("gidx", [128, epad // 16], i16)
    s8 = din("s8", [128, ngroups * Bd], fp8)
    wspa1 = din("wspa1", [128, 128], bf16)
    wspa2 = din("wspa2", [128, 128], bf16)
    wspe1 = din("wspe1", [128, 128], bf16)
    wspe2 = din("wspe2", [128, 128], bf16)
    wproj = din("wproj", [128, 128], bf16)
    chebw = din("chebw", [128, K * 128], bf16)
    wq = din("wq", [128, 128], bf16)
    wk = din("wk", [128, 128], bf16)
    wv = din("wv", [128, 128], bf16)
    wout = din("wout", [128, 128], bf16)
    mw1 = din("mw1", [128, 256], bf16)
    mw2a = din("mw2a", [128, 128], bf16)
    mw2b = din("mw2b", [128, 128], bf16)
    biasp = din("biasp", [128, NBIAS], fp32)
    bqh = din("bqh", [32, H], fp32)
    bkh = din("bkh", [32, H], fp32)
    idm = din("idm", [128, 128], fp32)
    gmat = din("gmat", [128, 128], fp32)
    u1col = din("u1col", [128, 1], fp32)
    usd = din("usd", [NLOC, 128], bf16)
    idmb = din("idmb", [128, 128], bf16)
    onesf = din("onesf", [128, 1], fp32)
    one1 = din("one1", [1, 1], fp32)
    m1x = din("m1x", [128, 128], fp32)
    xfT = din("xfT", [128, N], bf16)
    discf = din("discf", [128, N // 128], fp32)

    out_nm = nc.dram_tensor("out_nm", [NLOC, 128], fp32, kind="ExternalOutput").ap()

    AF = mybir.ActivationFunctionType
    OP = mybir.AluOpType

    with tile.TileContext(nc) as tc:
        with tc.tile_pool(name="const", bufs=1) as cp, \
             tc.tile_pool(name="big", bufs=1) as bp, \
             tc.tile_pool(name="work", bufs=2) as wp, \
             tc.tile_pool(name="psmm", bufs=2, space="PSUM") as psmm, \
             tc.tile_pool(name="psat", bufs=2, space="PSUM") as psatp, \
             tc.tile_pool(name="pspt", bufs=2, space="PSUM") as psptp, \
             tc.tile_pool(name="psagg", bufs=2, space="PSUM") as psaggp, \
             tc.tile_pool(name="dram", bufs=1, space="DRAM") as dp:

            # uniform-tag psum allocators (PSUM = 8 banks total: 2+2+2+2)
            def PS_MM():   # transient [128,512] matmul outputs
                return psmm.tile([128, 512], fp32, tag="mm", name="psmm_t")

            def PS_AT(shape):  # long-lived accumulators / phase-2 partial
                return psatp.tile(shape, fp32, tag="at", name="psat_t")

            def PS_ATW():  # MHA attn+denom accumulator [128, 132]
                return psatp.tile([128, 33 * H], fp32, tag="at", name="psatw_t")

            def PS_PT2():  # [32, 512] head q/k psum
                return psptp.tile([32, 512], fp32, tag="pt", name="pspt2_t")

            def PS_PT():   # [128,128] transposes / small matmuls
                return psptp.tile([128, 128], fp32, tag="pt", name="pspt_t")

            def PS_AGG():  # [128,128] cheb aggregation
                return psaggp.tile([128, 128], fp32, tag="agg", name="psagg_t")

            nc.gpsimd.load_library(library_config.mlp)

            # ---- load constants ----
            def ld(ap_in, shape, dt, name, eng=None):
                t = cp.tile(shape, dt, tag=name, name=name)
                (eng or nc.sync).dma_start(t[:], ap_in[:])
                return t

            xT_s = bp.tile([128, NLOC], bf16, tag="xT")
            nc.sync.dma_start(xT_s[:], xT[:])
            s8_s = ld(s8, [128, ngroups * Bd], fp8, "s8")
            wspa1_s = ld(wspa1, [128, 128], bf16, "wspa1")
            wspa2_s = ld(wspa2, [128, 128], bf16, "wspa2")
            wspe1_s = ld(wspe1, [128, 128], bf16, "wspe1")
            wspe2_s = ld(wspe2, [128, 128], bf16, "wspe2")
            wproj_s = ld(wproj, [128, 128], bf16, "wproj")
            chebw_s = ld(chebw, [128, K * 128], bf16, "chebw")
            wq_s = ld(wq, [128, 128], bf16, "wq")
            wk_s = ld(wk, [128, 128], bf16, "wk")
            wv_s = ld(wv, [128, 128], bf16, "wv")
            wout_s = ld(wout, [128, 128], bf16, "wout")
            mw1_s = ld(mw1, [128, 256], bf16, "mw1")
            mw2a_s = ld(mw2a, [128, 128], bf16, "mw2a")
            mw2b_s = ld(mw2b, [128, 128], bf16, "mw2b")
            biasp_s = ld(biasp, [128, NBIAS], fp32, "biasp")
            bqh_s = ld(bqh, [32, H], fp32, "bqh")
            bkh_s = ld(bkh, [32, H], fp32, "bkh")
            idm_s = ld(idm, [128, 128], fp32, "idm")
            idmb_s = ld(idmb, [128, 128], bf16, "idmb")
            vbuf = bp.tile([128, NT, 128], bf16, tag="vbuf")

            def th_accum(t_ps):
                # T_h = v~^T (Us/dis): stream usd in 8-tile chunks (window
                # work, off the gather critical path; scalar-queue DMA)
                for t0 in range(0, NT, 8):
                    ub8 = wp.tile([128, 8, 128], bf16, tag="ut8", bufs=1)
                    nc.scalar.dma_start(
                        ub8[:], usd[t0 * 128:(t0 + 8) * 128, :]
                        .rearrange("(t p) c -> p t c", p=128))
                    for j in range(8):
                        t = t0 + j
                        nc.tensor.matmul(t_ps[:], lhsT=vbuf[:, t, :],
                                         rhs=ub8[:, j, :],
                                         start=(t == 0), stop=(t == NT - 1))
            gmat_s = ld(gmat, [128, 128], fp32, "gmat")
            u1col_s = ld(u1col, [128, 1], fp32, "u1col")
            onesf_s = ld(onesf, [128, 1], fp32, "onesf")
            one1_s = ld(one1, [1, 1], fp32, "one1")
            m1x_s = ld(m1x, [128, 128], fp32, "m1x")
            m1sb = cp.tile([128, 128], fp32, tag="m1sb", name="m1sb")
            discf_s = ld(discf, [128, N // 128], fp32, "discf")
            disc_s = ld(disc, [128, NT], fp32, "disc")
            discm_s = ld(disc_m, [128, 2 * NT], fp32, "discm")
            discm2_s = discm_s[:, NT:2 * NT]

            def bcol(i):
                return biasp_s[:, i:i + 1]

            # persistent big buffers
            TxA = bp.tile([128, NT, 128], fp32, tag="TxA")   # node-major
            TxB = bp.tile([128, NT, 128], fp32, tag="TxB")
            chebT = bp.tile([128, NLOC], fp32, tag="chebT")  # later: pre1, outT
            h2T = bp.tile([128, NLOC], fp32, tag="h2T")      # later: pre2, out2T
            if not do_mha:
                nc.vector.memset(h2T[:], 0.0)
            statc = cp.tile([128, 32], fp32, tag="statc")    # stats/affine cols
            statc2 = cp.tile([128, 16], fp32, tag="statc2")  # per-graph h2 stats
            outnm = TxA  # reuse (dead after cheb)

            # DRAM bounce buffers
            vdt = fp8 if FP8V else bf16
            ag_in = dp.tile([NLOC, 128], vdt, tag="ag_in")
            ag_outs = [None] + [dp.tile([N, 128], vdt, tag=f"ag_out{h}", name=f"ag_out{h}", addr_space="Shared") for h in range(1, 4)]
            vfull0 = dp.tile([N, 128], vdt, tag="vfull0", name="vfull0")
            bn12_in = dp.tile([128, 260], fp32, tag="bn12_in")
            bn12_out = dp.tile([128, 260], fp32, tag="bn12_out", addr_space="Shared")
            bn3_in = dp.tile([128, 2], fp32, tag="bn3_in")
            bn3_out = dp.tile([128, 2], fp32, tag="bn3_out", addr_space="Shared")

            CH = NLOC // 512  # 8 chunks of 512

            from contextlib import ExitStack
            ep_stack = ExitStack()
            ep = ep_stack.enter_context(tc.tile_pool(name="early", bufs=1))
            xspT = ep.tile([128, NLOC], bf16, tag="xspT", name="xspT")

            # ================= Phase 1: local spa MLP (feature-major) =======
            # (the spe MLP + spectral partial run inside AllGather window 1,
            # recomputed from the resident xT_s, off the pre-hop critical path)
            for ch in range(CH):
                sl = slice(ch * 512, (ch + 1) * 512)
                p1 = PS_MM()
                nc.tensor.matmul(p1[:], lhsT=wspa1_s[:], rhs=xT_s[:, sl],
                                 start=True, stop=True)
                t1 = wp.tile([128, 512], bf16, tag="t1b")
                nc.scalar.activation(t1[:], p1[:], AF.Relu, bias=bcol(BI_SPA1))
                p2 = PS_MM()
                nc.tensor.matmul(p2[:], lhsT=wspa2_s[:], rhs=t1[:],
                                 start=True, stop=True)
                nc.scalar.activation(xspT[:, sl], p2[:], AF.Identity,
                                     bias=bcol(BI_SPA2))

            # Tx0 node-major (local shard, for recurrence) + v~0 into vbuf
            for t in range(NT):
                tsl = slice(t * 128, (t + 1) * 128)
                pt = PS_PT()
                nc.tensor.matmul(pt[:], lhsT=xspT[:, tsl], rhs=idmb_s[:],
                                 start=True, stop=True)
                nc.vector.tensor_copy(TxB[:, t, :], pt[:])
                nc.scalar.activation(vbuf[:, t, :], pt[:], AF.Identity,
                                     scale=disc_s[:, t:t + 1])
            # v~0 for ALL nodes computed locally (replaces hop-1 AllGather):
            # every core redundantly runs the spatial MLP over the full x.
            if hops >= 1:
                for gch in range(N // 512):
                    gsl2 = slice(gch * 512, (gch + 1) * 512)
                    xc = wp.tile([128, 512], bf16, tag="t1b")
                    nc.sync.dma_start(xc[:], xfT[:, gsl2])
                    pf1 = PS_MM()
                    nc.tensor.matmul(pf1[:], lhsT=wspa1_s[:], rhs=xc[:],
                                     start=True, stop=True)
                    tf1 = wp.tile([128, 512], bf16, tag="midab")
                    nc.scalar.activation(tf1[:], pf1[:], AF.Relu,
                                         bias=bcol(BI_SPA1))
                    pf2 = PS_MM()
                    nc.tensor.matmul(pf2[:], lhsT=wspa2_s[:], rhs=tf1[:],
                                     start=True, stop=True)
                    spf = wp.tile([128, 512], bf16, tag="t1b")
                    nc.scalar.activation(spf[:], pf2[:], AF.Identity,
                                         bias=bcol(BI_SPA2))
                    vt4 = wp.tile([128, 4, 128], vdt, tag="vt4", bufs=2)
                    for j in range(4):
                        tg = gch * 4 + j
                        ptf = PS_PT()
                        nc.tensor.matmul(ptf[:], lhsT=spf[:, j * 128:(j + 1) * 128],
                                         rhs=idmb_s[:], start=True, stop=True)
                        nc.scalar.activation(vt4[:, j, :], ptf[:], AF.Identity,
                                             scale=discf_s[:, tg:tg + 1])
                    nc.sync.dma_start(
                        vfull0[gch * 512:(gch + 1) * 512, :]
                        .rearrange("(t p) c -> p t c", p=128), vt4[:])

            for ch in range(CH):
                sl = slice(ch * 512, (ch + 1) * 512)
                pw = PS_MM()
                nc.tensor.matmul(pw[:], lhsT=chebw_s[:, 0:128], rhs=xspT[:, sl],
                                 start=True, stop=True)
                nc.vector.tensor_copy(chebT[:, sl], pw[:])
            # T_0 = v~0^T (Us/dis) (accumulate over tiles); m1sb = m1x + W0^T T_0
            t_ps = PS_AGG()
            th_accum(t_ps)
            tsb = wp.tile([128, 128], bf16, tag="tsb")
            nc.vector.tensor_copy(tsb[:], t_ps[:])
            pWt = PS_PT()
            nc.tensor.matmul(pWt[:], lhsT=chebw_s[:, 0:128], rhs=tsb[:],
                             start=True, stop=True)
            nc.vector.tensor_add(m1sb[:], m1x_s[:], pWt[:])

            # ===== Phase 2 (deferred): spectral partial, run in AG window 1 =
            # Recomputes the spe MLP chunk-wise from resident xT_s so xspecT
            # needs no SBUF residency across the hops.
            part_s = wp.tile([128, 128], fp32, tag="part_s", bufs=1)

            def spectral_partial():
                part_ps = PS_AT([128, 128])
                for ch2 in range(CH):
                    sl2 = slice(ch2 * 512, (ch2 + 1) * 512)
                    p3 = PS_MM()
                    nc.tensor.matmul(p3[:], lhsT=wspe1_s[:], rhs=xT_s[:, sl2],
                                     start=True, stop=True)
                    t2 = wp.tile([128, 512], bf16, tag="t1b")
                    nc.scalar.activation(t2[:], p3[:], AF.Relu,
                                         bias=bcol(BI_SPE1))
                    p4 = PS_MM()
                    nc.tensor.matmul(p4[:], lhsT=wspe2_s[:], rhs=t2[:],
                                     start=True, stop=True)
                    xsp2 = wp.tile([128, 512], bf16, tag="t1b")
                    nc.scalar.activation(xsp2[:], p4[:], AF.Identity,
                                         bias=bcol(BI_SPE2))
                    ub = wp.tile([128, 4, 128], bf16, tag="ut4", bufs=1)
                    nc.sync.dma_start(
                        ub[:], u_nm[ch2 * 512:(ch2 + 1) * 512, :]
                        .rearrange("(t p) c -> p t c", p=128))
                    for j in range(4):
                        t = ch2 * 4 + j
                        ph = PS_PT()
                        nc.tensor.matmul(ph[:], lhsT=xsp2[:, j * 128:(j + 1) * 128],
                                         rhs=wproj_s[:], start=True, stop=True)
                        hp = wp.tile([128, 128], bf16, tag="hp")
                        nc.vector.tensor_copy(hp[:], ph[:])
                        nc.tensor.matmul(part_ps[:], lhsT=ub[:, j, :], rhs=hp[:],
                                         start=(t == 0), stop=(t == NT - 1))
                nc.vector.tensor_copy(part_s[:], part_ps[:])

            ep_stack.close()  # free xspT space for later pools
            late_stack = ExitStack()
            gp = late_stack.enter_context(tc.tile_pool(name="gath", bufs=2))
            mp = late_stack.enter_context(tc.tile_pool(name="mha", bufs=2))

            # ---- MHA for one graph (interleaved into AllGather windows) ----
            def mha_graph(g):
                gsl = slice(g * 512, (g + 1) * 512)
                # head-major q/k: per-head matmuls so all operands are base-0
                qT = mp.tile([32, H * 512], bf16, tag="qT", bufs=1)
                kT = mp.tile([32, H * 512], bf16, tag="kT", bufs=1)
                for hh in range(H):
                    csl = slice(hh * 32, (hh + 1) * 32)
                    pqh = PS_PT2()
                    nc.tensor.matmul(pqh[:], lhsT=wq_s[:, csl],
                                     rhs=xT_s[:, gsl], start=True, stop=True)
                    nc.scalar.activation(qT[:, hh * 512:(hh + 1) * 512], pqh[:],
                                         AF.Identity, bias=bqh_s[:, hh:hh + 1])
                    pkh = PS_PT2()
                    nc.tensor.matmul(pkh[:], lhsT=wk_s[:, csl],
                                     rhs=xT_s[:, gsl], start=True, stop=True)
                    nc.scalar.activation(kT[:, hh * 512:(hh + 1) * 512], pkh[:],
                                         AF.Identity, bias=bkh_s[:, hh:hh + 1])
                # v node-major, augmented per head with a ones column
                vaug = mp.tile([128, 4, 33 * H], bf16, tag="vaug")
                nc.vector.memset(vaug[:, :, 32::33], 1.0)
                for j in range(4):
                    pv = PS_PT()
                    nc.tensor.matmul(pv[:], lhsT=xT_s[:, g * 512 + j * 128:
                                                      g * 512 + (j + 1) * 128],
                                     rhs=wv_s[:], start=True, stop=True)
                    for hh in range(H):
                        nc.vector.tensor_copy(
                            vaug[:, j, hh * 33:hh * 33 + 32],
                            pv[:, hh * 32:(hh + 1) * 32])
                # scores_T + exp, per (head, k-chunk)
                ess = {}
                for hh in range(H):
                    qsl = slice(hh * 512, (hh + 1) * 512)
                    for j in range(4):
                        pss = PS_MM()
                        nc.tensor.matmul(
                            pss[:], lhsT=kT[:, hh * 512 + j * 128:
                                            hh * 512 + (j + 1) * 128],
                            rhs=qT[:, qsl], start=True, stop=True)
                        es = mp.tile([128, 512], bf16, tag="es", bufs=16)
                        nc.scalar.activation(es[:], pss[:], AF.Exp)
                        ess[(hh, j)] = es
                # attn + denom per q-chunk
                for qc in range(4):
                    pat = PS_ATW()
                    for hh in range(H):
                        for j in range(4):
                            nc.tensor.matmul(
                                pat[:, hh * 33:(hh + 1) * 33],
                                lhsT=ess[(hh, j)][:, qc * 128:(qc + 1) * 128],
                                rhs=vaug[:, j, hh * 33:(hh + 1) * 33],
                                start=(j == 0), stop=(j == 3),
                                skip_group_check=True)
                    recip = wp.tile([128, 4], fp32, tag="recip")
                    nc.vector.reciprocal(recip[:], pat[:, 32::33])
                    anm = wp.tile([128, 128], fp32, tag="anm")
                    for hh in range(H):
                        nc.vector.tensor_scalar(
                            out=anm[:, hh * 32:(hh + 1) * 32],
                            in0=pat[:, hh * 33:hh * 33 + 32],
                            scalar1=recip[:, hh:hh + 1], scalar2=None,
                            op0=OP.mult)
                    ptr = PS_PT()
                    nc.tensor.transpose(ptr[:], anm[:], idm_s[:])
                    attnT = wp.tile([128, 128], bf16, tag="attnT", bufs=1)
                    nc.vector.tensor_copy(attnT[:], ptr[:])
                    ph2 = PS_PT()
                    nc.tensor.matmul(ph2[:], lhsT=wout_s[:], rhs=attnT[:],
                                     start=True, stop=True)
                    osl = slice(g * 512 + qc * 128, g * 512 + (qc + 1) * 128)
                    # pre2 = h2 + b_out' + x
                    nc.vector.scalar_tensor_tensor(
                        out=h2T[:, osl], in0=ph2[:], scalar=bcol(BI_OUTP),
                        in1=xT_s[:, osl], op0=OP.add, op1=OP.add)
                # incremental BN2 stats for this graph's 512 columns
                nc.vector.tensor_reduce(statc2[:, g:g + 1], h2T[:, gsl],
                                        mybir.AxisListType.X, OP.add)
                tsq = wp.tile([128, 512], fp32, tag="sqt", bufs=1)
                nc.vector.scalar_tensor_tensor(
                    out=tsq[:], in0=h2T[:, gsl], scalar=1.0, in1=h2T[:, gsl],
                    op0=OP.mult, op1=OP.mult,
                    accum_out=statc2[:, 8 + g:9 + g])

            # graphs run inside AllGather wait windows (PE idle otherwise)
            mha_sched = {1: [0, 1, 2], 2: [3, 4, 5], 3: [6, 7]} \
                if (do_mha and hops == K - 1) else {}
            mha_left = [g for g in range(GPC if do_mha else 0)
                        if not any(g in v for v in mha_sched.values())]

            # ================= Phase 3: cheb hops ===========================
            # tile_wait_until stamps are scheduler-only hints (virtual
            # earliest-start): they stop the list scheduler from hoisting
            # hop h+1's recurrence ops ahead of the window-h MHA work in
            # the in-order DVE queue (head-of-line blocking during the
            # AllGather). They emit no HW waits.
            cur, prev = TxB, TxA  # cur holds Tx_{h-1}; prev gets Tx_h
            for h in range(1, 1 + hops):
                t_hop = ST0 + STH * (h - 1)
                ag_src = vfull0 if h == 1 else ag_outs[h - 1]
                hop_stack = ExitStack()
                hop_stack.enter_context(tc.tile_wait_until(t_hop))
                for q in range(NCALL):
                    gt = gp.tile([128, BPG * ngrp, 128], vdt, tag="gt", bufs=12)
                    isl = slice(q * BPG * epb // 16, (q + 1) * BPG * epb // 16)
                    gix = wp.tile([128, BPG * epb // 16], i16, tag="gix", bufs=12)
                    nc.sync.dma_start(gix[:], gidx[:, isl])
                    nc.gpsimd.dma_gather(gt[:], ag_src[:], gix[:],
                                         BPG * epb, BPG * epb, 128,
                                         single_packet=False,
                                         queue_num=q % 4)
                    for r in range(BPG):
                        b = q * BPG + r
                        t, half = b // 2, b % 2
                        if half == 0:
                            aps = PS_AGG()
                        for j in range(ngrp):
                            gcol = b * ngrp + j
                            nc.tensor.matmul(
                                aps[half * 64:(half + 1) * 64, :],
                                lhsT=s8_s[:, gcol * Bd:(gcol + 1) * Bd],
                                rhs=gt[:, r * ngrp + j, :],
                                start=(j == 0), stop=(j == ngrp - 1))
                        if half == 1:
                            # recurrence for tile t
                            tmp = wp.tile([128, 128], fp32, tag="rectmp")
                            if h == 1:
                                nc.vector.tensor_scalar(
                                    out=tmp[:], in0=aps[:],
                                    scalar1=discm_s[:, t:t + 1], scalar2=None,
                                    op0=OP.mult)
                                # Tx1 = (scale-1)*Tx0 + tmp
                                nc.vector.scalar_tensor_tensor(
                                    out=prev[:, t, :], in0=cur[:, t, :],
                                    scalar=float(scale - 1.0), in1=tmp[:],
                                    op0=OP.mult, op1=OP.add)
                            else:
                                nc.vector.tensor_scalar(
                                    out=tmp[:], in0=aps[:],
                                    scalar1=discm_s[:, t:t + 1], scalar2=2.0,
                                    op0=OP.mult, op1=OP.mult)
                                # tmp2 = tmp - Tx_{h-2}
                                tmp2 = wp.tile([128, 128], fp32, tag="rectmp2")
                                nc.vector.scalar_tensor_tensor(
                                    out=tmp2[:], in0=prev[:, t, :],
                                    scalar=-1.0, in1=tmp[:],
                                    op0=OP.mult, op1=OP.add)
                                # Tx_h = 2(scale-1)*Tx_{h-1} + tmp2
                                nc.vector.scalar_tensor_tensor(
                                    out=prev[:, t, :], in0=cur[:, t, :],
                                    scalar=float(2.0 * (scale - 1.0)),
                                    in1=tmp2[:], op0=OP.mult, op1=OP.add)
                            nc.scalar.activation(
                                vbuf[:, t, :], prev[:, t, :], AF.Identity,
                                scale=disc_s[:, t:t + 1])
                            if h < hops:
                                nc.sync.dma_start(
                                    ag_in[t * 128:(t + 1) * 128, :],
                                    vbuf[:, t, :])
                # launch AG for next hop once all v~ tiles written
                if h < hops:
                    nc.gpsimd.collective_compute(
                        "AllGather", OP.bypass,
                        replica_groups=[list(range(NCORES))],
                        ins=[ag_in.opt()], outs=[ag_outs[h].opt()])
                hop_stack.close()
                win_stack = ExitStack()
                win_stack.enter_context(tc.tile_wait_until(t_hop + STW))
                # out_cheb += Tx_h @ W_h  (transpose tiles chunk-wise)
                for ch in range(CH):
                    tpb = wp.tile([128, 512], bf16, tag="tpb", bufs=1)
                    for j in range(4):
                        t = ch * 4 + j
                        pt = PS_PT()
                        nc.tensor.transpose(pt[:], prev[:, t, :], idm_s[:])
                        nc.vector.tensor_copy(tpb[:, j * 128:(j + 1) * 128], pt[:])
                    sl = slice(ch * 512, (ch + 1) * 512)
                    pw = PS_MM()
                    nc.tensor.matmul(pw[:], lhsT=chebw_s[:, h * 128:(h + 1) * 128],
                                     rhs=tpb[:], start=True, stop=True)
                    nc.vector.tensor_add(chebT[:, sl], chebT[:, sl], pw[:])
                # T_h = v~_h^T (Us/dis) ; m1sb += W_h^T T_h  (off the tail)
                t_ps = PS_AGG()
                th_accum(t_ps)
                tsb = wp.tile([128, 128], bf16, tag="tsb")
                nc.vector.tensor_copy(tsb[:], t_ps[:])
                pWt = PS_PT()
                nc.tensor.matmul(pWt[:], lhsT=chebw_s[:, h * 128:(h + 1) * 128],
                                 rhs=tsb[:], start=True, stop=True)
                nc.vector.tensor_add(m1sb[:], m1sb[:], pWt[:])
                if h == 1:
                    spectral_partial()
                for g in mha_sched.get(h, []):
                    mha_graph(g)
                win_stack.close()
                cur, prev = prev, cur

            # ================= Phase 4: MHA (remaining graphs) ==============
            for g in mha_left:
                mha_graph(g)

            tail_stack = ExitStack()
            tail_stack.enter_context(tc.tile_wait_until(ST0 + STH * hops))
            # ===== Phase 5: pre1' (no spec) + BN stats + M1 + joint AR ======
            for ch in range(CH):
                sl = slice(ch * 512, (ch + 1) * 512)
                # pre1' = chebT + cheb_b + x   (overwrite chebT)
                nc.vector.scalar_tensor_tensor(
                    out=chebT[:, sl], in0=chebT[:, sl], scalar=bcol(BI_CHEB),
                    in1=xT_s[:, sl], op0=OP.add, op1=OP.add)
            # BN1/BN2 stats
            def sumsq(buf, out_col):
                for c2 in range(CH):
                    s2 = slice(c2 * 512, (c2 + 1) * 512)
                    tt = wp.tile([128, 512], fp32, tag="sqt", bufs=1)
                    nc.vector.scalar_tensor_tensor(
                        out=tt[:], in0=buf[:, s2], scalar=1.0, in1=buf[:, s2],
                        op0=OP.mult, op1=OP.mult,
                        accum_out=statc[:, 24 + c2:25 + c2])
                nc.vector.tensor_reduce(out_col, statc[:, 24:32],
                                        mybir.AxisListType.X, OP.add)

            nc.vector.tensor_reduce(statc[:, 0:1], chebT[:], mybir.AxisListType.X, OP.add)
            sumsq(chebT, statc[:, 1:2])
            if do_mha:
                nc.vector.tensor_reduce(statc[:, 2:3], statc2[:, 0:8],
                                        mybir.AxisListType.X, OP.add)
                nc.vector.tensor_reduce(statc[:, 3:4], statc2[:, 8:16],
                                        mybir.AxisListType.X, OP.add)
            else:
                nc.vector.tensor_reduce(statc[:, 2:3], h2T[:], mybir.AxisListType.X, OP.add)
                sumsq(h2T, statc[:, 3:4])
            st12 = wp.tile([128, 260], fp32, tag="st12", bufs=1)
            nc.vector.tensor_copy(st12[:, 0:4], statc[:, 0:4])
            nc.vector.tensor_copy(st12[:, 4:132], m1sb[:])
            nc.vector.tensor_copy(st12[:, 132:260], part_s[:])
            nc.sync.dma_start(bn12_in[:], st12[:])
            nc.gpsimd.collective_compute(
                "AllReduce", OP.add, replica_groups=[list(range(NCORES))],
                ins=[bn12_in.opt()], outs=[bn12_out.opt()])
            sr12 = wp.tile([128, 260], fp32, tag="sr12", bufs=1)
            nc.sync.dma_start(sr12[:], bn12_out[:])
            Pm = sr12[:, 132:260]   # AR'd spectral partial [keig, C]
            M1g = sr12[:, 4:132]    # [C, keig]
            # spectral stat terms:
            # s_sum[c] = sum_k u1[k] P[k,c];  s_sq[c] = sum_k P[k,c](G P)[k,c]
            # cross[c] = sum_k M1g[c,k] P[k,c]
            w12 = wp.tile([128, 256], fp32, tag="w12", bufs=1)
            nc.vector.tensor_scalar(out=w12[:, 0:128], in0=Pm,
                                    scalar1=u1col_s[:], scalar2=None,
                                    op0=OP.mult)
            t1_ps = PS_PT()
            nc.tensor.matmul(t1_ps[:], lhsT=gmat_s[:], rhs=Pm,
                             start=True, stop=True)
            nc.vector.tensor_tensor(w12[:, 128:256], t1_ps[:], Pm, OP.mult)
            # column sums: ones-matmul -> [1,256] row, then row -> two cols
            r_ps = PS_MM()
            nc.tensor.matmul(r_ps[0:1, 0:256], lhsT=onesf_s[:], rhs=w12[:],
                             start=True, stop=True)
            rowbuf = wp.tile([1, 256], fp32, tag="rowbuf")
            nc.vector.tensor_copy(rowbuf[:], r_ps[0:1, 0:256])
            c_ps = PS_PT()
            nc.tensor.matmul(c_ps[:, 0:1], lhsT=rowbuf[:, 0:128], rhs=one1_s[:],
                             start=True, stop=True, skip_group_check=True)
            nc.tensor.matmul(c_ps[:, 1:2], lhsT=rowbuf[:, 128:256], rhs=one1_s[:],
                             start=True, stop=True, skip_group_check=True)
            s_cols = wp.tile([128, 2], fp32, tag="s_cols")
            nc.vector.tensor_copy(s_cols[:], c_ps[:, 0:2])
            # cross: transpose P, multiply with M1g, reduce
            pt_ps = PS_PT()
            nc.tensor.transpose(pt_ps[:], sr12[:, 132:260], idm_s[:])
            ptm = wp.tile([128, 128], fp32, tag="w1t")
            nc.vector.tensor_tensor(ptm[:], pt_ps[:], M1g, OP.mult)
            crossc = statc[:, 14:15]
            nc.vector.tensor_reduce(crossc, ptm[:], mybir.AxisListType.X, OP.add)
            # BN1 totals: sum1 = sr12[:,0] + s_sum ; sq1 = sr12[:,1] + 2*cross + s_sq
            sum1c = statc[:, 15:16]
            nc.vector.tensor_tensor(sum1c, sr12[:, 0:1], s_cols[:, 0:1], OP.add)
            sq1c = statc[:, 16:17]
            nc.vector.scalar_tensor_tensor(out=sq1c, in0=crossc, scalar=2.0,
                                           in1=sr12[:, 1:2], op0=OP.mult,
                                           op1=OP.add)
            nc.vector.tensor_tensor(sq1c, sq1c, s_cols[:, 1:2], OP.add)

            # affine coefs: A = w/sqrt(var+eps), Bc = b - mu*A
            def bn_affine(sum_col, sq_col, w_col, b_col, a_out, b_out_col):
                mu = statc[:, 8:9]
                nc.vector.tensor_scalar(out=mu, in0=sum_col, scalar1=1.0 / N,
                                        scalar2=None, op0=OP.mult)
                msq = statc[:, 9:10]
                nc.vector.tensor_scalar(out=msq, in0=sq_col, scalar1=1.0 / N,
                                        scalar2=None, op0=OP.mult)
                nvar = statc[:, 10:11]
                nc.vector.scalar_tensor_tensor(out=nvar, in0=mu, scalar=mu,
                                               in1=msq, op0=OP.mult,
                                               op1=OP.subtract)  # mu^2 - msq
                sd = statc[:, 11:12]
                nc.scalar.activation(sd, nvar, AF.Sqrt, bias=bcol(BI_EPSC),
                                     scale=-1.0)
                rsd = statc[:, 12:13]
                nc.vector.reciprocal(rsd, sd)
                nc.vector.tensor_tensor(a_out, rsd, w_col, OP.mult)
                nbc = statc[:, 13:14]
                nc.vector.scalar_tensor_tensor(out=nbc, in0=mu, scalar=a_out,
                                               in1=b_col, op0=OP.mult,
                                               op1=OP.subtract)  # mu*A - b
                nc.vector.tensor_scalar(out=b_out_col, in0=nbc, scalar1=-1.0,
                                        scalar2=None, op0=OP.mult)

            A1, B1 = statc[:, 4:5], statc[:, 5:6]
            A2, B2 = statc[:, 6:7], statc[:, 7:8]
            bn_affine(statc[:, 15:16], statc[:, 16:17], bcol(BI_BN1W), bcol(BI_BN1B), A1, B1)
            bn_affine(sr12[:, 2:3], sr12[:, 3:4], bcol(BI_BN2W), bcol(BI_BN2B), A2, B2)
            B12 = statc[:, 5:6]
            nc.vector.tensor_tensor(B12, B1, B2, OP.add)  # B1 += B2 (in place)

            # ============ Phase 6: out = h1 + h2n; MLP2; BN3 ================
            out2T = h2T   # overwrite pre2 per chunk
            Pmb = wp.tile([128, 128], bf16, tag="pmb16", bufs=1)
            nc.vector.tensor_copy(Pmb[:], sr12[:, 132:260])
            for ch in range(CH):
                sl = slice(ch * 512, (ch + 1) * 512)
                ust = wp.tile([128, 512], bf16, tag="ust")
                nc.scalar.dma_start(ust[:], usT[:, sl])
                pso = PS_MM()
                nc.tensor.matmul(pso[:], lhsT=Pmb[:], rhs=ust[:],
                                 start=True, stop=True)
                t1 = wp.tile([128, 512], fp32, tag="t1")
                nc.scalar.activation(t1[:], chebT[:, sl], AF.Identity,
                                     bias=B12, scale=A1)
                tsp = wp.tile([128, 512], fp32, tag="t1")
                nc.vector.scalar_tensor_tensor(
                    out=tsp[:], in0=pso[:], scalar=A1, in1=t1[:],
                    op0=OP.mult, op1=OP.add)
                outT = wp.tile([128, 512], bf16, tag="outTb", bufs=2)
                nc.vector.scalar_tensor_tensor(
                    out=outT[:], in0=h2T[:, sl], scalar=A2, in1=tsp[:],
                    op0=OP.mult, op1=OP.add)
                pma = PS_MM()
                nc.tensor.matmul(pma[:], lhsT=mw1_s[:, 0:128], rhs=outT[:],
                                 start=True, stop=True)
                mida = wp.tile([128, 512], bf16, tag="midab")
                nc.scalar.activation(mida[:], pma[:], AF.Relu, bias=bcol(BI_M1A))
                pmb = PS_MM()
                nc.tensor.matmul(pmb[:], lhsT=mw1_s[:, 128:256], rhs=outT[:],
                                 start=True, stop=True)
                midb = wp.tile([128, 512], bf16, tag="midab")
                nc.scalar.activation(midb[:], pmb[:], AF.Relu, bias=bcol(BI_M1B))
                pmo = PS_MM()
                nc.tensor.matmul(pmo[:], lhsT=mw2a_s[:], rhs=mida[:],
                                 start=True, stop=False)
                nc.tensor.matmul(pmo[:], lhsT=mw2b_s[:], rhs=midb[:],
                                 start=False, stop=True)
                # out2 = out + mlp_b2 + psum
                nc.vector.scalar_tensor_tensor(
                    out=out2T[:, sl], in0=outT[:], scalar=bcol(BI_M2),
                    in1=pmo[:], op0=OP.add, op1=OP.add)
            # BN3 stats
            nc.vector.tensor_reduce(statc[:, 0:1], out2T[:], mybir.AxisListType.X, OP.add)
            sumsq(out2T, statc[:, 1:2])
            st3 = wp.tile([128, 2], fp32, tag="st")
            nc.vector.tensor_copy(st3[:], statc[:, 0:2])
            nc.sync.dma_start(bn3_in[:], st3[:])
            nc.gpsimd.collective_compute(
                "AllReduce", OP.add, replica_groups=[list(range(NCORES))],
                ins=[bn3_in.opt()], outs=[bn3_out.opt()])
            sr3 = wp.tile([128, 2], fp32, tag="st")
            nc.sync.dma_start(sr3[:], bn3_out[:])
            A3, B3 = statc[:, 4:5], statc[:, 5:6]
            bn_affine(sr3[:, 0:1], sr3[:, 1:2], bcol(BI_BN3W), bcol(BI_BN3B), A3, B3)

            # apply BN3, transpose to node-major, write out
            for ch in range(CH):
                sl = slice(ch * 512, (ch + 1) * 512)
                bn3b = wp.tile([128, 512], fp32, tag="t1")
                nc.scalar.activation(bn3b[:], out2T[:, sl], AF.Identity,
                                     bias=B3, scale=A3)
                for j in range(4):
                    t = ch * 4 + j
                    pt = PS_PT()
                    nc.tensor.transpose(pt[:], bn3b[:, j * 128:(j + 1) * 128],
                                        idm_s[:])
                    nc.vector.tensor_copy(outnm[:, t, :], pt[:])
            nc.sync.dma_start(
                out_nm[:].rearrange("(t p) c -> p t c", p=128), outnm[:])
            tail_stack.close()
            late_stack.close()

    nc.compile()
    return nc


def kernel(**inputs):
    inp = {k: np.asarray(v) for k, v in inputs.items()}
    cores, dis, scale, ngrp = _preprocess(inp)

    key = (ngrp, float(scale))
    if key not in _CACHE:
        _CACHE[key] = _build(ngrp, scale)
    nc = _CACHE[key]

    x = inp["x"].astype(F32)
    U = inp["U"].astype(F32)
    s_lam = np.exp(-float(inp["gamma"].reshape(-1)[0]) *
                   inp["Lambda"].astype(np.float64) ** 2).astype(F32)

    wqkv = inp["w_qkv"].astype(F32)
    bqkv = inp["b_qkv"].astype(F32)
    wq = (wqkv[:, :C] / np.sqrt(DH)).astype(F32)
    bq = (bqkv[:C] / np.sqrt(DH)).astype(F32)
    wk, bk = wqkv[:, C:2 * C].copy(), bqkv[C:2 * C]
    wv, bv = wqkv[:, 2 * C:].copy(), bqkv[2 * C:]
    b_out_p = (bv @ inp["w_out"] + inp["b_out"]).astype(F32)

    biasp = np.zeros((128, NBIAS), F32)
    for i, vec in [(BI_SPA1, inp["b_spa1"]), (BI_SPA2, inp["b_spa2"]),
                   (BI_SPE1, inp["b_spe1"]), (BI_SPE2, inp["b_spe2"]),
                   (BI_Q, bq), (BI_K, bk), (BI_OUTP, b_out_p),
                   (BI_CHEB, inp["cheb_b"]),
                   (BI_M1A, inp["mlp_b1"][:128]), (BI_M1B, inp["mlp_b1"][128:]),
                   (BI_M2, inp["mlp_b2"]),
                   (BI_BN1W, inp["bn1_w"]), (BI_BN1B, inp["bn1_b"]),
                   (BI_BN2W, inp["bn2_w"]), (BI_BN2B, inp["bn2_b"]),
                   (BI_BN3W, inp["bn3_w"]), (BI_BN3B, inp["bn3_b"]),
                   (BI_EPSC, np.full(128, EPS, F32))]:
        biasp[:, i] = vec.astype(F32)

    chebw_cols = np.concatenate([inp["cheb_w"][k].astype(F32) for k in range(K)],
                                axis=1)  # [128, 5*128]

    common = dict(
        wspa1=inp["w_spa1"].astype(BF16), wspa2=inp["w_spa2"].astype(BF16),
        wspe1=inp["w_spe1"].astype(BF16), wspe2=inp["w_spe2"].astype(BF16),
        wproj=inp["w_proj"].astype(BF16), chebw=chebw_cols.astype(BF16),
        wq=wq.astype(BF16), wk=wk.astype(BF16), wv=wv.astype(BF16),
        wout=inp["w_out"].astype(BF16),
        mw1=inp["mlp_w1"].astype(BF16),
        mw2a=inp["mlp_w2"][:128].astype(BF16), mw2b=inp["mlp_w2"][128:].astype(BF16),
        biasp=biasp,
        bqh=np.ascontiguousarray(bq.reshape(H, DH).T),
        bkh=np.ascontiguousarray(bk.astype(F32).reshape(H, DH).T),
        idm=np.eye(128, dtype=F32), idmb=np.eye(128, dtype=BF16),
        gmat=None, u1col=None, onesf=np.ones((128, 1), F32),
        one1=np.ones((1, 1), F32),
    )

    vs = VSCALE if FP8V else 1.0
    Us_full = (U * s_lam[None, :]).astype(F32)
    dis_cl = np.where(dis > 0, dis, 1.0).astype(F32)
    xfT_np = np.ascontiguousarray(x.T).astype(BF16)
    discf_np = np.ascontiguousarray((vs * dis_cl).reshape(N // 128, 128).T)
    gmat_np = (Us_full.T @ Us_full).astype(F32)
    u1_np = np.ascontiguousarray(Us_full.sum(0).astype(F32)[:, None])
    in_maps = []
    for c in range(NCORES):
        sl = slice(c * NLOC, (c + 1) * NLOC)
        dis_c = dis[sl]
        m = dict(common)
        m["xT"] = np.ascontiguousarray(x[sl].T).astype(BF16)
        m["u_nm"] = np.ascontiguousarray(U[sl]).astype(BF16)
        m["usT"] = np.ascontiguousarray((U[sl] * s_lam[None, :]).T).astype(BF16)
        m["usd"] = np.ascontiguousarray(
            Us_full[sl] / dis_cl[sl][:, None]).astype(BF16)
        m["gmat"] = gmat_np
        m["u1col"] = u1_np
        m["xfT"] = xfT_np
        m["discf"] = discf_np
        us_loc = Us_full[sl]
        m["m1x"] = np.ascontiguousarray(
            x[sl].T @ us_loc
            + np.outer(inp["cheb_b"].astype(F32), us_loc.sum(0)))
        m["disc"] = np.ascontiguousarray(
            (vs * dis_cl[sl]).reshape(NT, 128).T)
        m["disc_m"] = np.ascontiguousarray(
            (-scale / vs * dis_c).reshape(NT, 128).T)
        m["gidx"] = cores[c]["gidx"]
        m["s8"] = cores[c]["s8"]
        in_maps.append(m)

    import os
    global LAST_NC, LAST_IN_MAPS
    LAST_NC = nc
    LAST_IN_MAPS = in_maps
    trace = os.environ.get("KERNEL_TRACE", "0") == "1"
    res = run_bass_kernel_spmd(nc, in_maps, core_ids=list(range(NCORES)),
                               trace=trace)
    global LAST_EXEC_NS, LAST_RESULT
    LAST_EXEC_NS = res.exec_time_ns
    LAST_RESULT = res
    out = np.concatenate([res.results[c]["out_nm"] for c in range(NCORES)], axis=0)
    return out.astype(inp["x"].dtype)



# revision 47
# speedup vs baseline: 1.9643x; 1.0141x over previous
"""Trainium2 Bass kernel for nn_CachedSpectralGPSLayer (8-core SPMD).

Self-contained: takes FULL inputs, shards per-core internally, runs one
Bass/Tile program SPMD on 8 NeuronCores, gathers the full output.
"""
import os
import sys

sys.path.insert(0, "/opt/trn_rl_repo")

import numpy as np
import ml_dtypes

import concourse.bacc as bacc
import concourse.bass as bass
import concourse.mybir as mybir
import concourse.tile as tile
from concourse import library_config
from concourse.bass_utils import run_bass_kernel_spmd

BF16 = ml_dtypes.bfloat16
FP8 = ml_dtypes.float8_e4m3
F32 = np.float32

N, C, K, KEIG, B, NG, H = 32768, 128, 5, 128, 64, 512, 4
NCORES = 8
NLOC = N // NCORES          # 4096
NT = NLOC // 128            # 32 node tiles per core
Bd = 64                     # dst nodes per block
NBLK = NLOC // Bd           # 64 blocks per core
GPC = B // NCORES           # 8 graphs per core
DH = C // H                 # 32
EPS = 1e-5
BPG = 1                     # blocks per dma_gather call
NCALL = NBLK // BPG         # 16 gather calls per hop
FP8V = False                # gather/AllGather payload (v~) in fp8e4m3
VSCALE = 16.0               # v~ pre-scale (keeps fp8 out of subnormals)
# virtual-schedule stamps (ms units): hop start = ST0 + STH*(h-1),
# window = hop + STW, tail = ST0 + STH*hops. Scheduler ordering hints only.
ST0 = float(os.environ.get("K_ST0", "0.45"))
STH = float(os.environ.get("K_STH", "0.40"))
STW = float(os.environ.get("K_STW", "0.20"))

fp32 = mybir.dt.float32
f32r = mybir.dt.float32r
bf16 = mybir.dt.bfloat16
fp8 = mybir.dt.float8e4
i16 = mybir.dt.int16


def R(ap):
    """Bitcast an fp32 AP to float32r: bit-identical fp32 data, but the PE
    runs replicated mode (1 cyc/row when moving dim >=256 vs 4 for fp32)."""
    return ap.bitcast(f32r)

# bias-pack column indices
(BI_SPA1, BI_SPA2, BI_SPE1, BI_SPE2, BI_Q, BI_K, BI_OUTP, BI_CHEB,
 BI_M1A, BI_M1B, BI_M2, BI_BN1W, BI_BN1B, BI_BN2W, BI_BN2B, BI_BN3W,
 BI_BN3B, BI_EPSC) = range(18)
NBIAS = 18

_CACHE = {}


def _wrap_idx(idx_flat):
    """dma_gather wrapped layout per call: idx i -> [i%16, i//16], replicated
    to all 8 groups of 16 partitions. idx_flat: [ncalls, n_per_call]."""
    ncalls, npc = idx_flat.shape
    base = idx_flat.reshape(ncalls, npc // 16, 16).transpose(0, 2, 1)  # [ncalls,16,npc/16]
    out = np.tile(base, (1, 8, 1))                                     # [ncalls,128,npc/16]
    return np.concatenate(list(out), axis=1)                           # [128, ncalls*npc/16]


def _preprocess(inputs):
    src = np.asarray(inputs["edge_index"][0]).astype(np.int64)
    dst = np.asarray(inputs["edge_index"][1]).astype(np.int64)
    deg = np.bincount(src, minlength=N).astype(np.float64)
    dis = np.where(deg > 0, 1.0 / np.sqrt(deg), 0.0).astype(F32)
    lam = float(np.asarray(inputs["lambda_max"]).reshape(-1)[0])
    scale = 2.0 / lam

    order = np.argsort(dst, kind="stable")
    srcs, dsts = src[order], dst[order]
    counts = np.bincount(dst // Bd, minlength=N // Bd)
    ngrp = int(np.ceil(counts.max() / 128))
    epb = ngrp * 128                    # padded edges per block
    epad = NBLK * epb                   # per core

    cores = []
    bounds = np.searchsorted(dsts, np.arange(0, N + 1, NLOC))
    for c in range(NCORES):
        lo = c * NLOC
        sl = slice(bounds[c], bounds[c + 1])
        sc, dc = srcs[sl], dsts[sl] - lo
        blk = dc // Bd
        ord2 = np.lexsort((sc, blk))  # sort by src within each dst block
        sc, dc, blk = sc[ord2], dc[ord2], blk[ord2]
        cnt = np.bincount(blk, minlength=NBLK)
        csum = np.concatenate([[0], np.cumsum(cnt)])
        pos_in_blk = np.arange(len(sc)) - csum[blk]
        slot = blk * epb + pos_in_blk
        src_pad = np.zeros(epad, np.int64)
        src_pad[slot] = sc
        # S one-hot fp8 bytes [128, ngroups*Bd]
        ngroups = NBLK * ngrp
        s8 = np.zeros((128, ngroups * Bd), np.uint8)
        g = slot // 128
        p = slot % 128
        s8[p, g * Bd + (dc - blk * Bd)] = 0x38  # fp8e4m3 1.0
        assert src_pad.max() < 2 ** 15
        gidx = _wrap_idx(src_pad.astype(np.int16).reshape(NCALL, BPG * epb))
        cores.append(dict(s8=s8.view(FP8), gidx=gidx))
    return cores, dis, scale, ngrp


def _build(ngrp, scale, hops=K - 1, do_mha=True, do_spec=True):
    do_spec = True
    """Build + compile the SPMD Bass program. Returns (nc, input names)."""
    epb = ngrp * 128
    epad = NBLK * epb
    ngroups = NBLK * ngrp

    nc = bacc.Bacc("TRN2", target_bir_lowering=False, debug=False,
                   enable_asserts=True, num_devices=NCORES,
                   num_swdge_queues=4)

    def din(name, shape, dt):
        return nc.dram_tensor(name, shape, dt, kind="ExternalInput").ap()

    xT = din("xT", [128, NLOC], bf16)
    u_nm = din("u_nm", [NLOC, 128], bf16)
    usT = din("usT", [128, NLOC], bf16)
    disc = din("disc", [128, NT], fp32)
    disc_m = din("disc_m", [128, NT], fp32)
    gidx = din("gidx", [128, epad // 16], i16)
    s8 = din("s8", [128, ngroups * Bd], fp8)
    wspa1 = din("wspa1", [128, 128], bf16)
    wspa2 = din("wspa2", [128, 128], bf16)
    wspe1 = din("wspe1", [128, 128], bf16)
    wspe2 = din("wspe2", [128, 128], bf16)
    wproj = din("wproj", [128, 128], bf16)
    chebw = din("chebw", [128, K * 128], bf16)
    wq = din("wq", [128, 128], bf16)
    wk = din("wk", [128, 128], bf16)
    wv = din("wv", [128, 128], bf16)
    wout = din("wout", [128, 128], bf16)
    mw1 = din("mw1", [128, 256], bf16)
    mw2a = din("mw2a", [128, 128], bf16)
    mw2b = din("mw2b", [128, 128], bf16)
    biasp = din("biasp", [128, NBIAS], fp32)
    bqh = din("bqh", [32, H], fp32)
    bkh = din("bkh", [32, H], fp32)
    idm = din("idm", [128, 128], fp32)
    gmat = din("gmat", [128, 128], fp32)
    u1col = din("u1col", [128, 1], fp32)
    usd = din("usd", [NLOC, 128], bf16)
    idmb = din("idmb", [128, 128], bf16)
    onesf = din("onesf", [128, 1], fp32)
    one1 = din("one1", [1, 1], fp32)
    m1x = din("m1x", [128, 128], fp32)
    xfT = din("xfT", [128, N], bf16)
    discf = din("discf", [128, N // 128], fp32)

    out_nm = nc.dram_tensor("out_nm", [NLOC, 128], fp32, kind="ExternalOutput").ap()

    AF = mybir.ActivationFunctionType
    OP = mybir.AluOpType

    with tile.TileContext(nc) as tc:
        with tc.tile_pool(name="const", bufs=1) as cp, \
             tc.tile_pool(name="big", bufs=1) as bp, \
             tc.tile_pool(name="work", bufs=2) as wp, \
             tc.tile_pool(name="psmm", bufs=2, space="PSUM") as psmm, \
             tc.tile_pool(name="psat", bufs=1, space="PSUM") as psatp, \
             tc.tile_pool(name="pspt", bufs=2, space="PSUM") as psptp, \
             tc.tile_pool(name="psagg", bufs=3, space="PSUM") as psaggp, \
             tc.tile_pool(name="dram", bufs=1, space="DRAM") as dp:

            # uniform-tag psum allocators (PSUM = 8 banks total: 2+2+2+2)
            def PS_MM():   # transient [128,512] matmul outputs
                return psmm.tile([128, 512], fp32, tag="mm", name="psmm_t")

            def PS_AT(shape):  # long-lived accumulators / phase-2 partial
                return psatp.tile(shape, fp32, tag="at", name="psat_t")

            def PS_ATW():  # MHA attn+denom accumulator [128, 132]
                return psatp.tile([128, 33 * H], fp32, tag="at", name="psatw_t")

            def PS_PT2():  # [32, 512] head q/k psum
                return psptp.tile([32, 512], fp32, tag="pt", name="pspt2_t")

            def PS_PT():   # [128,128] transposes / small matmuls
                return psptp.tile([128, 128], fp32, tag="pt", name="pspt_t")

            def PS_AGG():  # [128,128] cheb aggregation
                return psaggp.tile([128, 128], fp32, tag="agg", name="psagg_t")

            nc.gpsimd.load_library(library_config.mlp)

            # ---- load constants ----
            def ld(ap_in, shape, dt, name, eng=None):
                t = cp.tile(shape, dt, tag=name, name=name)
                (eng or nc.sync).dma_start(t[:], ap_in[:])
                return t

            xT_s = bp.tile([128, NLOC], bf16, tag="xT")
            nc.sync.dma_start(xT_s[:], xT[:])
            s8_s = ld(s8, [128, ngroups * Bd], fp8, "s8")
            wspa1_s = ld(wspa1, [128, 128], bf16, "wspa1")
            wspa2_s = ld(wspa2, [128, 128], bf16, "wspa2")
            wspe1_s = ld(wspe1, [128, 128], bf16, "wspe1")
            wspe2_s = ld(wspe2, [128, 128], bf16, "wspe2")
            wproj_s = ld(wproj, [128, 128], bf16, "wproj")
            chebw_s = ld(chebw, [128, K * 128], bf16, "chebw")
            wq_s = ld(wq, [128, 128], bf16, "wq")
            wk_s = ld(wk, [128, 128], bf16, "wk")
            wv_s = ld(wv, [128, 128], bf16, "wv")
            wout_s = ld(wout, [128, 128], bf16, "wout")
            mw1_s = ld(mw1, [128, 256], bf16, "mw1")
            mw2a_s = ld(mw2a, [128, 128], bf16, "mw2a")
            mw2b_s = ld(mw2b, [128, 128], bf16, "mw2b")
            biasp_s = ld(biasp, [128, NBIAS], fp32, "biasp")
            bqh_s = ld(bqh, [32, H], fp32, "bqh")
            bkh_s = ld(bkh, [32, H], fp32, "bkh")
            idm_s = ld(idm, [128, 128], fp32, "idm")
            idmb_s = ld(idmb, [128, 128], bf16, "idmb")
            vbuf = bp.tile([128, NT, 128], bf16, tag="vbuf")

            def th_accum(t_ps):
                # T_h = v~^T (Us/dis): stream usd in 8-tile chunks (window
                # work, off the gather critical path; scalar-queue DMA)
                for t0 in range(0, NT, 8):
                    ub8 = wp.tile([128, 8, 128], bf16, tag="ut8", bufs=1)
                    nc.scalar.dma_start(
                        ub8[:], usd[t0 * 128:(t0 + 8) * 128, :]
                        .rearrange("(t p) c -> p t c", p=128))
                    for j in range(8):
                        t = t0 + j
                        nc.tensor.matmul(t_ps[:], lhsT=vbuf[:, t, :],
                                         rhs=ub8[:, j, :],
                                         start=(t == 0), stop=(t == NT - 1))
            gmat_s = ld(gmat, [128, 128], fp32, "gmat")
            u1col_s = ld(u1col, [128, 1], fp32, "u1col")
            onesf_s = ld(onesf, [128, 1], fp32, "onesf")
            one1_s = ld(one1, [1, 1], fp32, "one1")
            m1x_s = ld(m1x, [128, 128], fp32, "m1x")
            m1sb = cp.tile([128, 128], fp32, tag="m1sb", name="m1sb")
            discf_s = ld(discf, [128, N // 128], fp32, "discf")
            disc_s = ld(disc, [128, NT], fp32, "disc")
            discm_s = ld(disc_m, [128, 2 * NT], fp32, "discm")
            discm2_s = discm_s[:, NT:2 * NT]

            def bcol(i):
                return biasp_s[:, i:i + 1]

            # persistent big buffers
            TxA = bp.tile([128, NT, 128], fp32, tag="TxA")   # node-major
            TxB = bp.tile([128, NT, 128], fp32, tag="TxB")
            chebT = bp.tile([128, NLOC], fp32, tag="chebT")  # later: pre1, outT
            h2T = bp.tile([128, NLOC], fp32, tag="h2T")      # later: pre2, out2T
            if not do_mha:
                nc.vector.memset(h2T[:], 0.0)
            statc = cp.tile([128, 32], fp32, tag="statc")    # stats/affine cols
            statc2 = cp.tile([128, 16], fp32, tag="statc2")  # per-graph h2 stats
            outnm = TxA  # reuse (dead after cheb)

            # DRAM bounce buffers
            vdt = fp8 if FP8V else bf16
            ag_in = dp.tile([NLOC, 128], vdt, tag="ag_in")
            ag_outs = [None] + [dp.tile([N, 128], vdt, tag=f"ag_out{h}", name=f"ag_out{h}", addr_space="Shared") for h in range(1, 4)]
            vfull0 = dp.tile([N, 128], vdt, tag="vfull0", name="vfull0")
            bn12_in = dp.tile([128, 260], fp32, tag="bn12_in")
            bn12_out = dp.tile([128, 260], fp32, tag="bn12_out", addr_space="Shared")
            bn3_in = dp.tile([128, 2], fp32, tag="bn3_in")
            bn3_out = dp.tile([128, 2], fp32, tag="bn3_out", addr_space="Shared")

            CH = NLOC // 512  # 8 chunks of 512

            from contextlib import ExitStack
            ep_stack = ExitStack()
            ep = ep_stack.enter_context(tc.tile_pool(name="early", bufs=1))
            xspT = ep.tile([128, NLOC], bf16, tag="xspT", name="xspT")

            # ================= Phase 1: local spa MLP (feature-major) =======
            # (the spe MLP + spectral partial run inside AllGather window 1,
            # recomputed from the resident xT_s, off the pre-hop critical path)
            for ch in range(CH):
                sl = slice(ch * 512, (ch + 1) * 512)
                p1 = PS_MM()
                nc.tensor.matmul(p1[:], lhsT=wspa1_s[:], rhs=xT_s[:, sl],
                                 start=True, stop=True)
                t1 = wp.tile([128, 512], bf16, tag="t1b")
                nc.scalar.activation(t1[:], p1[:], AF.Relu, bias=bcol(BI_SPA1))
                p2 = PS_MM()
                nc.tensor.matmul(p2[:], lhsT=wspa2_s[:], rhs=t1[:],
                                 start=True, stop=True)
                nc.scalar.activation(xspT[:, sl], p2[:], AF.Identity,
                                     bias=bcol(BI_SPA2))

            # Tx0 node-major (local shard, for recurrence) + v~0 into vbuf
            for t in range(NT):
                tsl = slice(t * 128, (t + 1) * 128)
                pt = PS_PT()
                nc.tensor.matmul(pt[:], lhsT=xspT[:, tsl], rhs=idmb_s[:],
                                 start=True, stop=True)
                nc.vector.tensor_copy(TxB[:, t, :], pt[:])
                nc.scalar.activation(vbuf[:, t, :], pt[:], AF.Identity,
                                     scale=disc_s[:, t:t + 1])
            # v~0 for ALL nodes computed locally (replaces hop-1 AllGather):
            # every core redundantly runs the spatial MLP over the full x.
            if hops >= 1:
                for gch in range(N // 512):
                    gsl2 = slice(gch * 512, (gch + 1) * 512)
                    xc = wp.tile([128, 512], bf16, tag="t1b")
                    nc.sync.dma_start(xc[:], xfT[:, gsl2])
                    pf1 = PS_MM()
                    nc.tensor.matmul(pf1[:], lhsT=wspa1_s[:], rhs=xc[:],
                                     start=True, stop=True)
                    tf1 = wp.tile([128, 512], bf16, tag="midab")
                    nc.scalar.activation(tf1[:], pf1[:], AF.Relu,
                                         bias=bcol(BI_SPA1))
                    pf2 = PS_MM()
                    nc.tensor.matmul(pf2[:], lhsT=wspa2_s[:], rhs=tf1[:],
                                     start=True, stop=True)
                    spf = wp.tile([128, 512], bf16, tag="t1b")
                    nc.scalar.activation(spf[:], pf2[:], AF.Identity,
                                         bias=bcol(BI_SPA2))
                    vt4 = wp.tile([128, 4, 128], vdt, tag="vt4", bufs=2)
                    for j in range(4):
                        tg = gch * 4 + j
                        ptf = PS_PT()
                        nc.tensor.matmul(ptf[:], lhsT=spf[:, j * 128:(j + 1) * 128],
                                         rhs=idmb_s[:], start=True, stop=True)
                        nc.scalar.activation(vt4[:, j, :], ptf[:], AF.Identity,
                                             scale=discf_s[:, tg:tg + 1])
                    nc.sync.dma_start(
                        vfull0[gch * 512:(gch + 1) * 512, :]
                        .rearrange("(t p) c -> p t c", p=128), vt4[:])

            for ch in range(CH):
                sl = slice(ch * 512, (ch + 1) * 512)
                pw = PS_MM()
                nc.tensor.matmul(pw[:], lhsT=chebw_s[:, 0:128], rhs=xspT[:, sl],
                                 start=True, stop=True)
                nc.vector.tensor_copy(chebT[:, sl], pw[:])
            # T_0 = v~0^T (Us/dis) (accumulate over tiles); m1sb = m1x + W0^T T_0
            t_ps = PS_AGG()
            th_accum(t_ps)
            tsb = wp.tile([128, 128], bf16, tag="tsb")
            nc.vector.tensor_copy(tsb[:], t_ps[:])
            pWt = PS_PT()
            nc.tensor.matmul(pWt[:], lhsT=chebw_s[:, 0:128], rhs=tsb[:],
                             start=True, stop=True)
            nc.vector.tensor_add(m1sb[:], m1x_s[:], pWt[:])

            # ===== Phase 2 (deferred): spectral partial, run in AG window 1 =
            # Recomputes the spe MLP chunk-wise from resident xT_s so xspecT
            # needs no SBUF residency across the hops.
            part_s = wp.tile([128, 128], fp32, tag="part_s", bufs=1)

            def spectral_partial():
                part_ps = PS_AT([128, 128])
                for ch2 in range(CH):
                    sl2 = slice(ch2 * 512, (ch2 + 1) * 512)
                    p3 = PS_MM()
                    nc.tensor.matmul(p3[:], lhsT=wspe1_s[:], rhs=xT_s[:, sl2],
                                     start=True, stop=True)
                    t2 = wp.tile([128, 512], bf16, tag="t1b")
                    nc.scalar.activation(t2[:], p3[:], AF.Relu,
                                         bias=bcol(BI_SPE1))
                    p4 = PS_MM()
                    nc.tensor.matmul(p4[:], lhsT=wspe2_s[:], rhs=t2[:],
                                     start=True, stop=True)
                    xsp2 = wp.tile([128, 512], bf16, tag="t1b")
                    nc.scalar.activation(xsp2[:], p4[:], AF.Identity,
                                         bias=bcol(BI_SPE2))
                    ub = wp.tile([128, 4, 128], bf16, tag="ut4", bufs=1)
                    nc.sync.dma_start(
                        ub[:], u_nm[ch2 * 512:(ch2 + 1) * 512, :]
                        .rearrange("(t p) c -> p t c", p=128))
                    for j in range(4):
                        t = ch2 * 4 + j
                        ph = PS_PT()
                        nc.tensor.matmul(ph[:], lhsT=xsp2[:, j * 128:(j + 1) * 128],
                                         rhs=wproj_s[:], start=True, stop=True)
                        hp = wp.tile([128, 128], bf16, tag="hp")
                        nc.vector.tensor_copy(hp[:], ph[:])
                        nc.tensor.matmul(part_ps[:], lhsT=ub[:, j, :], rhs=hp[:],
                                         start=(t == 0), stop=(t == NT - 1))
                nc.vector.tensor_copy(part_s[:], part_ps[:])

            ep_stack.close()  # free xspT space for later pools
            late_stack = ExitStack()
            gp = late_stack.enter_context(tc.tile_pool(name="gath", bufs=2))
            mp = late_stack.enter_context(tc.tile_pool(name="mha", bufs=2))

            # ---- MHA for one graph (interleaved into AllGather windows) ----
            def mha_graph(g):
                gsl = slice(g * 512, (g + 1) * 512)
                # head-major q/k: per-head matmuls so all operands are base-0
                qT = mp.tile([32, H * 512], bf16, tag="qT", bufs=1)
                kT = mp.tile([32, H * 512], bf16, tag="kT", bufs=1)
                for hh in range(H):
                    csl = slice(hh * 32, (hh + 1) * 32)
                    pqh = PS_PT2()
                    nc.tensor.matmul(pqh[:], lhsT=wq_s[:, csl],
                                     rhs=xT_s[:, gsl], start=True, stop=True)
                    nc.scalar.activation(qT[:, hh * 512:(hh + 1) * 512], pqh[:],
                                         AF.Identity, bias=bqh_s[:, hh:hh + 1])
                    pkh = PS_PT2()
                    nc.tensor.matmul(pkh[:], lhsT=wk_s[:, csl],
                                     rhs=xT_s[:, gsl], start=True, stop=True)
                    nc.scalar.activation(kT[:, hh * 512:(hh + 1) * 512], pkh[:],
                                         AF.Identity, bias=bkh_s[:, hh:hh + 1])
                # v node-major, augmented per head with a ones column
                vaug = mp.tile([128, 4, 33 * H], bf16, tag="vaug")
                nc.vector.memset(vaug[:, :, 32::33], 1.0)
                for j in range(4):
                    pv = PS_PT()
                    nc.tensor.matmul(pv[:], lhsT=xT_s[:, g * 512 + j * 128:
                                                      g * 512 + (j + 1) * 128],
                                     rhs=wv_s[:], start=True, stop=True)
                    for hh in range(H):
                        nc.vector.tensor_copy(
                            vaug[:, j, hh * 33:hh * 33 + 32],
                            pv[:, hh * 32:(hh + 1) * 32])
                # scores_T + exp, per (head, k-chunk)
                ess = {}
                for hh in range(H):
                    qsl = slice(hh * 512, (hh + 1) * 512)
                    for j in range(4):
                        pss = PS_MM()
                        nc.tensor.matmul(
                            pss[:], lhsT=kT[:, hh * 512 + j * 128:
                                            hh * 512 + (j + 1) * 128],
                            rhs=qT[:, qsl], start=True, stop=True)
                        es = mp.tile([128, 512], bf16, tag="es", bufs=16)
                        nc.scalar.activation(es[:], pss[:], AF.Exp)
                        ess[(hh, j)] = es
                # attn + denom per q-chunk
                for qc in range(4):
                    pat = PS_ATW()
                    for hh in range(H):
                        for j in range(4):
                            nc.tensor.matmul(
                                pat[:, hh * 33:(hh + 1) * 33],
                                lhsT=ess[(hh, j)][:, qc * 128:(qc + 1) * 128],
                                rhs=vaug[:, j, hh * 33:(hh + 1) * 33],
                                start=(j == 0), stop=(j == 3),
                                skip_group_check=True)
                    recip = wp.tile([128, 4], fp32, tag="recip")
                    nc.vector.reciprocal(recip[:], pat[:, 32::33])
                    anm = wp.tile([128, 128], fp32, tag="anm")
                    for hh in range(H):
                        nc.vector.tensor_scalar(
                            out=anm[:, hh * 32:(hh + 1) * 32],
                            in0=pat[:, hh * 33:hh * 33 + 32],
                            scalar1=recip[:, hh:hh + 1], scalar2=None,
                            op0=OP.mult)
                    ptr = PS_PT()
                    nc.tensor.transpose(ptr[:], anm[:], idm_s[:])
                    attnT = wp.tile([128, 128], bf16, tag="attnT", bufs=1)
                    nc.vector.tensor_copy(attnT[:], ptr[:])
                    ph2 = PS_PT()
                    nc.tensor.matmul(ph2[:], lhsT=wout_s[:], rhs=attnT[:],
                                     start=True, stop=True)
                    osl = slice(g * 512 + qc * 128, g * 512 + (qc + 1) * 128)
                    # pre2 = h2 + b_out' + x
                    nc.vector.scalar_tensor_tensor(
                        out=h2T[:, osl], in0=ph2[:], scalar=bcol(BI_OUTP),
                        in1=xT_s[:, osl], op0=OP.add, op1=OP.add)
                # incremental BN2 stats for this graph's 512 columns
                nc.vector.tensor_reduce(statc2[:, g:g + 1], h2T[:, gsl],
                                        mybir.AxisListType.X, OP.add)
                tsq = wp.tile([128, 512], fp32, tag="sqt", bufs=1)
                nc.vector.scalar_tensor_tensor(
                    out=tsq[:], in0=h2T[:, gsl], scalar=1.0, in1=h2T[:, gsl],
                    op0=OP.mult, op1=OP.mult,
                    accum_out=statc2[:, 8 + g:9 + g])

            # graphs run inside AllGather wait windows (PE idle otherwise)
            mha_sched = {1: [0, 1, 2], 2: [3, 4, 5], 3: [6, 7]} \
                if (do_mha and hops == K - 1) else {}
            mha_left = [g for g in range(GPC if do_mha else 0)
                        if not any(g in v for v in mha_sched.values())]

            # ================= Phase 3: cheb hops ===========================
            # tile_wait_until stamps are scheduler-only hints (virtual
            # earliest-start): they stop the list scheduler from hoisting
            # hop h+1's recurrence ops ahead of the window-h MHA work in
            # the in-order DVE queue (head-of-line blocking during the
            # AllGather). They emit no HW waits.
            cur, prev = TxB, TxA  # cur holds Tx_{h-1}; prev gets Tx_h
            for h in range(1, 1 + hops):
                t_hop = ST0 + STH * (h - 1)
                ag_src = vfull0 if h == 1 else ag_outs[h - 1]
                hop_stack = ExitStack()
                hop_stack.enter_context(tc.tile_wait_until(t_hop))
                for q in range(NCALL):
                    gt = gp.tile([128, BPG * ngrp, 128], vdt, tag="gt", bufs=12)
                    isl = slice(q * BPG * epb // 16, (q + 1) * BPG * epb // 16)
                    gix = wp.tile([128, BPG * epb // 16], i16, tag="gix", bufs=12)
                    nc.sync.dma_start(gix[:], gidx[:, isl])
                    nc.gpsimd.dma_gather(gt[:], ag_src[:], gix[:],
                                         BPG * epb, BPG * epb, 128,
                                         single_packet=False,
                                         queue_num=q % 4)
                    for r in range(BPG):
                        b = q * BPG + r
                        t, half = b // 2, b % 2
                        if half == 0:
                            aps = PS_AGG()
                        for j in range(ngrp):
                            gcol = b * ngrp + j
                            nc.tensor.matmul(
                                aps[half * 64:(half + 1) * 64, :],
                                lhsT=s8_s[:, gcol * Bd:(gcol + 1) * Bd],
                                rhs=gt[:, r * ngrp + j, :],
                                start=(j == 0), stop=(j == ngrp - 1))
                        if half == 1:
                            # recurrence for tile t
                            tmp = wp.tile([128, 128], fp32, tag="rectmp")
                            if h == 1:
                                nc.vector.tensor_scalar(
                                    out=tmp[:], in0=aps[:],
                                    scalar1=discm_s[:, t:t + 1], scalar2=None,
                                    op0=OP.mult)
                                # Tx1 = (scale-1)*Tx0 + tmp
                                nc.vector.scalar_tensor_tensor(
                                    out=prev[:, t, :], in0=cur[:, t, :],
                                    scalar=float(scale - 1.0), in1=tmp[:],
                                    op0=OP.mult, op1=OP.add)
                            else:
                                nc.vector.tensor_scalar(
                                    out=tmp[:], in0=aps[:],
                                    scalar1=discm_s[:, t:t + 1], scalar2=2.0,
                                    op0=OP.mult, op1=OP.mult)
                                # tmp2 = tmp - Tx_{h-2}
                                tmp2 = wp.tile([128, 128], fp32, tag="rectmp2")
                                nc.vector.scalar_tensor_tensor(
                                    out=tmp2[:], in0=prev[:, t, :],
                                    scalar=-1.0, in1=tmp[:],
                                    op0=OP.mult, op1=OP.add)
                                # Tx_h = 2(scale-1)*Tx_{h-1} + tmp2
                                nc.vector.scalar_tensor_tensor(
                                    out=prev[:, t, :], in0=cur[:, t, :],
                                    scalar=float(2.0 * (scale - 1.0)),
                                    in1=tmp2[:], op0=OP.mult, op1=OP.add)
                            nc.scalar.activation(
                                vbuf[:, t, :], prev[:, t, :], AF.Identity,
                                scale=disc_s[:, t:t + 1])
                            if h < hops:
                                nc.sync.dma_start(
                                    ag_in[t * 128:(t + 1) * 128, :],
                                    vbuf[:, t, :])
                # launch AG for next hop once all v~ tiles written
                if h < hops:
                    nc.gpsimd.collective_compute(
                        "AllGather", OP.bypass,
                        replica_groups=[list(range(NCORES))],
                        ins=[ag_in.opt()], outs=[ag_outs[h].opt()])
                hop_stack.close()
                win_stack = ExitStack()
                win_stack.enter_context(tc.tile_wait_until(t_hop + STW))
                # out_cheb += Tx_h @ W_h  (transpose tiles chunk-wise)
                for ch in range(CH):
                    tpb = wp.tile([128, 512], bf16, tag="tpb", bufs=1)
                    for j in range(4):
                        t = ch * 4 + j
                        pt = PS_PT()
                        nc.tensor.transpose(pt[:], prev[:, t, :], idm_s[:])
                        nc.vector.tensor_copy(tpb[:, j * 128:(j + 1) * 128], pt[:])
                    sl = slice(ch * 512, (ch + 1) * 512)
                    pw = PS_MM()
                    nc.tensor.matmul(pw[:], lhsT=chebw_s[:, h * 128:(h + 1) * 128],
                                     rhs=tpb[:], start=True, stop=True)
                    nc.vector.tensor_add(chebT[:, sl], chebT[:, sl], pw[:])
                # T_h = v~_h^T (Us/dis) ; m1sb += W_h^T T_h  (off the tail)
                t_ps = PS_AGG()
                th_accum(t_ps)
                tsb = wp.tile([128, 128], bf16, tag="tsb")
                nc.vector.tensor_copy(tsb[:], t_ps[:])
                pWt = PS_PT()
                nc.tensor.matmul(pWt[:], lhsT=chebw_s[:, h * 128:(h + 1) * 128],
                                 rhs=tsb[:], start=True, stop=True)
                nc.vector.tensor_add(m1sb[:], m1sb[:], pWt[:])
                if h == 1:
                    spectral_partial()
                for g in mha_sched.get(h, []):
                    mha_graph(g)
                win_stack.close()
                cur, prev = prev, cur

            # ================= Phase 4: MHA (remaining graphs) ==============
            for g in mha_left:
                mha_graph(g)

            tail_stack = ExitStack()
            tail_stack.enter_context(tc.tile_wait_until(ST0 + STH * hops))
            # ===== Phase 5: pre1' (no spec) + BN stats + M1 + joint AR ======
            for ch in range(CH):
                sl = slice(ch * 512, (ch + 1) * 512)
                # pre1' = chebT + cheb_b + x   (overwrite chebT)
                nc.vector.scalar_tensor_tensor(
                    out=chebT[:, sl], in0=chebT[:, sl], scalar=bcol(BI_CHEB),
                    in1=xT_s[:, sl], op0=OP.add, op1=OP.add)
            # BN1/BN2 stats
            def sumsq(buf, out_col):
                for c2 in range(CH):
                    s2 = slice(c2 * 512, (c2 + 1) * 512)
                    tt = wp.tile([128, 512], fp32, tag="sqt", bufs=1)
                    nc.vector.scalar_tensor_tensor(
                        out=tt[:], in0=buf[:, s2], scalar=1.0, in1=buf[:, s2],
                        op0=OP.mult, op1=OP.mult,
                        accum_out=statc[:, 24 + c2:25 + c2])
                nc.vector.tensor_reduce(out_col, statc[:, 24:32],
                                        mybir.AxisListType.X, OP.add)

            nc.vector.tensor_reduce(statc[:, 0:1], chebT[:], mybir.AxisListType.X, OP.add)
            sumsq(chebT, statc[:, 1:2])
            if do_mha:
                nc.vector.tensor_reduce(statc[:, 2:3], statc2[:, 0:8],
                                        mybir.AxisListType.X, OP.add)
                nc.vector.tensor_reduce(statc[:, 3:4], statc2[:, 8:16],
                                        mybir.AxisListType.X, OP.add)
            else:
                nc.vector.tensor_reduce(statc[:, 2:3], h2T[:], mybir.AxisListType.X, OP.add)
                sumsq(h2T, statc[:, 3:4])
            st12 = wp.tile([128, 260], fp32, tag="st12", bufs=1)
            nc.vector.tensor_copy(st12[:, 0:4], statc[:, 0:4])
            nc.vector.tensor_copy(st12[:, 4:132], m1sb[:])
            nc.vector.tensor_copy(st12[:, 132:260], part_s[:])
            nc.sync.dma_start(bn12_in[:], st12[:])
            nc.gpsimd.collective_compute(
                "AllReduce", OP.add, replica_groups=[list(range(NCORES))],
                ins=[bn12_in.opt()], outs=[bn12_out.opt()])
            sr12 = wp.tile([128, 260], fp32, tag="sr12", bufs=1)
            nc.sync.dma_start(sr12[:], bn12_out[:])
            Pm = sr12[:, 132:260]   # AR'd spectral partial [keig, C]
            M1g = sr12[:, 4:132]    # [C, keig]
            # spectral stat terms:
            # s_sum[c] = sum_k u1[k] P[k,c];  s_sq[c] = sum_k P[k,c](G P)[k,c]
            # cross[c] = sum_k M1g[c,k] P[k,c]
            w12 = wp.tile([128, 256], fp32, tag="w12", bufs=1)
            nc.vector.tensor_scalar(out=w12[:, 0:128], in0=Pm,
                                    scalar1=u1col_s[:], scalar2=None,
                                    op0=OP.mult)
            t1_ps = PS_PT()
            nc.tensor.matmul(t1_ps[:], lhsT=gmat_s[:], rhs=Pm,
                             start=True, stop=True)
            nc.vector.tensor_tensor(w12[:, 128:256], t1_ps[:], Pm, OP.mult)
            # column sums: ones-matmul -> [1,256] row, then row -> two cols
            r_ps = PS_MM()
            nc.tensor.matmul(r_ps[0:1, 0:256], lhsT=onesf_s[:], rhs=w12[:],
                             start=True, stop=True)
            rowbuf = wp.tile([1, 256], fp32, tag="rowbuf")
            nc.vector.tensor_copy(rowbuf[:], r_ps[0:1, 0:256])
            c_ps = PS_PT()
            nc.tensor.matmul(c_ps[:, 0:1], lhsT=rowbuf[:, 0:128], rhs=one1_s[:],
                             start=True, stop=True, skip_group_check=True)
            nc.tensor.matmul(c_ps[:, 1:2], lhsT=rowbuf[:, 128:256], rhs=one1_s[:],
                             start=True, stop=True, skip_group_check=True)
            s_cols = wp.tile([128, 2], fp32, tag="s_cols")
            nc.vector.tensor_copy(s_cols[:], c_ps[:, 0:2])
            # cross: transpose P, multiply with M1g, reduce
            pt_ps = PS_PT()
            nc.tensor.transpose(pt_ps[:], sr12[:, 132:260], idm_s[:])
            ptm = wp.tile([128, 128], fp32, tag="w1t")
            nc.vector.tensor_tensor(ptm[:], pt_ps[:], M1g, OP.mult)
            crossc = statc[:, 14:15]
            nc.vector.tensor_reduce(crossc, ptm[:], mybir.AxisListType.X, OP.add)
            # BN1 totals: sum1 = sr12[:,0] + s_sum ; sq1 = sr12[:,1] + 2*cross + s_sq
            sum1c = statc[:, 15:16]
            nc.vector.tensor_tensor(sum1c, sr12[:, 0:1], s_cols[:, 0:1], OP.add)
            sq1c = statc[:, 16:17]
            nc.vector.scalar_tensor_tensor(out=sq1c, in0=crossc, scalar=2.0,
                                           in1=sr12[:, 1:2], op0=OP.mult,
                                           op1=OP.add)
            nc.vector.tensor_tensor(sq1c, sq1c, s_cols[:, 1:2], OP.add)

            # affine coefs: A = w/sqrt(var+eps), Bc = b - mu*A
            def bn_affine(sum_col, sq_col, w_col, b_col, a_out, b_out_col):
                mu = statc[:, 8:9]
                nc.vector.tensor_scalar(out=mu, in0=sum_col, scalar1=1.0 / N,
                                        scalar2=None, op0=OP.mult)
                msq = statc[:, 9:10]
                nc.vector.tensor_scalar(out=msq, in0=sq_col, scalar1=1.0 / N,
                                        scalar2=None, op0=OP.mult)
                nvar = statc[:, 10:11]
                nc.vector.scalar_tensor_tensor(out=nvar, in0=mu, scalar=mu,
                                               in1=msq, op0=OP.mult,
                                               op1=OP.subtract)  # mu^2 - msq
                sd = statc[:, 11:12]
                nc.scalar.activation(sd, nvar, AF.Sqrt, bias=bcol(BI_EPSC),
                                     scale=-1.0)
                rsd = statc[:, 12:13]
                nc.vector.reciprocal(rsd, sd)
                nc.vector.tensor_tensor(a_out, rsd, w_col, OP.mult)
                nbc = statc[:, 13:14]
                nc.vector.scalar_tensor_tensor(out=nbc, in0=mu, scalar=a_out,
                                               in1=b_col, op0=OP.mult,
                                               op1=OP.subtract)  # mu*A - b
                nc.vector.tensor_scalar(out=b_out_col, in0=nbc, scalar1=-1.0,
                                        scalar2=None, op0=OP.mult)

            A1, B1 = statc[:, 4:5], statc[:, 5:6]
            A2, B2 = statc[:, 6:7], statc[:, 7:8]
            bn_affine(statc[:, 15:16], statc[:, 16:17], bcol(BI_BN1W), bcol(BI_BN1B), A1, B1)
            bn_affine(sr12[:, 2:3], sr12[:, 3:4], bcol(BI_BN2W), bcol(BI_BN2B), A2, B2)
            B12 = statc[:, 5:6]
            nc.vector.tensor_tensor(B12, B1, B2, OP.add)  # B1 += B2 (in place)

            # ============ Phase 6: out = h1 + h2n; MLP2; BN3 ================
            out2T = h2T   # overwrite pre2 per chunk
            Pmb = wp.tile([128, 128], bf16, tag="pmb16", bufs=1)
            nc.vector.tensor_copy(Pmb[:], sr12[:, 132:260])
            for ch in range(CH):
                sl = slice(ch * 512, (ch + 1) * 512)
                ust = wp.tile([128, 512], bf16, tag="ust")
                nc.scalar.dma_start(ust[:], usT[:, sl])
                pso = PS_MM()
                nc.tensor.matmul(pso[:], lhsT=Pmb[:], rhs=ust[:],
                                 start=True, stop=True)
                t1 = wp.tile([128, 512], fp32, tag="t1")
                nc.scalar.activation(t1[:], chebT[:, sl], AF.Identity,
                                     bias=B12, scale=A1)
                tsp = wp.tile([128, 512], fp32, tag="t1")
                nc.vector.scalar_tensor_tensor(
                    out=tsp[:], in0=pso[:], scalar=A1, in1=t1[:],
                    op0=OP.mult, op1=OP.add)
                outT = wp.tile([128, 512], bf16, tag="outTb", bufs=2)
                nc.vector.scalar_tensor_tensor(
                    out=outT[:], in0=h2T[:, sl], scalar=A2, in1=tsp[:],
                    op0=OP.mult, op1=OP.add)
                pma = PS_MM()
                nc.tensor.matmul(pma[:], lhsT=mw1_s[:, 0:128], rhs=outT[:],
                                 start=True, stop=True)
                mida = wp.tile([128, 512], bf16, tag="midab")
                nc.scalar.activation(mida[:], pma[:], AF.Relu, bias=bcol(BI_M1A))
                pmb = PS_MM()
                nc.tensor.matmul(pmb[:], lhsT=mw1_s[:, 128:256], rhs=outT[:],
                                 start=True, stop=True)
                midb = wp.tile([128, 512], bf16, tag="midab")
                nc.scalar.activation(midb[:], pmb[:], AF.Relu, bias=bcol(BI_M1B))
                pmo = PS_MM()
                nc.tensor.matmul(pmo[:], lhsT=mw2a_s[:], rhs=mida[:],
                                 start=True, stop=False)
                nc.tensor.matmul(pmo[:], lhsT=mw2b_s[:], rhs=midb[:],
                                 start=False, stop=True)
                # out2 = out + mlp_b2 + psum
                nc.vector.scalar_tensor_tensor(
                    out=out2T[:, sl], in0=outT[:], scalar=bcol(BI_M2),
                    in1=pmo[:], op0=OP.add, op1=OP.add)
            # BN3 stats
            nc.vector.tensor_reduce(statc[:, 0:1], out2T[:], mybir.AxisListType.X, OP.add)
            sumsq(out2T, statc[:, 1:2])
            st3 = wp.tile([128, 2], fp32, tag="st")
            nc.vector.tensor_copy(st3[:], statc[:, 0:2])
            nc.sync.dma_start(bn3_in[:], st3[:])
            nc.gpsimd.collective_compute(
                "AllReduce", OP.add, replica_groups=[list(range(NCORES))],
                ins=[bn3_in.opt()], outs=[bn3_out.opt()])
            sr3 = wp.tile([128, 2], fp32, tag="st")
            nc.sync.dma_start(sr3[:], bn3_out[:])
            A3, B3 = statc[:, 4:5], statc[:, 5:6]
            bn_affine(sr3[:, 0:1], sr3[:, 1:2], bcol(BI_BN3W), bcol(BI_BN3B), A3, B3)

            # apply BN3, transpose to node-major, write out
            for ch in range(CH):
                sl = slice(ch * 512, (ch + 1) * 512)
                bn3b = wp.tile([128, 512], fp32, tag="t1")
                nc.scalar.activation(bn3b[:], out2T[:, sl], AF.Identity,
                                     bias=B3, scale=A3)
                for j in range(4):
                    t = ch * 4 + j
                    pt = PS_PT()
                    nc.tensor.transpose(pt[:], bn3b[:, j * 128:(j + 1) * 128],
                                        idm_s[:])
                    nc.vector.tensor_copy(outnm[:, t, :], pt[:])
            nc.sync.dma_start(
                out_nm[:].rearrange("(t p) c -> p t c", p=128), outnm[:])
            tail_stack.close()
            late_stack.close()

    nc.compile()
    return nc


def kernel(**inputs):
    inp = {k: np.asarray(v) for k, v in inputs.items()}
    cores, dis, scale, ngrp = _preprocess(inp)

    key = (ngrp, float(scale))
    if key not in _CACHE:
        _CACHE[key] = _build(ngrp, scale)
    nc = _CACHE[key]

    x = inp["x"].astype(F32)
    U = inp["U"].astype(F32)
    s_lam = np.exp(-float(inp["gamma"].reshape(-1)[0]) *
                   inp["Lambda"].astype(np.float64) ** 2).astype(F32)

    wqkv = inp["w_qkv"].astype(F32)
    bqkv = inp["b_qkv"].astype(F32)
    wq = (wqkv[:, :C] / np.sqrt(DH)).astype(F32)
    bq = (bqkv[:C] / np.sqrt(DH)).astype(F32)
    wk, bk = wqkv[:, C:2 * C].copy(), bqkv[C:2 * C]
    wv, bv = wqkv[:, 2 * C:].copy(), bqkv[2 * C:]
    b_out_p = (bv @ inp["w_out"] + inp["b_out"]).astype(F32)

    biasp = np.zeros((128, NBIAS), F32)
    for i, vec in [(BI_SPA1, inp["b_spa1"]), (BI_SPA2, inp["b_spa2"]),
                   (BI_SPE1, inp["b_spe1"]), (BI_SPE2, inp["b_spe2"]),
                   (BI_Q, bq), (BI_K, bk), (BI_OUTP, b_out_p),
                   (BI_CHEB, inp["cheb_b"]),
                   (BI_M1A, inp["mlp_b1"][:128]), (BI_M1B, inp["mlp_b1"][128:]),
                   (BI_M2, inp["mlp_b2"]),
                   (BI_BN1W, inp["bn1_w"]), (BI_BN1B, inp["bn1_b"]),
                   (BI_BN2W, inp["bn2_w"]), (BI_BN2B, inp["bn2_b"]),
                   (BI_BN3W, inp["bn3_w"]), (BI_BN3B, inp["bn3_b"]),
                   (BI_EPSC, np.full(128, EPS, F32))]:
        biasp[:, i] = vec.astype(F32)

    chebw_cols = np.concatenate([inp["cheb_w"][k].astype(F32) for k in range(K)],
                                axis=1)  # [128, 5*128]

    common = dict(
        wspa1=inp["w_spa1"].astype(BF16), wspa2=inp["w_spa2"].astype(BF16),
        wspe1=inp["w_spe1"].astype(BF16), wspe2=inp["w_spe2"].astype(BF16),
        wproj=inp["w_proj"].astype(BF16), chebw=chebw_cols.astype(BF16),
        wq=wq.astype(BF16), wk=wk.astype(BF16), wv=wv.astype(BF16),
        wout=inp["w_out"].astype(BF16),
        mw1=inp["mlp_w1"].astype(BF16),
        mw2a=inp["mlp_w2"][:128].astype(BF16), mw2b=inp["mlp_w2"][128:].astype(BF16),
        biasp=biasp,
        bqh=np.ascontiguousarray(bq.reshape(H, DH).T),
        bkh=np.ascontiguousarray(bk.astype(F32).reshape(H, DH).T),
        idm=np.eye(128, dtype=F32), idmb=np.eye(128, dtype=BF16),
        gmat=None, u1col=None, onesf=np.ones((128, 1), F32),
        one1=np.ones((1, 1), F32),
    )

    vs = VSCALE if FP8V else 1.0
    Us_full = (U * s_lam[None, :]).astype(F32)
    dis_cl = np.where(dis > 0, dis, 1.0).astype(F32)
    xfT_np = np.ascontiguousarray(x.T).astype(BF16)
    discf_np = np.ascontiguousarray((vs * dis_cl).reshape(N // 128, 128).T)
    gmat_np = (Us_full.T @ Us_full).astype(F32)
    u1_np = np.ascontiguousarray(Us_full.sum(0).astype(F32)[:, None])
    in_maps = []
    for c in range(NCORES):
        sl = slice(c * NLOC, (c + 1) * NLOC)
        dis_c = dis[sl]
        m = dict(common)
        m["xT"] = np.ascontiguousarray(x[sl].T).astype(BF16)
        m["u_nm"] = np.ascontiguousarray(U[sl]).astype(BF16)
        m["usT"] = np.ascontiguousarray((U[sl] * s_lam[None, :]).T).astype(BF16)
        m["usd"] = np.ascontiguousarray(
            Us_full[sl] / dis_cl[sl][:, None]).astype(BF16)
        m["gmat"] = gmat_np
        m["u1col"] = u1_np
        m["xfT"] = xfT_np
        m["discf"] = discf_np
        us_loc = Us_full[sl]
        m["m1x"] = np.ascontiguousarray(
            x[sl].T @ us_loc
            + np.outer(inp["cheb_b"].astype(F32), us_loc.sum(0)))
        m["disc"] = np.ascontiguousarray(
            (vs * dis_cl[sl]).reshape(NT, 128).T)
        m["disc_m"] = np.ascontiguousarray(
            (-scale / vs * dis_c).reshape(NT, 128).T)
        m["gidx"] = cores[c]["gidx"]
        m["s8"] = cores[c]["s8"]
        in_maps.append(m)

    import os
    global LAST_NC, LAST_IN_MAPS
    LAST_NC = nc
    LAST_IN_MAPS = in_maps
    trace = os.environ.get("KERNEL_TRACE", "0") == "1"
    res = run_bass_kernel_spmd(nc, in_maps, core_ids=list(range(NCORES)),
                               trace=trace)
    global LAST_EXEC_NS, LAST_RESULT
    LAST_EXEC_NS = res.exec_time_ns
    LAST_RESULT = res
    out = np.concatenate([res.results[c]["out_nm"] for c in range(NCORES)], axis=0)
    return out.astype(inp["x"].dtype)

